# revision 1
# baseline (speedup 1.0000x reference)
"""GAT (2-layer, 4-head, segment-softmax) message-passing kernel for 8 Trainium2
NeuronCores.

Strategy (dst-sharded, edge aggregation as one-hot matmuls):
  * Nodes are assigned to cores/groups with degree-balanced packing (LPT). The
    node permutation is defined as (core, group, slot) order, so each core owns
    a contiguous block of rows and each group's 128 nodes are contiguous.
  * Per layer, each core computes the full "record" table
    rec[n] = [xh(256) | a_src-score(4) | pad] ([N, 320]) with one matmul per
    128-node tile (replicated compute - cheap), plus had[n] = [h(64) | ad(4)].
  * For each destination group (128 nodes), the core gathers the records of
    the group's in-edges' source nodes with gpsimd dma_gather (int16 indices,
    source-bucketed in 32768-row windows), builds the one-hot incidence matrix
    M[edge, dst_slot] on the vector engine (iota compare), broadcasts the
    a_dst scores to edges via transposed-one-hot matmuls, and reduces both the
    softmax denominators and the weighted feature sums with PSUM-accumulated
    matmuls (contracting over edges).  Softmax normalization is applied after
    the reduction (denominator scaling on the dst side) - mathematically
    identical to the reference's segment softmax (max-subtraction is a no-op
    at these magnitudes; verified < 1e-6).
  * Head-mean + LayerNorm + ReLU + residual run on vector/scalar engines per
    group; staging writes are contiguous (the permutation IS group-slot
    order); an 8-core AllGather rebuilds the full h between the two layers.
"""

import os
import sys

sys.path.insert(0, "/opt/trn_rl_repo")

import numpy as np

# ---- problem constants (hardcoded; kernel.py must be self-contained) ----
N = 100000
E = 1600000
G = 64
H = 4
CDIM = 64
NODE_F = 32
DRONE_F = 16
OUT_F = 32
LN_EPS = 1e-5
NEG_SLOPE = 0.2
NCORES = 8
P = 128
HC = H * CDIM          # 256
REC = HC + H           # 260: [V(256) | as/ex(4)]
BUCKET = 32768         # int16 index range per dma_gather bucket
TB = 6                 # phase-1 tile batch

REC_DT_NAME = os.environ.get("GAT_REC_DT", "bfloat16")


class _Cfg:
    def __init__(self, n, ncores, cbs, rec_dt=REC_DT_NAME, debug=False):
        assert n % ncores == 0
        self.n = n
        self.ncores = ncores
        self.npc = n // ncores
        self.ngroup = -(-self.npc // P)
        self.cbs = cbs                       # [ngroup][nbuckets] chunk counts
        self.nbuckets = len(cbs[0])
        self.chg = [sum(row) for row in cbs]  # chunks per group
        self.chmax = max(self.chg)
        self.cols = sum(self.chg)            # total chunk columns
        self.rec_dt = rec_dt
        self.recp = 320 if rec_dt == "float32" else 384  # padded record elems
        self.debug = debug
        self.nt_full, self.nt_rem = divmod(n, P)
        self.last_cnt = self.npc - (self.ngroup - 1) * P


# --------------------------------------------------------------------------
# host-side preprocessing
# --------------------------------------------------------------------------

def _lpt(loads, caps):
    """LPT packing into len(caps) bins with given item capacities, balancing
    total load. Returns assignment array."""
    import heapq

    nbins = len(caps)
    order = np.argsort(-loads, kind="stable")
    heap = [(0, b) for b in range(nbins)]
    heapq.heapify(heap)
    cnt = np.zeros(nbins, np.int64)
    tot = np.zeros(nbins, np.int64)
    assign = np.empty(len(loads), np.int32)
    for i in order:
        while True:
            _, b = heapq.heappop(heap)
            if cnt[b] < caps[b]:
                break
        assign[i] = b
        cnt[b] += 1
        tot[b] += loads[i]
        if cnt[b] < caps[b]:
            heapq.heappush(heap, (int(tot[b]), b))
    return assign


def _host_prep(edge_index, n, ncores):
    """Node permutation + per-core gather index streams."""
    npc = n // ncores
    ngroup = -(-npc // P)
    last_cnt = npc - (ngroup - 1) * P
    nbuckets = -(-n // BUCKET)

    loop = np.arange(n, dtype=np.int64)
    src = np.concatenate([edge_index[0].astype(np.int64), loop])
    dst = np.concatenate([edge_index[1].astype(np.int64), loop])
    deg = np.bincount(dst, minlength=n)

    core_of = _lpt(deg, [npc] * ncores)
    group_of = np.empty(n, np.int32)
    slot_of = np.empty(n, np.int32)
    pos_of = np.empty(n, np.int64)
    order = np.empty(n, np.int64)
    caps = [P] * (ngroup - 1) + [last_cnt]
    for k in range(ncores):
        nodes_k = np.where(core_of == k)[0]
        g_assign = _lpt(deg[nodes_k], caps)
        o = np.argsort(g_assign, kind="stable")
        cnts = np.bincount(g_assign, minlength=ngroup)
        starts = np.concatenate([[0], np.cumsum(cnts)])[:-1]
        slot = np.empty(len(nodes_k), np.int64)
        slot[o] = np.arange(len(nodes_k)) - starts[g_assign[o]]
        group_of[nodes_k] = g_assign
        slot_of[nodes_k] = slot
        pos = k * npc + g_assign * P + slot
        pos_of[nodes_k] = pos
        order[pos] = nodes_k

    # per-(group,bucket) edge counts per core -> uniform chunk schedule
    e_core = core_of[dst]
    e_group = group_of[dst]
    e_bucket = pos_of[src] // BUCKET
    cnts = np.zeros((ncores, ngroup, nbuckets), np.int64)
    np.add.at(cnts, (e_core, e_group, e_bucket), 1)
    cbs_np = -(-cnts.max(axis=0) // P)       # [ngroup, nbuckets] chunks
    cbs = [[int(c) for c in row] for row in cbs_np]
    chg = np.array([sum(row) for row in cbs])
    cols = int(chg.sum())
    goff = np.concatenate([[0], np.cumsum(chg)])[:-1]
    boff = np.zeros((ngroup, nbuckets), np.int64)
    for g in range(ngroup):
        o = goff[g]
        for b in range(nbuckets):
            boff[g, b] = o
            o += cbs[g][b]

    per_core = []
    for k in range(ncores):
        mask = e_core == k
        es = pos_of[src[mask]]
        eg = e_group[mask]
        eb = e_bucket[mask]
        esl = slot_of[dst[mask]]
        o = np.lexsort((eb, eg))
        es, eg, eb, esl = es[o], eg[o], eb[o], esl[o]
        cnt_k = np.zeros((ngroup, nbuckets), np.int64)
        np.add.at(cnt_k, (eg, eb), 1)
        flat = cnt_k.reshape(-1)
        starts = np.concatenate([[0], np.cumsum(flat)])[:-1].reshape(
            ngroup, nbuckets)
        j = np.arange(len(es)) - starts[eg, eb]      # pos within (g,b)
        slotj = boff[eg, eb] * P + j                 # global slot in stream

        dstslot = np.full((P, cols), -1, np.int32)
        dstslot[slotj % P, slotj // P] = esl
        idx16 = np.zeros((16, cols * 8), np.int16)   # 8 int16 cols per chunk
        idx16[slotj % 16, slotj // 16] = es - eb * BUCKET
        idx16 = np.ascontiguousarray(np.tile(idx16, (8, 1)))

        gread = (k * npc + np.arange(ngroup)[None, :] * P
                 + np.arange(P)[:, None])
        gread = np.minimum(gread, (k + 1) * npc - 1).astype(np.int32)
        per_core.append(dict(dstslot=dstslot, idx16=idx16, gread=gread))
    return dict(order=order, pos_of=pos_of, cbs=cbs, per_core=per_core)


def _host_weights(inputs, order, n):
    """Permuted/augmented weight + input tensors (all float32)."""
    f = np.float32
    x = np.asarray(inputs["x"], f)[order]            # perm rows
    batch = np.asarray(inputs["batch"])[order]
    xTa = np.concatenate([x.T, np.ones((1, n), f)], 0)           # [33, n]
    oneT = (batch[None, :] == np.arange(G)[:, None]).astype(f)   # [G, n]
    droneTa = np.concatenate(
        [np.asarray(inputs["drone_feat"], f).T, np.ones((1, G), f)], 0)
    droneWa = np.concatenate(
        [np.asarray(inputs["drone_W"], f).T, np.asarray(inputs["drone_b"], f)[None]], 0)
    nodeWa = np.concatenate(
        [np.asarray(inputs["node_W"], f).T, np.asarray(inputs["node_b"], f)[None]], 0)
    out = dict(xTa=xTa, oneT=oneT, droneTa=droneTa, droneWa=droneWa,
               nodeWa=nodeWa,
               outWT=np.ascontiguousarray(np.asarray(inputs["out_W"], f).T),
               outb=np.tile(np.asarray(inputs["out_b"], f), (P, 1)))
    for l in range(2):
        W = np.asarray(inputs[f"convW{l}"], f)       # [HC, CDIM]
        a_s = np.asarray(inputs[f"att_src{l}"], f)   # [H, CDIM]
        a_d = np.asarray(inputs[f"att_dst{l}"], f)
        Wh = W.reshape(H, CDIM, CDIM)
        Ws = np.einsum("hcf,hc->fh", Wh, a_s)        # [CDIM, H]
        Wd = np.einsum("hcf,hc->fh", Wh, a_d)
        out[f"wcomb{l}"] = np.concatenate([W.T, Ws, Wd], 1)   # [CDIM, 264]
        out[f"convb{l}"] = np.tile(np.asarray(inputs[f"convb{l}"], f), (P, 1))
        out[f"lng{l}"] = np.tile(np.asarray(inputs[f"ln_g{l}"], f), (P, 1))
        out[f"lnb{l}"] = np.tile(np.asarray(inputs[f"ln_b{l}"], f), (P, 1))
    return out


# --------------------------------------------------------------------------
# bass kernel
# --------------------------------------------------------------------------

def _build(cfg):
    import concourse.bass as bass
    import concourse.bacc as bacc
    import concourse.tile as tile
    from concourse import mybir
    from concourse.masks import make_identity

    f32 = mybir.dt.float32
    i32 = mybir.dt.int32
    i16 = mybir.dt.int16
    rdt = getattr(mybir.dt, cfg.rec_dt)
    is_bf = cfg.rec_dt != "float32"
    Alu = mybir.AluOpType
    Act = mybir.ActivationFunctionType

    n, npc, ngroup = cfg.n, cfg.npc, cfg.ngroup
    RECP, CHMAX = cfg.recp, cfg.chmax

    nc = bacc.Bacc("TRN2", target_bir_lowering=False, debug=cfg.debug,
                   num_devices=cfg.ncores)

    def ein(nm, sh, dt=f32):
        return nc.dram_tensor(nm, sh, dt, kind="ExternalInput")

    xTa_d = ein("xTa", [NODE_F + 1, n])
    oneT_d = ein("oneT", [G, n])
    droneTa_d = ein("droneTa", [DRONE_F + 1, G])
    droneWa_d = ein("droneWa", [DRONE_F + 1, CDIM])
    nodeWa_d = ein("nodeWa", [NODE_F + 1, CDIM])
    wcomb_d = [ein(f"wcomb{l}", [CDIM, REC + H]) for l in range(2)]
    convb_d = [ein(f"convb{l}", [P, CDIM]) for l in range(2)]
    lng_d = [ein(f"lng{l}", [P, CDIM]) for l in range(2)]
    lnb_d = [ein(f"lnb{l}", [P, CDIM]) for l in range(2)]
    outWT_d = ein("outWT", [CDIM, OUT_F])
    outb_d = ein("outb", [P, OUT_F])
    dstslot_d = ein("dstslot", [P, cfg.cols], i32)
    idx16_d = ein("idx16", [P, cfg.cols * 8], i16)
    gread_d = ein("gread", [P, ngroup], i32)

    out_d = nc.dram_tensor("out", [npc, OUT_F], f32, kind="ExternalOutput")

    rec_d = nc.dram_tensor("rec", [n, RECP], rdt)
    had_d = [nc.dram_tensor(f"had{l}", [n, CDIM + H], f32) for l in range(2)]
    h1_d = nc.dram_tensor("h1", [n, CDIM], f32,
                          addr_space="Shared" if cfg.ncores > 4 else "Local")
    stag_d = [nc.dram_tensor(f"stag{l}", [ngroup * P, CDIM], f32)
              for l in range(2)]

    from contextlib import ExitStack
    with tile.TileContext(nc) as tc, ExitStack() as ctx:
        cpool = ctx.enter_context(tc.tile_pool(name="const", bufs=1))
        p1 = ctx.enter_context(tc.tile_pool(name="p1", bufs=2))
        p2 = ctx.enter_context(tc.tile_pool(name="p2", bufs=2))

        def cload(dram):
            t = cpool.tile(list(dram.shape), dram.dtype, tag=f"c_{dram.name}")
            nc.sync.dma_start(out=t[:], in_=dram[:])
            return t

        droneTa_sb = cload(droneTa_d)
        droneWa_sb = cload(droneWa_d)
        nodeWa_sb = cload(nodeWa_d)
        wcomb_sb = [cload(d) for d in wcomb_d]
        convb_sb = [cload(d) for d in convb_d]
        lng_sb = [cload(d) for d in lng_d]
        lnb_sb = [cload(d) for d in lnb_d]
        outWT_sb = cload(outWT_d)
        outb_sb = cload(outb_d)
        dstslot_sb = cload(dstslot_d)
        gread_sb = cload(gread_d)

        iota_sb = cpool.tile([P, P], i32)
        nc.gpsimd.iota(iota_sb[:], pattern=[[1, P]], base=0, channel_multiplier=0)
        ident_sb = cpool.tile([P, P], f32)
        make_identity(nc, ident_sb[:])
        identr_sb = ident_sb
        if is_bf:
            identr_sb = cpool.tile([P, P], rdt)
            nc.vector.tensor_copy(identr_sb[:], ident_sb[:])

        dr_sb = cpool.tile([G, CDIM], f32)
        with tc.tile_pool(name="psdr", bufs=1, space="PSUM") as ppdr:
            pdr_t = ppdr.tile([P, CDIM], f32)
            pdr = pdr_t[:G]
            nc.tensor.matmul(pdr, lhsT=droneTa_sb[:], rhs=droneWa_sb[:],
                             start=True, stop=True)
            nc.scalar.copy(dr_sb[:], pdr)

        # ------------------------------------------------------------------
        def phase1(l):
            """Build rec[n, RECP] and had[n, 68] tile by tile."""
            with tc.tile_pool(name=f"ps1_{l}", bufs=2, space="PSUM") as pp:

                def do_batch(r0, tb, rows):
                    if l == 0:
                        xb = p1.tile([NODE_F + 1, TB * P], f32, tag="xb")
                        nc.sync.dma_start(out=xb[:, :rows],
                                          in_=xTa_d[:, r0:r0 + rows])
                        ob = p1.tile([G, TB * P], f32, tag="ob")
                        nc.sync.dma_start(out=ob[:, :rows],
                                          in_=oneT_d[:, r0:r0 + rows])
                    hadb = p1.tile([P, TB, CDIM + H], f32, tag="hadb")
                    if l == 1:
                        if rows == tb * P:
                            nc.sync.dma_start(
                                out=hadb[:, :tb, :CDIM],
                                in_=h1_d[r0:r0 + rows, :].rearrange(
                                    "(c p) f -> p c f", p=P))
                        else:
                            nc.sync.dma_start(out=hadb[:rows, 0, :CDIM],
                                              in_=h1_d[r0:r0 + rows, :])
                    recb = p1.tile([P, TB, RECP], rdt, tag="recb")
                    nc.vector.memset(recb[:, :, REC:], 0.0)
                    for t in range(tb):
                        pr_ = min(P, rows - t * P)
                        if l == 0:
                            ph = pp.tile([P, CDIM], f32, tag="ph")
                            nc.tensor.matmul(ph[:pr_],
                                             lhsT=xb[:, t * P:t * P + pr_],
                                             rhs=nodeWa_sb[:], start=True,
                                             stop=False)
                            nc.tensor.matmul(ph[:pr_],
                                             lhsT=ob[:, t * P:t * P + pr_],
                                             rhs=dr_sb[:], start=False,
                                             stop=True)
                            nc.scalar.copy(hadb[:pr_, t, :CDIM], ph[:pr_])
                        pt = pp.tile([CDIM, P], f32, tag="pt")
                        nc.tensor.transpose(pt[:, :pr_], hadb[:pr_, t, :CDIM],
                                            ident_sb[:pr_, :pr_])
                        hT = p1.tile([CDIM, P], f32, tag="hT")
                        nc.scalar.copy(hT[:, :pr_], pt[:, :pr_])
                        prc = pp.tile([P, REC + H], f32, tag="pr")
                        nc.tensor.matmul(prc[:pr_], lhsT=hT[:, :pr_],
                                         rhs=wcomb_sb[l][:], start=True,
                                         stop=True)
                        nc.scalar.copy(recb[:pr_, t, 0:REC], prc[:pr_, 0:REC])
                        nc.vector.tensor_copy(hadb[:pr_, t, CDIM:],
                                              prc[:pr_, REC:REC + H])
                    if rows == tb * P:
                        nc.sync.dma_start(
                            out=rec_d[r0:r0 + rows, :].rearrange(
                                "(c p) f -> p c f", p=P),
                            in_=recb[:, :tb, :])
                        nc.sync.dma_start(
                            out=had_d[l][r0:r0 + rows, :].rearrange(
                                "(c p) f -> p c f", p=P),
                            in_=hadb[:, :tb, :])
                    else:
                        nc.sync.dma_start(out=rec_d[r0:r0 + rows, :],
                                          in_=recb[:rows, 0, :])
                        nc.sync.dma_start(out=had_d[l][r0:r0 + rows, :],
                                          in_=hadb[:rows, 0, :])

                for b0 in range(0, cfg.nt_full, TB):
                    tb = min(TB, cfg.nt_full - b0)
                    do_batch(b0 * P, tb, tb * P)
                if cfg.nt_rem:
                    do_batch(cfg.nt_full * P, 1, cfg.nt_rem)

        # ------------------------------------------------------------------
        def phase2(l):
            with tc.tile_pool(name=f"ps2_{l}", bufs=2, space="PSUM") as pp:
                col0 = 0
                for g in range(ngroup):
                    CH = cfg.chg[g]
                    rows_g = P if g < ngroup - 1 else cfg.last_cnt
                    idxt = p2.tile([P, CHMAX * 8], i16, tag="idxt")
                    nc.sync.dma_start(out=idxt[:, :CH * 8],
                                      in_=idx16_d[:, col0 * 8:(col0 + CH) * 8])
                    rect = p2.tile([P, CHMAX, RECP], rdt, tag="rect")
                    c0 = 0
                    for b in range(cfg.nbuckets):
                        cb = cfg.cbs[g][b]
                        if cb == 0:
                            continue
                        nrows = min(BUCKET, n - b * BUCKET)
                        done = 0
                        while done < cb:   # HW envelope: <=256 idxs per call
                            st = min(2, cb - done)
                            nc.gpsimd.dma_gather(
                                rect[:, c0 + done:c0 + done + st, :],
                                rec_d[b * BUCKET:b * BUCKET + nrows, :],
                                idxt[:, (c0 + done) * 8:(c0 + done + st) * 8],
                                st * P, st * P, RECP)
                            done += st
                        c0 += cb
                    # h_old + a_dst rows for this group's nodes
                    hadt = p2.tile([P, CDIM + H], f32, tag="hadt")
                    nc.gpsimd.indirect_dma_start(
                        out=hadt[:], out_offset=None, in_=had_d[l][:],
                        in_offset=bass.IndirectOffsetOnAxis(
                            ap=gread_sb[:, g:g + 1], axis=0))
                    ad_rhs = hadt[:, CDIM:]
                    if is_bf:
                        adr = p2.tile([P, H], rdt, tag="adr")
                        nc.vector.tensor_copy(adr[:], hadt[:, CDIM:])
                        ad_rhs = adr[:]
                    # one-hot M[edge, dst_slot]
                    Mt = p2.tile([P, CHMAX, P], rdt, tag="Mt")
                    nc.vector.tensor_tensor(
                        Mt[:, :CH, :],
                        dstslot_sb[:, col0:col0 + CH][:, :, None].to_broadcast(
                            [P, CH, P]),
                        iota_sb[:, None, :].to_broadcast([P, CH, P]),
                        Alu.is_equal)
                    # e_d: broadcast a_dst scores to edges via M^T matmuls
                    ped = pp.tile([P, CHMAX * H], f32, tag="ped")
                    for c in range(CH):
                        pmt = pp.tile([P, P], rdt, tag="pmt")
                        nc.tensor.transpose(pmt[:], Mt[:, c, :], identr_sb[:])
                        mt_sb = p2.tile([P, P], rdt, tag="mt_sb")
                        nc.scalar.copy(mt_sb[:], pmt[:])
                        nc.tensor.matmul(ped[:, c * H:(c + 1) * H],
                                         lhsT=mt_sb[:], rhs=ad_rhs,
                                         start=True, stop=True)
                    # e = lrelu(as + ad); ex = exp(e) -> rec[..., 256:260]
                    et = p2.tile([P, CHMAX, H], f32, tag="et")
                    nc.vector.tensor_tensor(
                        et[:, :CH, :], rect[:, :CH, HC:REC],
                        ped[:, 0:CH * H].rearrange("p (c h) -> p c h", h=H),
                        Alu.add)
                    lt = p2.tile([P, CHMAX, H], f32, tag="lt")
                    nc.vector.tensor_scalar_mul(lt[:, :CH, :], et[:, :CH, :],
                                                NEG_SLOPE)
                    nc.vector.tensor_tensor(et[:, :CH, :], lt[:, :CH, :],
                                            et[:, :CH, :], Alu.max)
                    nc.scalar.activation(rect[:, :CH, HC:REC], et[:, :CH, :],
                                         Act.Exp)
                    # V = ex * xh (per head, in place)
                    for h_ in range(H):
                        nc.vector.tensor_tensor(
                            rect[:, :CH, h_ * CDIM:(h_ + 1) * CDIM],
                            rect[:, :CH, h_ * CDIM:(h_ + 1) * CDIM],
                            rect[:, :CH, HC + h_:HC + h_ + 1].to_broadcast(
                                [P, CH, CDIM]),
                            Alu.mult)
                    # contract over edges: psum[:, 0:256]=sum alpha*xh, [256:260]=s
                    pg = pp.tile([P, REC], f32, tag="pg")
                    for c in range(CH):
                        nc.tensor.matmul(pg[:], lhsT=Mt[:, c, :],
                                         rhs=rect[:, c, 0:REC],
                                         start=(c == 0), stop=(c == CH - 1))
                    # r = 1 / (s + eps) / H
                    s4 = p2.tile([P, H], f32, tag="s4")
                    nc.vector.tensor_scalar(s4[:], pg[:, HC:REC], 1e-16, None,
                                            Alu.add)
                    r4 = p2.tile([P, H], f32, tag="r4")
                    nc.vector.reciprocal(r4[:], s4[:])
                    nc.vector.tensor_scalar_mul(r4[:], r4[:], 1.0 / H)
                    # head mean
                    yt = p2.tile([P, CDIM], f32, tag="yt")
                    tmp = p2.tile([P, CDIM], f32, tag="tmp")
                    nc.vector.tensor_scalar(yt[:], pg[:, 0:CDIM], r4[:, 0:1],
                                            None, Alu.mult)
                    for h_ in range(1, H):
                        nc.vector.tensor_scalar(tmp[:],
                                                pg[:, h_ * CDIM:(h_ + 1) * CDIM],
                                                r4[:, h_:h_ + 1], None, Alu.mult)
                        nc.vector.tensor_add(yt[:], yt[:], tmp[:])
                    nc.vector.tensor_add(yt[:], yt[:], convb_sb[l][:])
                    # layernorm
                    mu = p2.tile([P, 1], f32, tag="mu")
                    nc.vector.tensor_reduce(mu[:], yt[:], mybir.AxisListType.X,
                                            Alu.add)
                    nc.vector.tensor_scalar_mul(mu[:], mu[:], 1.0 / CDIM)
                    nc.vector.tensor_scalar(yt[:], yt[:], mu[:, 0:1], None,
                                            Alu.subtract)
                    sq = p2.tile([P, CDIM], f32, tag="sq")
                    var = p2.tile([P, 1], f32, tag="var")
                    nc.scalar.activation(sq[:], yt[:], Act.Square,
                                         accum_out=var[:])
                    nc.vector.tensor_scalar(var[:], var[:], 1.0 / CDIM, LN_EPS,
                                            Alu.mult, Alu.add)
                    sd = p2.tile([P, 1], f32, tag="sd")
                    nc.scalar.sqrt(sd[:], var[:])
                    inv = p2.tile([P, 1], f32, tag="inv")
                    nc.vector.reciprocal(inv[:], sd[:])
                    nc.vector.tensor_scalar(yt[:], yt[:], inv[:, 0:1], None,
                                            Alu.mult)
                    nc.vector.tensor_mul(yt[:], yt[:], lng_sb[l][:])
                    nc.vector.tensor_add(yt[:], yt[:], lnb_sb[l][:])
                    nc.vector.tensor_scalar_max(yt[:], yt[:], 0.0)
                    # residual + contiguous staging write
                    nc.vector.tensor_add(yt[:], yt[:], hadt[:, 0:CDIM])
                    nc.sync.dma_start(out=stag_d[l][g * P:g * P + rows_g, :],
                                      in_=yt[:rows_g, :])
                    col0 += CH

        # ------------------------------------------------------------------
        phase1(0)
        phase2(0)
        nc.gpsimd.collective_compute(
            "AllGather", mybir.AluOpType.bypass,
            replica_groups=[list(range(cfg.ncores))],
            ins=[stag_d[0][0:npc, :].opt()],
            outs=[h1_d[:, :].opt()])
        phase1(1)
        phase2(1)

        # final projection over own rows
        with tc.tile_pool(name="psf", bufs=2, space="PSUM") as pp:
            for t0 in range(0, npc, P):
                wr = min(P, npc - t0)
                ht2 = p2.tile([P, CDIM], f32, tag="ht2")
                nc.sync.dma_start(out=ht2[:wr], in_=stag_d[1][t0:t0 + wr, :])
                pt2 = pp.tile([CDIM, P], f32, tag="pt2")
                nc.tensor.transpose(pt2[:, :wr], ht2[:wr], ident_sb[:wr, :wr])
                hT2 = p2.tile([CDIM, P], f32, tag="hT2")
                nc.scalar.copy(hT2[:, :wr], pt2[:, :wr])
                po = pp.tile([P, OUT_F], f32, tag="po")
                nc.tensor.matmul(po[:wr], lhsT=hT2[:, :wr], rhs=outWT_sb[:],
                                 start=True, stop=True)
                ot = p2.tile([P, OUT_F], f32, tag="ot")
                nc.vector.tensor_add(ot[:wr], po[:wr], outb_sb[:wr])
                nc.sync.dma_start(out=out_d[t0:t0 + wr, :], in_=ot[:wr, :])

    nc.compile()
    return nc


# --------------------------------------------------------------------------
# entry point
# --------------------------------------------------------------------------

def _in_maps(cfg, prep, wts):
    shared = dict(xTa=wts["xTa"], oneT=wts["oneT"], droneTa=wts["droneTa"],
                  droneWa=wts["droneWa"], nodeWa=wts["nodeWa"],
                  outWT=wts["outWT"], outb=wts["outb"])
    for l in range(2):
        for nm in ("wcomb", "convb", "lng", "lnb"):
            shared[f"{nm}{l}"] = wts[f"{nm}{l}"]
    maps = []
    for k in range(cfg.ncores):
        m = dict(shared)
        m.update(prep["per_core"][k])
        maps.append({k_: np.ascontiguousarray(v) for k_, v in m.items()})
    return maps


def kernel(**inputs):
    edge_index = np.asarray(inputs["edge_index"])
    prep = _host_prep(edge_index, N, NCORES)
    cfg = _Cfg(N, NCORES, prep["cbs"])
    wts = _host_weights(inputs, prep["order"], N)
    nc = _build(cfg)
    maps = _in_maps(cfg, prep, wts)

    from concourse import bass_utils
    res = bass_utils.run_bass_kernel_spmd(nc, maps, core_ids=list(range(NCORES)))
    out = np.empty((N, OUT_F), np.float32)
    for k in range(NCORES):
        out[prep["order"][k * cfg.npc:(k + 1) * cfg.npc]] = res.results[k]["out"]
    return out



# revision 2
# speedup vs baseline: 2.8694x; 2.8694x over previous
"""GAT (2-layer, 4-head, segment-softmax) message-passing kernel for 8 Trainium2
NeuronCores.

Strategy (dst-sharded, edge aggregation as one-hot matmuls):
  * Nodes are assigned to cores/groups with degree-balanced packing (LPT). The
    node permutation is (core, group, slot) order, so each core owns a
    contiguous block of rows and each group's 128 nodes are contiguous.
  * Phase 1 is SHARDED: each core computes the record table
    rec[n] = [xh(256) | a_src-score(4) | pad] only for its own npc rows, plus
    had[n] = [h(64) | ad(4)]; an 8-core AllGather replicates rec on-device
    (NeuronLink) so phase 2 can gather any source node's record locally.
  * For each destination group (128 nodes), the core gathers the records of
    the group's in-edges' source nodes with gpsimd dma_gather (int16 indices,
    source-bucketed in 32768-row windows), builds the one-hot incidence matrix
    M[edge, dst_slot] on the vector engine (iota compare), broadcasts the
    a_dst scores to edges via transposed-one-hot matmuls, and reduces both the
    softmax denominators and the weighted feature sums with PSUM-accumulated
    matmuls (contracting over edges). Softmax normalization is applied after
    the reduction - mathematically identical to the reference's segment
    softmax (max-subtraction is a no-op at these magnitudes).
  * Host->device traffic is minimized (the axon tunnel is ~65 MB/s with ~75ms
    per-array overhead): all inputs are packed into two blobs per core
    (one f32 ~1.9MB with weights + own xT shard, one i16 ~1MB with gather
    indices / dst slots / batch ids). The drone-feature term is an on-device
    indirect gather of the 64x64 projected table instead of a shipped
    [64, n] one-hot; gread offsets are iota-generated on device.
"""

import os
import sys

sys.path.insert(0, "/opt/trn_rl_repo")

import numpy as np

# ---- problem constants (hardcoded; kernel.py must be self-contained) ----
N = 100000
E = 1600000
G = 64
H = 4
CDIM = 64
NODE_F = 32
DRONE_F = 16
OUT_F = 32
LN_EPS = 1e-5
NEG_SLOPE = 0.2
NCORES = 8
P = 128
HC = H * CDIM          # 256
REC = HC + H           # 260: [V(256) | as/ex(4)]
BUCKET = 32768         # int16 index range per dma_gather bucket
TB = 6                 # phase-1 tile batch

REC_DT_NAME = os.environ.get("GAT_REC_DT", "bfloat16")


class _Cfg:
    def __init__(self, n, ncores, cbs, rec_dt=REC_DT_NAME, debug=False):
        assert n % ncores == 0
        self.n = n
        self.ncores = ncores
        self.npc = n // ncores
        self.ngroup = -(-self.npc // P)
        self.cbs = cbs                       # [ngroup][nbuckets] chunk counts
        self.nbuckets = len(cbs[0])
        self.chg = [sum(row) for row in cbs]  # chunks per group
        self.chmax = max(self.chg)
        self.cols = sum(self.chg)            # total chunk columns
        self.rec_dt = rec_dt
        self.recp = 320 if rec_dt == "float32" else 384  # padded record elems
        self.debug = debug
        # own-shard tiling (phase 1 + final projection)
        self.nt_full, self.nt_rem = divmod(self.npc, P)
        self.last_cnt = self.npc - (self.ngroup - 1) * P


def _layout(cfg):
    """Blob layouts: name -> (offset, shape). Shared across host + device."""
    f32 = {}
    off = 0
    for nm, sh in [("nodeWa", (NODE_F + 1, CDIM)),
                   ("droneTa", (DRONE_F + 1, G)),
                   ("droneWa", (DRONE_F + 1, CDIM)),
                   ("wcomb0", (CDIM, REC + H)),
                   ("wcomb1", (CDIM, REC + H)),
                   ("outWT", (CDIM, OUT_F)),
                   ("convb0", (P, CDIM)), ("convb1", (P, CDIM)),
                   ("lng0", (P, CDIM)), ("lng1", (P, CDIM)),
                   ("lnb0", (P, CDIM)), ("lnb1", (P, CDIM)),
                   ("outb", (P, OUT_F)),
                   ("xT", (NODE_F + 1, cfg.npc))]:
        sz = sh[0] * sh[1]
        f32[nm] = (off, sh)
        off += sz
    f32["_total"] = off
    i16 = {}
    off = 0
    for nm, sh in [("idx16", (16, cfg.cols * 8)),
                   ("dstslot", (P, cfg.cols)),
                   ("batch", (P, cfg.ngroup))]:
        sz = sh[0] * sh[1]
        i16[nm] = (off, sh)
        off += sz
    i16["_total"] = off
    return f32, i16


# --------------------------------------------------------------------------
# host-side preprocessing
# --------------------------------------------------------------------------

def _lpt(loads, caps):
    """LPT packing into len(caps) bins with given item capacities, balancing
    total load. Returns assignment array."""
    import heapq

    nbins = len(caps)
    order = np.argsort(-loads, kind="stable")
    heap = [(0, b) for b in range(nbins)]
    heapq.heapify(heap)
    cnt = np.zeros(nbins, np.int64)
    tot = np.zeros(nbins, np.int64)
    assign = np.empty(len(loads), np.int32)
    for i in order:
        while True:
            _, b = heapq.heappop(heap)
            if cnt[b] < caps[b]:
                break
        assign[i] = b
        cnt[b] += 1
        tot[b] += loads[i]
        if cnt[b] < caps[b]:
            heapq.heappush(heap, (int(tot[b]), b))
    return assign


def _host_prep(edge_index, n, ncores):
    """Node permutation + per-core gather index streams."""
    npc = n // ncores
    ngroup = -(-npc // P)
    last_cnt = npc - (ngroup - 1) * P
    nbuckets = -(-n // BUCKET)

    loop = np.arange(n, dtype=np.int64)
    src = np.concatenate([edge_index[0].astype(np.int64), loop])
    dst = np.concatenate([edge_index[1].astype(np.int64), loop])
    deg = np.bincount(dst, minlength=n)

    core_of = _lpt(deg, [npc] * ncores)
    group_of = np.empty(n, np.int32)
    slot_of = np.empty(n, np.int32)
    pos_of = np.empty(n, np.int64)
    order = np.empty(n, np.int64)
    caps = [P] * (ngroup - 1) + [last_cnt]
    for k in range(ncores):
        nodes_k = np.where(core_of == k)[0]
        g_assign = _lpt(deg[nodes_k], caps)
        o = np.argsort(g_assign, kind="stable")
        cnts = np.bincount(g_assign, minlength=ngroup)
        starts = np.concatenate([[0], np.cumsum(cnts)])[:-1]
        slot = np.empty(len(nodes_k), np.int64)
        slot[o] = np.arange(len(nodes_k)) - starts[g_assign[o]]
        group_of[nodes_k] = g_assign
        slot_of[nodes_k] = slot
        pos = k * npc + g_assign * P + slot
        pos_of[nodes_k] = pos
        order[pos] = nodes_k

    # per-(group,bucket) edge counts per core -> uniform chunk schedule
    e_core = core_of[dst]
    e_group = group_of[dst]
    e_bucket = pos_of[src] // BUCKET
    cnts = np.zeros((ncores, ngroup, nbuckets), np.int64)
    np.add.at(cnts, (e_core, e_group, e_bucket), 1)
    cbs_np = -(-cnts.max(axis=0) // P)       # [ngroup, nbuckets] chunks
    cbs = [[int(c) for c in row] for row in cbs_np]
    chg = np.array([sum(row) for row in cbs])
    cols = int(chg.sum())
    goff = np.concatenate([[0], np.cumsum(chg)])[:-1]
    boff = np.zeros((ngroup, nbuckets), np.int64)
    for g in range(ngroup):
        o = goff[g]
        for b in range(nbuckets):
            boff[g, b] = o
            o += cbs[g][b]

    per_core = []
    for k in range(ncores):
        mask = e_core == k
        es = pos_of[src[mask]]
        eg = e_group[mask]
        eb = e_bucket[mask]
        esl = slot_of[dst[mask]]
        o = np.lexsort((eb, eg))
        es, eg, eb, esl = es[o], eg[o], eb[o], esl[o]
        cnt_k = np.zeros((ngroup, nbuckets), np.int64)
        np.add.at(cnt_k, (eg, eb), 1)
        flat = cnt_k.reshape(-1)
        starts = np.concatenate([[0], np.cumsum(flat)])[:-1].reshape(
            ngroup, nbuckets)
        j = np.arange(len(es)) - starts[eg, eb]      # pos within (g,b)
        slotj = boff[eg, eb] * P + j                 # global slot in stream

        dstslot = np.full((P, cols), -1, np.int16)
        dstslot[slotj % P, slotj // P] = esl
        idx16 = np.zeros((16, cols * 8), np.int16)   # 8 int16 cols per chunk
        idx16[slotj % 16, slotj // 16] = es - eb * BUCKET
        per_core.append(dict(dstslot=dstslot, idx16=idx16))
    return dict(order=order, pos_of=pos_of, cbs=cbs, per_core=per_core)


def _host_weights(inputs, order, n):
    """Permuted/augmented weight + input tensors (all float32)."""
    f = np.float32
    x = np.asarray(inputs["x"], f)[order]            # perm rows
    batch = np.asarray(inputs["batch"])[order]
    xTa = np.concatenate([x.T, np.ones((1, n), f)], 0)           # [33, n]
    droneTa = np.concatenate(
        [np.asarray(inputs["drone_feat"], f).T, np.ones((1, G), f)], 0)
    droneWa = np.concatenate(
        [np.asarray(inputs["drone_W"], f).T, np.asarray(inputs["drone_b"], f)[None]], 0)
    nodeWa = np.concatenate(
        [np.asarray(inputs["node_W"], f).T, np.asarray(inputs["node_b"], f)[None]], 0)
    out = dict(xTa=xTa, batch=batch, droneTa=droneTa, droneWa=droneWa,
               nodeWa=nodeWa,
               outWT=np.ascontiguousarray(np.asarray(inputs["out_W"], f).T),
               outb=np.tile(np.asarray(inputs["out_b"], f), (P, 1)))
    for l in range(2):
        W = np.asarray(inputs[f"convW{l}"], f)       # [HC, CDIM]
        a_s = np.asarray(inputs[f"att_src{l}"], f)   # [H, CDIM]
        a_d = np.asarray(inputs[f"att_dst{l}"], f)
        Wh = W.reshape(H, CDIM, CDIM)
        Ws = np.einsum("hcf,hc->fh", Wh, a_s)        # [CDIM, H]
        Wd = np.einsum("hcf,hc->fh", Wh, a_d)
        out[f"wcomb{l}"] = np.concatenate([W.T, Ws, Wd], 1)   # [CDIM, 264]
        out[f"convb{l}"] = np.tile(np.asarray(inputs[f"convb{l}"], f), (P, 1))
        out[f"lng{l}"] = np.tile(np.asarray(inputs[f"ln_g{l}"], f), (P, 1))
        out[f"lnb{l}"] = np.tile(np.asarray(inputs[f"ln_b{l}"], f), (P, 1))
    return out


# --------------------------------------------------------------------------
# bass kernel
# --------------------------------------------------------------------------

def _build(cfg):
    import concourse.bass as bass
    import concourse.bacc as bacc
    import concourse.tile as tile
    from concourse import mybir
    from concourse.masks import make_identity

    f32 = mybir.dt.float32
    i32 = mybir.dt.int32
    i16 = mybir.dt.int16
    rdt = getattr(mybir.dt, cfg.rec_dt)
    is_bf = cfg.rec_dt != "float32"
    Alu = mybir.AluOpType
    Act = mybir.ActivationFunctionType

    n, npc, ngroup = cfg.n, cfg.npc, cfg.ngroup
    RECP, CHMAX = cfg.recp, cfg.chmax
    LAYF, LAYI = _layout(cfg)

    nc = bacc.Bacc("TRN2", target_bir_lowering=False, debug=cfg.debug,
                   num_devices=cfg.ncores)

    blobf_d = nc.dram_tensor("blobf", [LAYF["_total"]], f32,
                             kind="ExternalInput")
    blobi_d = nc.dram_tensor("blobi", [LAYI["_total"]], i16,
                             kind="ExternalInput")

    def fview(nm):
        o, sh = LAYF[nm]
        return blobf_d[o:o + sh[0] * sh[1]].rearrange("(a b) -> a b", a=sh[0])

    def iview(nm):
        o, sh = LAYI[nm]
        return blobi_d[o:o + sh[0] * sh[1]].rearrange("(a b) -> a b", a=sh[0])

    out_d = nc.dram_tensor("out", [npc, OUT_F], f32, kind="ExternalOutput")

    rec_loc_d = nc.dram_tensor("rec_loc", [npc, RECP], rdt)
    rec_d = nc.dram_tensor("rec", [n, RECP], rdt,
                           addr_space="Shared" if cfg.ncores > 1 else "Local")
    had_d = [nc.dram_tensor(f"had{l}", [npc, CDIM + H], f32) for l in range(2)]
    stag_d = [nc.dram_tensor(f"stag{l}", [ngroup * P, CDIM], f32)
              for l in range(2)]
    idxrep_d = nc.dram_tensor("idxrep", [P, cfg.cols * 8], i16)
    dr_d = nc.dram_tensor("dr", [G, CDIM], f32)

    from contextlib import ExitStack
    with tile.TileContext(nc) as tc, ExitStack() as ctx:
        cpool = ctx.enter_context(tc.tile_pool(name="const", bufs=1))
        p1 = ctx.enter_context(tc.tile_pool(name="p1", bufs=2))
        p2 = ctx.enter_context(tc.tile_pool(name="p2", bufs=2))

        def cload(nm):
            o, sh = LAYF[nm]
            t = cpool.tile(list(sh), f32, tag=f"c_{nm}")
            nc.sync.dma_start(out=t[:], in_=fview(nm))
            return t

        droneTa_sb = cload("droneTa")
        droneWa_sb = cload("droneWa")
        nodeWa_sb = cload("nodeWa")
        wcomb_sb = [cload("wcomb0"), cload("wcomb1")]
        convb_sb = [cload("convb0"), cload("convb1")]
        lng_sb = [cload("lng0"), cload("lng1")]
        lnb_sb = [cload("lnb0"), cload("lnb1")]
        outWT_sb = cload("outWT")
        outb_sb = cload("outb")

        # int16 streams: dst slots + batch ids (+ iota / gread)
        dst16_sb = cpool.tile([P, cfg.cols], i16, tag="dst16")
        nc.sync.dma_start(out=dst16_sb[:], in_=iview("dstslot"))
        bat16_sb = cpool.tile([P, ngroup], i16, tag="bat16")
        nc.sync.dma_start(out=bat16_sb[:], in_=iview("batch"))
        bat32_sb = cpool.tile([P, ngroup], i32, tag="bat32")
        nc.vector.tensor_copy(bat32_sb[:], bat16_sb[:])

        # replicate the 16-partition gather-index stream to 128 partitions
        # (dma_gather wants idxs wrapped in 16 partitions x 8 gpsimd cores)
        for k8 in range(8):
            nc.sync.dma_start(out=idxrep_d[k8 * 16:(k8 + 1) * 16, :],
                              in_=iview("idx16"))

        iota_sb = cpool.tile([P, P], i32)
        nc.gpsimd.iota(iota_sb[:], pattern=[[1, P]], base=0,
                       channel_multiplier=0)
        iota16_sb = cpool.tile([P, P], i16)
        nc.vector.tensor_copy(iota16_sb[:], iota_sb[:])
        ident_sb = cpool.tile([P, P], f32)
        make_identity(nc, ident_sb[:])
        identr_sb = ident_sb
        if is_bf:
            identr_sb = cpool.tile([P, P], rdt)
            nc.vector.tensor_copy(identr_sb[:], ident_sb[:])

        # gread[p, g] = min(g*128 + p, npc-1): offsets into local had_d
        gread_sb = cpool.tile([P, ngroup], i32, tag="gread")
        nc.gpsimd.iota(gread_sb[:], pattern=[[P, ngroup]], base=0,
                       channel_multiplier=1)
        nc.vector.tensor_scalar(gread_sb[:], gread_sb[:], npc - 1, None,
                                Alu.min)

        # dr = droneTa.T @ droneWa  -> dram (indirect-gather source)
        dr_sb = cpool.tile([G, CDIM], f32)
        with tc.tile_pool(name="psdr", bufs=1, space="PSUM") as ppdr:
            pdr_t = ppdr.tile([P, CDIM], f32)
            pdr = pdr_t[:G]
            nc.tensor.matmul(pdr, lhsT=droneTa_sb[:], rhs=droneWa_sb[:],
                             start=True, stop=True)
            nc.scalar.copy(dr_sb[:], pdr)
        nc.sync.dma_start(out=dr_d[:, :], in_=dr_sb[:])

        # ------------------------------------------------------------------
        def phase1(l):
            """Build rec_loc[npc, RECP] and had[npc, 68] tile by tile
            (own shard only; AllGather replicates rec afterwards)."""
            xT_v = fview("xT")
            with tc.tile_pool(name=f"ps1_{l}", bufs=2, space="PSUM") as pp:

                def do_batch(r0, tb, rows):
                    if l == 0:
                        xb = p1.tile([NODE_F + 1, TB * P], f32, tag="xb")
                        nc.sync.dma_start(out=xb[:, :rows],
                                          in_=xT_v[:, r0:r0 + rows])
                    hadb = p1.tile([P, TB, CDIM + H], f32, tag="hadb")
                    if l == 1:
                        if rows == tb * P:
                            nc.sync.dma_start(
                                out=hadb[:, :tb, :CDIM],
                                in_=stag_d[0][r0:r0 + rows, :].rearrange(
                                    "(c p) f -> p c f", p=P))
                        else:
                            nc.sync.dma_start(out=hadb[:rows, 0, :CDIM],
                                              in_=stag_d[0][r0:r0 + rows, :])
                    recb = p1.tile([P, TB, RECP], rdt, tag="recb")
                    nc.vector.memset(recb[:, :, REC:], 0.0)
                    for t in range(tb):
                        pr_ = min(P, rows - t * P)
                        g_abs = r0 // P + t
                        if l == 0:
                            drt = p1.tile([P, CDIM], f32, tag="drt")
                            nc.gpsimd.indirect_dma_start(
                                out=drt[:], out_offset=None, in_=dr_d[:],
                                in_offset=bass.IndirectOffsetOnAxis(
                                    ap=bat32_sb[:, g_abs:g_abs + 1], axis=0))
                            ph = pp.tile([P, CDIM], f32, tag="ph")
                            nc.tensor.matmul(ph[:pr_],
                                             lhsT=xb[:, t * P:t * P + pr_],
                                             rhs=nodeWa_sb[:], start=True,
                                             stop=True)
                            nc.vector.tensor_tensor(hadb[:pr_, t, :CDIM],
                                                    ph[:pr_], drt[:pr_],
                                                    Alu.add)
                        pt = pp.tile([CDIM, P], f32, tag="pt")
                        nc.tensor.transpose(pt[:, :pr_], hadb[:pr_, t, :CDIM],
                                            ident_sb[:pr_, :pr_])
                        hT = p1.tile([CDIM, P], f32, tag="hT")
                        nc.scalar.copy(hT[:, :pr_], pt[:, :pr_])
                        prc = pp.tile([P, REC + H], f32, tag="pr")
                        nc.tensor.matmul(prc[:pr_], lhsT=hT[:, :pr_],
                                         rhs=wcomb_sb[l][:], start=True,
                                         stop=True)
                        nc.scalar.copy(recb[:pr_, t, 0:REC], prc[:pr_, 0:REC])
                        nc.vector.tensor_copy(hadb[:pr_, t, CDIM:],
                                              prc[:pr_, REC:REC + H])
                    if rows == tb * P:
                        nc.sync.dma_start(
                            out=rec_loc_d[r0:r0 + rows, :].rearrange(
                                "(c p) f -> p c f", p=P),
                            in_=recb[:, :tb, :])
                        nc.sync.dma_start(
                            out=had_d[l][r0:r0 + rows, :].rearrange(
                                "(c p) f -> p c f", p=P),
                            in_=hadb[:, :tb, :])
                    else:
                        nc.sync.dma_start(out=rec_loc_d[r0:r0 + rows, :],
                                          in_=recb[:rows, 0, :])
                        nc.sync.dma_start(out=had_d[l][r0:r0 + rows, :],
                                          in_=hadb[:rows, 0, :])

                for b0 in range(0, cfg.nt_full, TB):
                    tb = min(TB, cfg.nt_full - b0)
                    do_batch(b0 * P, tb, tb * P)
                if cfg.nt_rem:
                    do_batch(cfg.nt_full * P, 1, cfg.nt_rem)

        def gather_rec():
            nc.gpsimd.collective_compute(
                "AllGather", mybir.AluOpType.bypass,
                replica_groups=[list(range(cfg.ncores))],
                ins=[rec_loc_d[0:npc, :].opt()],
                outs=[rec_d[:, :].opt()])

        # ------------------------------------------------------------------
        def phase2(l):
            with tc.tile_pool(name=f"ps2_{l}", bufs=2, space="PSUM") as pp:
                col0 = 0
                for g in range(ngroup):
                    CH = cfg.chg[g]
                    rows_g = P if g < ngroup - 1 else cfg.last_cnt
                    idxt = p2.tile([P, CHMAX * 8], i16, tag="idxt")
                    nc.sync.dma_start(out=idxt[:, :CH * 8],
                                      in_=idxrep_d[:, col0 * 8:(col0 + CH) * 8])
                    rect = p2.tile([P, CHMAX, RECP], rdt, tag="rect")
                    c0 = 0
                    for b in range(cfg.nbuckets):
                        cb = cfg.cbs[g][b]
                        if cb == 0:
                            continue
                        nrows = min(BUCKET, n - b * BUCKET)
                        done = 0
                        while done < cb:   # HW envelope: <=256 idxs per call
                            st = min(2, cb - done)
                            nc.gpsimd.dma_gather(
                                rect[:, c0 + done:c0 + done + st, :],
                                rec_d[b * BUCKET:b * BUCKET + nrows, :],
                                idxt[:, (c0 + done) * 8:(c0 + done + st) * 8],
                                st * P, st * P, RECP)
                            done += st
                        c0 += cb
                    # h_old + a_dst rows for this group's nodes
                    hadt = p2.tile([P, CDIM + H], f32, tag="hadt")
                    nc.gpsimd.indirect_dma_start(
                        out=hadt[:], out_offset=None, in_=had_d[l][:],
                        in_offset=bass.IndirectOffsetOnAxis(
                            ap=gread_sb[:, g:g + 1], axis=0))
                    ad_rhs = hadt[:, CDIM:]
                    if is_bf:
                        adr = p2.tile([P, H], rdt, tag="adr")
                        nc.vector.tensor_copy(adr[:], hadt[:, CDIM:])
                        ad_rhs = adr[:]
                    # one-hot M[edge, dst_slot]
                    Mt = p2.tile([P, CHMAX, P], rdt, tag="Mt")
                    nc.vector.tensor_tensor(
                        Mt[:, :CH, :],
                        dst16_sb[:, col0:col0 + CH][:, :, None].to_broadcast(
                            [P, CH, P]),
                        iota16_sb[:, None, :].to_broadcast([P, CH, P]),
                        Alu.is_equal)
                    # e_d: broadcast a_dst scores to edges via M^T matmuls
                    ped = pp.tile([P, CHMAX * H], f32, tag="ped")
                    for c in range(CH):
                        pmt = pp.tile([P, P], rdt, tag="pmt")
                        nc.tensor.transpose(pmt[:], Mt[:, c, :], identr_sb[:])
                        mt_sb = p2.tile([P, P], rdt, tag="mt_sb")
                        nc.scalar.copy(mt_sb[:], pmt[:])
                        nc.tensor.matmul(ped[:, c * H:(c + 1) * H],
                                         lhsT=mt_sb[:], rhs=ad_rhs,
                                         start=True, stop=True)
                    # e = lrelu(as + ad); ex = exp(e) -> rec[..., 256:260]
                    et = p2.tile([P, CHMAX, H], f32, tag="et")
                    nc.vector.tensor_tensor(
                        et[:, :CH, :], rect[:, :CH, HC:REC],
                        ped[:, 0:CH * H].rearrange("p (c h) -> p c h", h=H),
                        Alu.add)
                    lt = p2.tile([P, CHMAX, H], f32, tag="lt")
                    nc.vector.tensor_scalar_mul(lt[:, :CH, :], et[:, :CH, :],
                                                NEG_SLOPE)
                    nc.vector.tensor_tensor(et[:, :CH, :], lt[:, :CH, :],
                                            et[:, :CH, :], Alu.max)
                    nc.scalar.activation(rect[:, :CH, HC:REC], et[:, :CH, :],
                                         Act.Exp)
                    # V = ex * xh (per head, in place)
                    for h_ in range(H):
                        nc.vector.tensor_tensor(
                            rect[:, :CH, h_ * CDIM:(h_ + 1) * CDIM],
                            rect[:, :CH, h_ * CDIM:(h_ + 1) * CDIM],
                            rect[:, :CH, HC + h_:HC + h_ + 1].to_broadcast(
                                [P, CH, CDIM]),
                            Alu.mult)
                    # contract over edges: psum[:, 0:256]=sum alpha*xh, [256:260]=s
                    pg = pp.tile([P, REC], f32, tag="pg")
                    for c in range(CH):
                        nc.tensor.matmul(pg[:], lhsT=Mt[:, c, :],
                                         rhs=rect[:, c, 0:REC],
                                         start=(c == 0), stop=(c == CH - 1))
                    # r = 1 / (s + eps) / H
                    s4 = p2.tile([P, H], f32, tag="s4")
                    nc.vector.tensor_scalar(s4[:], pg[:, HC:REC], 1e-16, None,
                                            Alu.add)
                    r4 = p2.tile([P, H], f32, tag="r4")
                    nc.vector.reciprocal(r4[:], s4[:])
                    nc.vector.tensor_scalar_mul(r4[:], r4[:], 1.0 / H)
                    # head mean
                    yt = p2.tile([P, CDIM], f32, tag="yt")
                    tmp = p2.tile([P, CDIM], f32, tag="tmp")
                    nc.vector.tensor_scalar(yt[:], pg[:, 0:CDIM], r4[:, 0:1],
                                            None, Alu.mult)
                    for h_ in range(1, H):
                        nc.vector.tensor_scalar(tmp[:],
                                                pg[:, h_ * CDIM:(h_ + 1) * CDIM],
                                                r4[:, h_:h_ + 1], None, Alu.mult)
                        nc.vector.tensor_add(yt[:], yt[:], tmp[:])
                    nc.vector.tensor_add(yt[:], yt[:], convb_sb[l][:])
                    # layernorm
                    mu = p2.tile([P, 1], f32, tag="mu")
                    nc.vector.tensor_reduce(mu[:], yt[:], mybir.AxisListType.X,
                                            Alu.add)
                    nc.vector.tensor_scalar_mul(mu[:], mu[:], 1.0 / CDIM)
                    nc.vector.tensor_scalar(yt[:], yt[:], mu[:, 0:1], None,
                                            Alu.subtract)
                    sq = p2.tile([P, CDIM], f32, tag="sq")
                    var = p2.tile([P, 1], f32, tag="var")
                    nc.scalar.activation(sq[:], yt[:], Act.Square,
                                         accum_out=var[:])
                    nc.vector.tensor_scalar(var[:], var[:], 1.0 / CDIM, LN_EPS,
                                            Alu.mult, Alu.add)
                    sd = p2.tile([P, 1], f32, tag="sd")
                    nc.scalar.sqrt(sd[:], var[:])
                    inv = p2.tile([P, 1], f32, tag="inv")
                    nc.vector.reciprocal(inv[:], sd[:])
                    nc.vector.tensor_scalar(yt[:], yt[:], inv[:, 0:1], None,
                                            Alu.mult)
                    nc.vector.tensor_mul(yt[:], yt[:], lng_sb[l][:])
                    nc.vector.tensor_add(yt[:], yt[:], lnb_sb[l][:])
                    nc.vector.tensor_scalar_max(yt[:], yt[:], 0.0)
                    # residual + contiguous staging write
                    nc.vector.tensor_add(yt[:], yt[:], hadt[:, 0:CDIM])
                    nc.sync.dma_start(out=stag_d[l][g * P:g * P + rows_g, :],
                                      in_=yt[:rows_g, :])
                    col0 += CH

        # ------------------------------------------------------------------
        phase1(0)
        gather_rec()
        phase2(0)
        phase1(1)
        gather_rec()
        phase2(1)

        # final projection over own rows
        with tc.tile_pool(name="psf", bufs=2, space="PSUM") as pp:
            for t0 in range(0, npc, P):
                wr = min(P, npc - t0)
                ht2 = p2.tile([P, CDIM], f32, tag="ht2")
                nc.sync.dma_start(out=ht2[:wr], in_=stag_d[1][t0:t0 + wr, :])
                pt2 = pp.tile([CDIM, P], f32, tag="pt2")
                nc.tensor.transpose(pt2[:, :wr], ht2[:wr], ident_sb[:wr, :wr])
                hT2 = p2.tile([CDIM, P], f32, tag="hT2")
                nc.scalar.copy(hT2[:, :wr], pt2[:, :wr])
                po = pp.tile([P, OUT_F], f32, tag="po")
                nc.tensor.matmul(po[:wr], lhsT=hT2[:, :wr], rhs=outWT_sb[:],
                                 start=True, stop=True)
                ot = p2.tile([P, OUT_F], f32, tag="ot")
                nc.vector.tensor_add(ot[:wr], po[:wr], outb_sb[:wr])
                nc.sync.dma_start(out=out_d[t0:t0 + wr, :], in_=ot[:wr, :])

    nc.compile()
    return nc


# --------------------------------------------------------------------------
# entry point
# --------------------------------------------------------------------------

def _in_maps(cfg, prep, wts):
    """Pack per-core inputs into one f32 + one i16 blob."""
    LAYF, LAYI = _layout(cfg)
    npc = cfg.npc

    blobf_shared = np.zeros(LAYF["_total"], np.float32)
    for nm in ("nodeWa", "droneTa", "droneWa", "wcomb0", "wcomb1", "outWT",
               "convb0", "convb1", "lng0", "lng1", "lnb0", "lnb1", "outb"):
        o, sh = LAYF[nm]
        blobf_shared[o:o + sh[0] * sh[1]] = np.asarray(wts[nm],
                                                       np.float32).ravel()

    batch = np.asarray(wts["batch"]).astype(np.int16)
    maps = []
    for k in range(cfg.ncores):
        blobf = blobf_shared.copy()
        o, sh = LAYF["xT"]
        blobf[o:o + sh[0] * sh[1]] = np.ascontiguousarray(
            wts["xTa"][:, k * npc:(k + 1) * npc]).ravel()

        pc = prep["per_core"][k]
        blobi = np.zeros(LAYI["_total"], np.int16)
        o, sh = LAYI["idx16"]
        blobi[o:o + sh[0] * sh[1]] = pc["idx16"].ravel()
        o, sh = LAYI["dstslot"]
        blobi[o:o + sh[0] * sh[1]] = pc["dstslot"].ravel()
        o, sh = LAYI["batch"]
        bp = np.zeros(cfg.ngroup * P, np.int16)
        bp[:npc] = batch[k * npc:(k + 1) * npc]
        # [p, g] layout = transpose of [g, p] row blocks
        blobi[o:o + sh[0] * sh[1]] = np.ascontiguousarray(
            bp.reshape(cfg.ngroup, P).T).ravel()
        maps.append(dict(blobf=blobf, blobi=blobi))
    return maps


def kernel(**inputs):
    edge_index = np.asarray(inputs["edge_index"])
    prep = _host_prep(edge_index, N, NCORES)
    cfg = _Cfg(N, NCORES, prep["cbs"])
    wts = _host_weights(inputs, prep["order"], N)
    nc = _build(cfg)
    maps = _in_maps(cfg, prep, wts)

    from concourse import bass_utils
    res = bass_utils.run_bass_kernel_spmd(nc, maps, core_ids=list(range(NCORES)))
    out = np.empty((N, OUT_F), np.float32)
    for k in range(NCORES):
        out[prep["order"][k * cfg.npc:(k + 1) * cfg.npc]] = res.results[k]["out"]
    return out


# revision 3
# speedup vs baseline: 24.5878x; 8.5691x over previous
"""GAT (2-layer, 4-head, segment-softmax) message-passing kernel for 8 Trainium2
NeuronCores.

Strategy (dst-sharded, edge aggregation as one-hot matmuls):
  * Nodes are assigned to cores/groups with degree-balanced packing (LPT). The
    node permutation is (core, group, slot) order, so each core owns a
    contiguous block of rows and each group's 128 nodes are contiguous.
  * Phase 1 is SHARDED: each core computes the record table
    rec[n] = [xh(256) | a_src-score(4) | pad] only for its own npc rows, plus
    had[n] = [h(64) | ad(4)]; an 8-core AllGather replicates rec on-device
    (NeuronLink) so phase 2 can gather any source node's record locally.
  * For each destination group (128 nodes), the core gathers the records of
    the group's in-edges' source nodes with gpsimd dma_gather (int16 indices,
    source-bucketed in 32768-row windows), builds the one-hot incidence matrix
    M[edge, dst_slot] on the vector engine (iota compare), broadcasts the
    a_dst scores to edges via transposed-one-hot matmuls, and reduces both the
    softmax denominators and the weighted feature sums with PSUM-accumulated
    matmuls (contracting over edges). Softmax normalization is applied after
    the reduction - mathematically identical to the reference's segment
    softmax (max-subtraction is a no-op at these magnitudes).
  * Host->device traffic is minimized (the axon tunnel is ~65 MB/s with ~75ms
    per-array overhead): ALL inputs are packed into ONE f32 blob per core
    (~2.2MB: f32 weights; int16 gather indices / dst slots / batch ids and
    the bf16 own-shard xT are stored via bitcast views). The drone-feature
    term is an on-device indirect gather of the 64x64 projected table; gread
    offsets are iota-generated on device; the output is returned as bf16.
  * Dispatch uses a cached jitted shard_map executable (compiled once per
    process) plus the JAX persistent compilation cache, so steady-state
    dispatch cost is input upload + execute + output download.
"""

import os
import sys

sys.path.insert(0, "/opt/trn_rl_repo")

import numpy as np

# ---- problem constants (hardcoded; kernel.py must be self-contained) ----
N = 100000
E = 1600000
G = 64
H = 4
CDIM = 64
NODE_F = 32
DRONE_F = 16
OUT_F = 32
LN_EPS = 1e-5
NEG_SLOPE = 0.2
NCORES = 8
P = 128
HC = H * CDIM          # 256
REC = HC + H           # 260: [V(256) | as/ex(4)]
BUCKET = 32768         # int16 index range per dma_gather bucket
TB = 6                 # phase-1 tile batch

REC_DT_NAME = os.environ.get("GAT_REC_DT", "bfloat16")


def _enable_jax_cc():
    import jax
    try:
        jax.config.update("jax_compilation_cache_dir",
                          os.environ.get("JAX_CC_DIR", "/tmp/jax_cc_cache"))
        jax.config.update("jax_persistent_cache_min_entry_size_bytes", 0)
        jax.config.update("jax_persistent_cache_min_compile_time_secs", 0.0)
    except Exception:
        pass


class _Cfg:
    def __init__(self, n, ncores, cbs, rec_dt=REC_DT_NAME, debug=False):
        assert n % ncores == 0
        self.n = n
        self.ncores = ncores
        self.npc = n // ncores
        self.ngroup = -(-self.npc // P)
        self.cbs = cbs                       # [ngroup][nbuckets] chunk counts
        self.nbuckets = len(cbs[0])
        self.chg = [sum(row) for row in cbs]  # chunks per group
        self.chmax = max(self.chg)
        self.cols = sum(self.chg)            # total chunk columns
        self.rec_dt = rec_dt
        self.recp = 320 if rec_dt == "float32" else 384  # padded record elems
        self.debug = debug
        # own-shard tiling (phase 1 + final projection)
        self.nt_full, self.nt_rem = divmod(self.npc, P)
        self.last_cnt = self.npc - (self.ngroup - 1) * P


def _layout(cfg):
    """Single-blob layout. Returns (f32 sections, i16 sections, total f32
    elems). i16 section offsets are in int16 units from the start of the
    int16 region, which begins at f32 elem F32SZ (i16 elem 2*F32SZ)."""
    f32 = {}
    off = 0
    for nm, sh in [("nodeWa", (NODE_F + 1, CDIM)),
                   ("droneTa", (DRONE_F + 1, G)),
                   ("droneWa", (DRONE_F + 1, CDIM)),
                   ("wcomb0", (CDIM, REC + H)),
                   ("wcomb1", (CDIM, REC + H)),
                   ("outWT", (CDIM, OUT_F)),
                   ("convb0", (P, CDIM)), ("convb1", (P, CDIM)),
                   ("lng0", (P, CDIM)), ("lng1", (P, CDIM)),
                   ("lnb0", (P, CDIM)), ("lnb1", (P, CDIM)),
                   ("outb", (P, OUT_F))]:
        f32[nm] = (off, sh)
        off += sh[0] * sh[1]
    f32sz = off
    i16 = {}
    off = 0
    for nm, sh in [("idx16", (16, cfg.cols * 8)),
                   ("dstslot", (P, cfg.cols)),
                   ("batch", (P, cfg.ngroup)),
                   ("xTbf", (NODE_F + 1, cfg.npc))]:
        sz = sh[0] * sh[1]
        i16[nm] = (off, sh)
        off += sz + (sz & 1)                 # keep 32-bit alignment
    total = f32sz + (off + 1) // 2
    return f32, i16, f32sz, total


# --------------------------------------------------------------------------
# host-side preprocessing
# --------------------------------------------------------------------------

def _lpt(loads, caps):
    """LPT packing into len(caps) bins with given item capacities, balancing
    total load. Returns assignment array."""
    import heapq

    nbins = len(caps)
    order = np.argsort(-loads, kind="stable")
    heap = [(0, b) for b in range(nbins)]
    heapq.heapify(heap)
    cnt = np.zeros(nbins, np.int64)
    tot = np.zeros(nbins, np.int64)
    assign = np.empty(len(loads), np.int32)
    for i in order:
        while True:
            _, b = heapq.heappop(heap)
            if cnt[b] < caps[b]:
                break
        assign[i] = b
        cnt[b] += 1
        tot[b] += loads[i]
        if cnt[b] < caps[b]:
            heapq.heappush(heap, (int(tot[b]), b))
    return assign


def _host_prep(edge_index, n, ncores):
    """Node permutation + per-core gather index streams."""
    npc = n // ncores
    ngroup = -(-npc // P)
    last_cnt = npc - (ngroup - 1) * P
    nbuckets = -(-n // BUCKET)

    loop = np.arange(n, dtype=np.int64)
    src = np.concatenate([edge_index[0].astype(np.int64), loop])
    dst = np.concatenate([edge_index[1].astype(np.int64), loop])
    deg = np.bincount(dst, minlength=n)

    core_of = _lpt(deg, [npc] * ncores)
    group_of = np.empty(n, np.int32)
    slot_of = np.empty(n, np.int32)
    pos_of = np.empty(n, np.int64)
    order = np.empty(n, np.int64)
    caps = [P] * (ngroup - 1) + [last_cnt]
    for k in range(ncores):
        nodes_k = np.where(core_of == k)[0]
        g_assign = _lpt(deg[nodes_k], caps)
        o = np.argsort(g_assign, kind="stable")
        cnts = np.bincount(g_assign, minlength=ngroup)
        starts = np.concatenate([[0], np.cumsum(cnts)])[:-1]
        slot = np.empty(len(nodes_k), np.int64)
        slot[o] = np.arange(len(nodes_k)) - starts[g_assign[o]]
        group_of[nodes_k] = g_assign
        slot_of[nodes_k] = slot
        pos = k * npc + g_assign * P + slot
        pos_of[nodes_k] = pos
        order[pos] = nodes_k

    # per-(group,bucket) edge counts per core -> uniform chunk schedule
    e_core = core_of[dst]
    e_group = group_of[dst]
    e_bucket = pos_of[src] // BUCKET
    cnts = np.zeros((ncores, ngroup, nbuckets), np.int64)
    np.add.at(cnts, (e_core, e_group, e_bucket), 1)
    cbs_np = -(-cnts.max(axis=0) // P)       # [ngroup, nbuckets] chunks
    cbs = [[int(c) for c in row] for row in cbs_np]
    chg = np.array([sum(row) for row in cbs])
    cols = int(chg.sum())
    goff = np.concatenate([[0], np.cumsum(chg)])[:-1]
    boff = np.zeros((ngroup, nbuckets), np.int64)
    for g in range(ngroup):
        o = goff[g]
        for b in range(nbuckets):
            boff[g, b] = o
            o += cbs[g][b]

    per_core = []
    for k in range(ncores):
        mask = e_core == k
        es = pos_of[src[mask]]
        eg = e_group[mask]
        eb = e_bucket[mask]
        esl = slot_of[dst[mask]]
        o = np.lexsort((eb, eg))
        es, eg, eb, esl = es[o], eg[o], eb[o], esl[o]
        cnt_k = np.zeros((ngroup, nbuckets), np.int64)
        np.add.at(cnt_k, (eg, eb), 1)
        flat = cnt_k.reshape(-1)
        starts = np.concatenate([[0], np.cumsum(flat)])[:-1].reshape(
            ngroup, nbuckets)
        j = np.arange(len(es)) - starts[eg, eb]      # pos within (g,b)
        slotj = boff[eg, eb] * P + j                 # global slot in stream

        dstslot = np.full((P, cols), -1, np.int16)
        dstslot[slotj % P, slotj // P] = esl
        idx16 = np.zeros((16, cols * 8), np.int16)   # 8 int16 cols per chunk
        idx16[slotj % 16, slotj // 16] = es - eb * BUCKET
        per_core.append(dict(dstslot=dstslot, idx16=idx16))
    return dict(order=order, pos_of=pos_of, cbs=cbs, per_core=per_core)


def _host_weights(inputs, order, n):
    """Permuted/augmented weight + input tensors (all float32)."""
    f = np.float32
    x = np.asarray(inputs["x"], f)[order]            # perm rows
    batch = np.asarray(inputs["batch"])[order]
    xTa = np.concatenate([x.T, np.ones((1, n), f)], 0)           # [33, n]
    droneTa = np.concatenate(
        [np.asarray(inputs["drone_feat"], f).T, np.ones((1, G), f)], 0)
    droneWa = np.concatenate(
        [np.asarray(inputs["drone_W"], f).T, np.asarray(inputs["drone_b"], f)[None]], 0)
    nodeWa = np.concatenate(
        [np.asarray(inputs["node_W"], f).T, np.asarray(inputs["node_b"], f)[None]], 0)
    out = dict(xTa=xTa, batch=batch, droneTa=droneTa, droneWa=droneWa,
               nodeWa=nodeWa,
               outWT=np.ascontiguousarray(np.asarray(inputs["out_W"], f).T),
               outb=np.tile(np.asarray(inputs["out_b"], f), (P, 1)))
    for l in range(2):
        W = np.asarray(inputs[f"convW{l}"], f)       # [HC, CDIM]
        a_s = np.asarray(inputs[f"att_src{l}"], f)   # [H, CDIM]
        a_d = np.asarray(inputs[f"att_dst{l}"], f)
        Wh = W.reshape(H, CDIM, CDIM)
        Ws = np.einsum("hcf,hc->fh", Wh, a_s)        # [CDIM, H]
        Wd = np.einsum("hcf,hc->fh", Wh, a_d)
        out[f"wcomb{l}"] = np.concatenate([W.T, Ws, Wd], 1)   # [CDIM, 264]
        out[f"convb{l}"] = np.tile(np.asarray(inputs[f"convb{l}"], f), (P, 1))
        out[f"lng{l}"] = np.tile(np.asarray(inputs[f"ln_g{l}"], f), (P, 1))
        out[f"lnb{l}"] = np.tile(np.asarray(inputs[f"ln_b{l}"], f), (P, 1))
    return out


# --------------------------------------------------------------------------
# bass kernel
# --------------------------------------------------------------------------

def _build(cfg):
    import concourse.bass as bass
    import concourse.bacc as bacc
    import concourse.tile as tile
    from concourse import mybir
    from concourse.masks import make_identity

    f32 = mybir.dt.float32
    i32 = mybir.dt.int32
    i16 = mybir.dt.int16
    bf16 = mybir.dt.bfloat16
    rdt = getattr(mybir.dt, cfg.rec_dt)
    is_bf = cfg.rec_dt != "float32"
    Alu = mybir.AluOpType
    Act = mybir.ActivationFunctionType

    n, npc, ngroup = cfg.n, cfg.npc, cfg.ngroup
    RECP, CHMAX = cfg.recp, cfg.chmax
    LAYF, LAYI, F32SZ, TOTAL = _layout(cfg)

    nc = bacc.Bacc("TRN2", target_bir_lowering=False, debug=cfg.debug,
                   num_devices=cfg.ncores)

    blob_d = nc.dram_tensor("blob", [TOTAL], f32, kind="ExternalInput")

    def fview(nm):
        o, sh = LAYF[nm]
        return blob_d[o:o + sh[0] * sh[1]].rearrange("(a b) -> a b", a=sh[0])

    def iview(nm, dt):
        o, sh = LAYI[nm]
        sz = sh[0] * sh[1]
        o32 = F32SZ + o // 2                 # o is even by construction
        return blob_d[o32:o32 + (sz + 1) // 2].bitcast(dt)[
            0:sz].rearrange("(a b) -> a b", a=sh[0])

    out_d = nc.dram_tensor("out", [npc, OUT_F], bf16, kind="ExternalOutput")

    rec_loc_d = nc.dram_tensor("rec_loc", [npc, RECP], rdt)
    rec_d = nc.dram_tensor("rec", [n, RECP], rdt,
                           addr_space="Shared" if cfg.ncores > 1 else "Local")
    had_d = [nc.dram_tensor(f"had{l}", [npc, CDIM + H], f32) for l in range(2)]
    stag_d = [nc.dram_tensor(f"stag{l}", [ngroup * P, CDIM], f32)
              for l in range(2)]
    idxrep_d = nc.dram_tensor("idxrep", [P, cfg.cols * 8], i16)
    dr_d = nc.dram_tensor("dr", [G, CDIM], f32)

    from contextlib import ExitStack
    with tile.TileContext(nc) as tc, ExitStack() as ctx:
        cpool = ctx.enter_context(tc.tile_pool(name="const", bufs=1))
        p1 = ctx.enter_context(tc.tile_pool(name="p1", bufs=2))
        p2 = ctx.enter_context(tc.tile_pool(name="p2", bufs=2))

        def cload(nm):
            o, sh = LAYF[nm]
            t = cpool.tile(list(sh), f32, tag=f"c_{nm}")
            nc.sync.dma_start(out=t[:], in_=fview(nm))
            return t

        droneTa_sb = cload("droneTa")
        droneWa_sb = cload("droneWa")
        nodeWa_sb = cload("nodeWa")
        wcomb_sb = [cload("wcomb0"), cload("wcomb1")]
        convb_sb = [cload("convb0"), cload("convb1")]
        lng_sb = [cload("lng0"), cload("lng1")]
        lnb_sb = [cload("lnb0"), cload("lnb1")]
        outWT_sb = cload("outWT")
        outb_sb = cload("outb")
        nodeWb_sb = cpool.tile([NODE_F + 1, CDIM], bf16, tag="nodeWb")
        nc.vector.tensor_copy(nodeWb_sb[:], nodeWa_sb[:])

        # int16 streams: dst slots + batch ids (+ iota / gread)
        dst16_sb = cpool.tile([P, cfg.cols], i16, tag="dst16")
        nc.sync.dma_start(out=dst16_sb[:], in_=iview("dstslot", i16))
        bat16_sb = cpool.tile([P, ngroup], i16, tag="bat16")
        nc.sync.dma_start(out=bat16_sb[:], in_=iview("batch", i16))
        bat32_sb = cpool.tile([P, ngroup], i32, tag="bat32")
        nc.vector.tensor_copy(bat32_sb[:], bat16_sb[:])

        # replicate the 16-partition gather-index stream to 128 partitions
        # (dma_gather wants idxs wrapped in 16 partitions x 8 gpsimd cores)
        for k8 in range(8):
            nc.sync.dma_start(out=idxrep_d[k8 * 16:(k8 + 1) * 16, :],
                              in_=iview("idx16", i16))

        iota_sb = cpool.tile([P, P], i32)
        nc.gpsimd.iota(iota_sb[:], pattern=[[1, P]], base=0,
                       channel_multiplier=0)
        iota16_sb = cpool.tile([P, P], i16)
        nc.vector.tensor_copy(iota16_sb[:], iota_sb[:])
        ident_sb = cpool.tile([P, P], f32)
        make_identity(nc, ident_sb[:])
        identr_sb = ident_sb
        if is_bf:
            identr_sb = cpool.tile([P, P], rdt)
            nc.vector.tensor_copy(identr_sb[:], ident_sb[:])

        # gread[p, g] = min(g*128 + p, npc-1): offsets into local had_d
        gread_sb = cpool.tile([P, ngroup], i32, tag="gread")
        nc.gpsimd.iota(gread_sb[:], pattern=[[P, ngroup]], base=0,
                       channel_multiplier=1)
        nc.vector.tensor_scalar(gread_sb[:], gread_sb[:], npc - 1, None,
                                Alu.min)

        # dr = droneTa.T @ droneWa  -> dram (indirect-gather source)
        dr_sb = cpool.tile([G, CDIM], f32)
        with tc.tile_pool(name="psdr", bufs=1, space="PSUM") as ppdr:
            pdr_t = ppdr.tile([P, CDIM], f32)
            pdr = pdr_t[:G]
            nc.tensor.matmul(pdr, lhsT=droneTa_sb[:], rhs=droneWa_sb[:],
                             start=True, stop=True)
            nc.scalar.copy(dr_sb[:], pdr)
        nc.sync.dma_start(out=dr_d[:, :], in_=dr_sb[:])

        # ------------------------------------------------------------------
        def phase1(l):
            """Build rec_loc[npc, RECP] and had[npc, 68] tile by tile
            (own shard only; AllGather replicates rec afterwards)."""
            xT_v = iview("xTbf", bf16)
            with tc.tile_pool(name=f"ps1_{l}", bufs=2, space="PSUM") as pp:

                def do_batch(r0, tb, rows):
                    if l == 0:
                        xb = p1.tile([NODE_F + 1, TB * P], bf16, tag="xb")
                        nc.sync.dma_start(out=xb[:, :rows],
                                          in_=xT_v[:, r0:r0 + rows])
                    hadb = p1.tile([P, TB, CDIM + H], f32, tag="hadb")
                    if l == 1:
                        if rows == tb * P:
                            nc.sync.dma_start(
                                out=hadb[:, :tb, :CDIM],
                                in_=stag_d[0][r0:r0 + rows, :].rearrange(
                                    "(c p) f -> p c f", p=P))
                        else:
                            nc.sync.dma_start(out=hadb[:rows, 0, :CDIM],
                                              in_=stag_d[0][r0:r0 + rows, :])
                    recb = p1.tile([P, TB, RECP], rdt, tag="recb")
                    nc.vector.memset(recb[:, :, REC:], 0.0)
                    for t in range(tb):
                        pr_ = min(P, rows - t * P)
                        g_abs = r0 // P + t
                        if l == 0:
                            drt = p1.tile([P, CDIM], f32, tag="drt")
                            nc.gpsimd.indirect_dma_start(
                                out=drt[:], out_offset=None, in_=dr_d[:],
                                in_offset=bass.IndirectOffsetOnAxis(
                                    ap=bat32_sb[:, g_abs:g_abs + 1], axis=0))
                            ph = pp.tile([P, CDIM], f32, tag="ph")
                            nc.tensor.matmul(ph[:pr_],
                                             lhsT=xb[:, t * P:t * P + pr_],
                                             rhs=nodeWb_sb[:], start=True,
                                             stop=True)
                            nc.vector.tensor_tensor(hadb[:pr_, t, :CDIM],
                                                    ph[:pr_], drt[:pr_],
                                                    Alu.add)
                        pt = pp.tile([CDIM, P], f32, tag="pt")
                        nc.tensor.transpose(pt[:, :pr_], hadb[:pr_, t, :CDIM],
                                            ident_sb[:pr_, :pr_])
                        hT = p1.tile([CDIM, P], f32, tag="hT")
                        nc.scalar.copy(hT[:, :pr_], pt[:, :pr_])
                        prc = pp.tile([P, REC + H], f32, tag="pr")
                        nc.tensor.matmul(prc[:pr_], lhsT=hT[:, :pr_],
                                         rhs=wcomb_sb[l][:], start=True,
                                         stop=True)
                        nc.scalar.copy(recb[:pr_, t, 0:REC], prc[:pr_, 0:REC])
                        nc.vector.tensor_copy(hadb[:pr_, t, CDIM:],
                                              prc[:pr_, REC:REC + H])
                    if rows == tb * P:
                        nc.sync.dma_start(
                            out=rec_loc_d[r0:r0 + rows, :].rearrange(
                                "(c p) f -> p c f", p=P),
                            in_=recb[:, :tb, :])
                        nc.sync.dma_start(
                            out=had_d[l][r0:r0 + rows, :].rearrange(
                                "(c p) f -> p c f", p=P),
                            in_=hadb[:, :tb, :])
                    else:
                        nc.sync.dma_start(out=rec_loc_d[r0:r0 + rows, :],
                                          in_=recb[:rows, 0, :])
                        nc.sync.dma_start(out=had_d[l][r0:r0 + rows, :],
                                          in_=hadb[:rows, 0, :])

                for b0 in range(0, cfg.nt_full, TB):
                    tb = min(TB, cfg.nt_full - b0)
                    do_batch(b0 * P, tb, tb * P)
                if cfg.nt_rem:
                    do_batch(cfg.nt_full * P, 1, cfg.nt_rem)

        def gather_rec():
            nc.gpsimd.collective_compute(
                "AllGather", mybir.AluOpType.bypass,
                replica_groups=[list(range(cfg.ncores))],
                ins=[rec_loc_d[0:npc, :].opt()],
                outs=[rec_d[:, :].opt()])

        # ------------------------------------------------------------------
        def phase2(l):
            with tc.tile_pool(name=f"ps2_{l}", bufs=2, space="PSUM") as pp:
                col0 = 0
                for g in range(ngroup):
                    CH = cfg.chg[g]
                    rows_g = P if g < ngroup - 1 else cfg.last_cnt
                    idxt = p2.tile([P, CHMAX * 8], i16, tag="idxt")
                    nc.sync.dma_start(out=idxt[:, :CH * 8],
                                      in_=idxrep_d[:, col0 * 8:(col0 + CH) * 8])
                    rect = p2.tile([P, CHMAX, RECP], rdt, tag="rect")
                    c0 = 0
                    for b in range(cfg.nbuckets):
                        cb = cfg.cbs[g][b]
                        if cb == 0:
                            continue
                        nrows = min(BUCKET, n - b * BUCKET)
                        done = 0
                        while done < cb:   # HW envelope: <=256 idxs per call
                            st = min(2, cb - done)
                            nc.gpsimd.dma_gather(
                                rect[:, c0 + done:c0 + done + st, :],
                                rec_d[b * BUCKET:b * BUCKET + nrows, :],
                                idxt[:, (c0 + done) * 8:(c0 + done + st) * 8],
                                st * P, st * P, RECP)
                            done += st
                        c0 += cb
                    # h_old + a_dst rows for this group's nodes
                    hadt = p2.tile([P, CDIM + H], f32, tag="hadt")
                    nc.gpsimd.indirect_dma_start(
                        out=hadt[:], out_offset=None, in_=had_d[l][:],
                        in_offset=bass.IndirectOffsetOnAxis(
                            ap=gread_sb[:, g:g + 1], axis=0))
                    ad_rhs = hadt[:, CDIM:]
                    if is_bf:
                        adr = p2.tile([P, H], rdt, tag="adr")
                        nc.vector.tensor_copy(adr[:], hadt[:, CDIM:])
                        ad_rhs = adr[:]
                    # one-hot M[edge, dst_slot]
                    Mt = p2.tile([P, CHMAX, P], rdt, tag="Mt")
                    nc.vector.tensor_tensor(
                        Mt[:, :CH, :],
                        dst16_sb[:, col0:col0 + CH][:, :, None].to_broadcast(
                            [P, CH, P]),
                        iota16_sb[:, None, :].to_broadcast([P, CH, P]),
                        Alu.is_equal)
                    # e_d: broadcast a_dst scores to edges via M^T matmuls
                    ped = pp.tile([P, CHMAX * H], f32, tag="ped")
                    for c in range(CH):
                        pmt = pp.tile([P, P], rdt, tag="pmt")
                        nc.tensor.transpose(pmt[:], Mt[:, c, :], identr_sb[:])
                        mt_sb = p2.tile([P, P], rdt, tag="mt_sb")
                        nc.scalar.copy(mt_sb[:], pmt[:])
                        nc.tensor.matmul(ped[:, c * H:(c + 1) * H],
                                         lhsT=mt_sb[:], rhs=ad_rhs,
                                         start=True, stop=True)
                    # e = lrelu(as + ad); ex = exp(e) -> rec[..., 256:260]
                    et = p2.tile([P, CHMAX, H], f32, tag="et")
                    nc.vector.tensor_tensor(
                        et[:, :CH, :], rect[:, :CH, HC:REC],
                        ped[:, 0:CH * H].rearrange("p (c h) -> p c h", h=H),
                        Alu.add)
                    lt = p2.tile([P, CHMAX, H], f32, tag="lt")
                    nc.vector.tensor_scalar_mul(lt[:, :CH, :], et[:, :CH, :],
                                                NEG_SLOPE)
                    nc.vector.tensor_tensor(et[:, :CH, :], lt[:, :CH, :],
                                            et[:, :CH, :], Alu.max)
                    nc.scalar.activation(rect[:, :CH, HC:REC], et[:, :CH, :],
                                         Act.Exp)
                    # V = ex * xh (per head, in place)
                    for h_ in range(H):
                        nc.vector.tensor_tensor(
                            rect[:, :CH, h_ * CDIM:(h_ + 1) * CDIM],
                            rect[:, :CH, h_ * CDIM:(h_ + 1) * CDIM],
                            rect[:, :CH, HC + h_:HC + h_ + 1].to_broadcast(
                                [P, CH, CDIM]),
                            Alu.mult)
                    # contract over edges: psum[:, 0:256]=sum alpha*xh, [256:260]=s
                    pg = pp.tile([P, REC], f32, tag="pg")
                    for c in range(CH):
                        nc.tensor.matmul(pg[:], lhsT=Mt[:, c, :],
                                         rhs=rect[:, c, 0:REC],
                                         start=(c == 0), stop=(c == CH - 1))
                    # r = 1 / (s + eps) / H
                    s4 = p2.tile([P, H], f32, tag="s4")
                    nc.vector.tensor_scalar(s4[:], pg[:, HC:REC], 1e-16, None,
                                            Alu.add)
                    r4 = p2.tile([P, H], f32, tag="r4")
                    nc.vector.reciprocal(r4[:], s4[:])
                    nc.vector.tensor_scalar_mul(r4[:], r4[:], 1.0 / H)
                    # head mean
                    yt = p2.tile([P, CDIM], f32, tag="yt")
                    tmp = p2.tile([P, CDIM], f32, tag="tmp")
                    nc.vector.tensor_scalar(yt[:], pg[:, 0:CDIM], r4[:, 0:1],
                                            None, Alu.mult)
                    for h_ in range(1, H):
                        nc.vector.tensor_scalar(tmp[:],
                                                pg[:, h_ * CDIM:(h_ + 1) * CDIM],
                                                r4[:, h_:h_ + 1], None, Alu.mult)
                        nc.vector.tensor_add(yt[:], yt[:], tmp[:])
                    nc.vector.tensor_add(yt[:], yt[:], convb_sb[l][:])
                    # layernorm
                    mu = p2.tile([P, 1], f32, tag="mu")
                    nc.vector.tensor_reduce(mu[:], yt[:], mybir.AxisListType.X,
                                            Alu.add)
                    nc.vector.tensor_scalar_mul(mu[:], mu[:], 1.0 / CDIM)
                    nc.vector.tensor_scalar(yt[:], yt[:], mu[:, 0:1], None,
                                            Alu.subtract)
                    sq = p2.tile([P, CDIM], f32, tag="sq")
                    var = p2.tile([P, 1], f32, tag="var")
                    nc.scalar.activation(sq[:], yt[:], Act.Square,
                                         accum_out=var[:])
                    nc.vector.tensor_scalar(var[:], var[:], 1.0 / CDIM, LN_EPS,
                                            Alu.mult, Alu.add)
                    sd = p2.tile([P, 1], f32, tag="sd")
                    nc.scalar.sqrt(sd[:], var[:])
                    inv = p2.tile([P, 1], f32, tag="inv")
                    nc.vector.reciprocal(inv[:], sd[:])
                    nc.vector.tensor_scalar(yt[:], yt[:], inv[:, 0:1], None,
                                            Alu.mult)
                    nc.vector.tensor_mul(yt[:], yt[:], lng_sb[l][:])
                    nc.vector.tensor_add(yt[:], yt[:], lnb_sb[l][:])
                    nc.vector.tensor_scalar_max(yt[:], yt[:], 0.0)
                    # residual + contiguous staging write
                    nc.vector.tensor_add(yt[:], yt[:], hadt[:, 0:CDIM])
                    nc.sync.dma_start(out=stag_d[l][g * P:g * P + rows_g, :],
                                      in_=yt[:rows_g, :])
                    col0 += CH

        # ------------------------------------------------------------------
        phase1(0)
        gather_rec()
        phase2(0)
        phase1(1)
        gather_rec()
        phase2(1)

        # final projection over own rows
        with tc.tile_pool(name="psf", bufs=2, space="PSUM") as pp:
            for t0 in range(0, npc, P):
                wr = min(P, npc - t0)
                ht2 = p2.tile([P, CDIM], f32, tag="ht2")
                nc.sync.dma_start(out=ht2[:wr], in_=stag_d[1][t0:t0 + wr, :])
                pt2 = pp.tile([CDIM, P], f32, tag="pt2")
                nc.tensor.transpose(pt2[:, :wr], ht2[:wr], ident_sb[:wr, :wr])
                hT2 = p2.tile([CDIM, P], f32, tag="hT2")
                nc.scalar.copy(hT2[:, :wr], pt2[:, :wr])
                po = pp.tile([P, OUT_F], f32, tag="po")
                nc.tensor.matmul(po[:wr], lhsT=hT2[:, :wr], rhs=outWT_sb[:],
                                 start=True, stop=True)
                ot = p2.tile([P, OUT_F], bf16, tag="ot")
                nc.vector.tensor_add(ot[:wr], po[:wr], outb_sb[:wr])
                nc.sync.dma_start(out=out_d[t0:t0 + wr, :], in_=ot[:wr, :])

    nc.compile()
    return nc


# --------------------------------------------------------------------------
# dispatch (cached jitted shard_map; mirrors bass2jax.run_bass_via_pjrt)
# --------------------------------------------------------------------------

_DISPATCH_CACHE = {}


def _make_dispatch(nc, ncores):
    key = id(nc)
    if key in _DISPATCH_CACHE:
        return _DISPATCH_CACHE[key]

    _enable_jax_cc()
    import jax
    from jax.sharding import Mesh, PartitionSpec
    from jax.experimental.shard_map import shard_map
    from concourse import bass2jax, mybir

    bass2jax.install_neuronx_cc_hook()
    partition_name = (nc.partition_id_tensor.name
                      if nc.partition_id_tensor else None)
    in_names, out_names, out_avals, out_shapes = [], [], [], []
    for alloc in nc.m.functions[0].allocations:
        if not isinstance(alloc, mybir.MemoryLocationSet):
            continue
        name = alloc.memorylocations[0].name
        if alloc.kind == "ExternalInput":
            if name != partition_name:
                in_names.append(name)
        elif alloc.kind == "ExternalOutput":
            out_names.append(name)
            shape = tuple(alloc.tensor_shape)
            dtype = mybir.dt.np(alloc.dtype)
            out_avals.append(jax.core.ShapedArray(shape, dtype))
            out_shapes.append((shape, dtype))
    n_params = len(in_names)
    n_outs = len(out_avals)
    all_names = list(in_names) + list(out_names)
    if partition_name is not None:
        all_names.append(partition_name)
    donate = tuple(range(n_params, n_params + n_outs))

    def _body(*args):
        operands = list(args)
        if partition_name is not None:
            operands.append(bass2jax.partition_id_tensor())
        outs = bass2jax._bass_exec_p.bind(
            *operands, out_avals=tuple(out_avals),
            in_names=tuple(all_names), out_names=tuple(out_names),
            lowering_input_output_aliases=(), sim_require_finite=True,
            sim_require_nnan=True, nc=nc)
        return tuple(outs)

    devices = jax.devices()[:ncores]
    mesh = Mesh(np.asarray(devices), ("core",))
    sharded = jax.jit(
        shard_map(_body, mesh=mesh,
                  in_specs=(PartitionSpec("core"),) * (n_params + n_outs),
                  out_specs=(PartitionSpec("core"),) * n_outs,
                  check_rep=False),
        donate_argnums=donate, keep_unused=True)

    def run(maps):
        concat_in = [np.concatenate([np.asarray(m[nm]) for m in maps], axis=0)
                     for nm in in_names]
        concat_zeros = [np.zeros((ncores * sh[0], *sh[1:]), dt)
                        for sh, dt in out_shapes]
        out_arrs = sharded(*concat_in, *concat_zeros)
        return [
            {name: np.asarray(out_arrs[i]).reshape(
                ncores, *out_shapes[i][0])[c]
             for i, name in enumerate(out_names)}
            for c in range(ncores)
        ]

    _DISPATCH_CACHE[key] = run
    return run


# --------------------------------------------------------------------------
# entry point
# --------------------------------------------------------------------------

def _in_maps(cfg, prep, wts):
    """Pack per-core inputs into one f32 blob (with i16/bf16 sections)."""
    import ml_dtypes
    LAYF, LAYI, F32SZ, TOTAL = _layout(cfg)
    npc = cfg.npc
    o16_base = 2 * F32SZ

    blob_shared = np.zeros(TOTAL, np.float32)
    for nm in ("nodeWa", "droneTa", "droneWa", "wcomb0", "wcomb1", "outWT",
               "convb0", "convb1", "lng0", "lng1", "lnb0", "lnb1", "outb"):
        o, sh = LAYF[nm]
        blob_shared[o:o + sh[0] * sh[1]] = np.asarray(
            wts[nm], np.float32).ravel()

    batch = np.asarray(wts["batch"]).astype(np.int16)
    maps = []
    for k in range(cfg.ncores):
        blob = blob_shared.copy()
        b16 = blob.view(np.int16)
        pc = prep["per_core"][k]

        def put(nm, data16):
            o, sh = LAYI[nm]
            sz = sh[0] * sh[1]
            b16[o16_base + o:o16_base + o + sz] = data16.ravel()

        put("idx16", pc["idx16"])
        put("dstslot", pc["dstslot"])
        bp = np.zeros(cfg.ngroup * P, np.int16)
        bp[:npc] = batch[k * npc:(k + 1) * npc]
        put("batch", np.ascontiguousarray(bp.reshape(cfg.ngroup, P).T))
        xbf = np.ascontiguousarray(
            wts["xTa"][:, k * npc:(k + 1) * npc]).astype(
                ml_dtypes.bfloat16).view(np.int16)
        put("xTbf", xbf)
        maps.append(dict(blob=blob))
    return maps


def kernel(**inputs):
    edge_index = np.asarray(inputs["edge_index"])
    prep = _host_prep(edge_index, N, NCORES)
    cfg = _Cfg(N, NCORES, prep["cbs"])
    wts = _host_weights(inputs, prep["order"], N)
    nc = _build(cfg)
    maps = _in_maps(cfg, prep, wts)

    run = _make_dispatch(nc, NCORES)
    res = run(maps)
    out = np.empty((N, OUT_F), np.float32)
    for k in range(NCORES):
        out[prep["order"][k * cfg.npc:(k + 1) * cfg.npc]] = \
            res[k]["out"].astype(np.float32)
    return out


# revision 11
# speedup vs baseline: 29.7387x; 1.2095x over previous
"""GAT (2-layer, 4-head, segment-softmax) message-passing kernel for 8 Trainium2
NeuronCores.

Strategy (dst-sharded, edge aggregation as one-hot matmuls):
  * Nodes are assigned to cores/groups with degree-balanced packing (LPT). The
    node permutation is (core, group, slot) order, so each core owns a
    contiguous block of rows and each group's 128 nodes are contiguous.
  * Phase 1 is SHARDED: each core computes the record table
    rec[n] = [xh(256) | a_src-score(4) | pad] only for its own npc rows, plus
    had[n] = [h(64) | ad(4)]; an 8-core AllGather replicates rec on-device
    (NeuronLink) so phase 2 can gather any source node's record locally.
  * For each destination group (128 nodes), the core gathers the records of
    the group's in-edges' source nodes with gpsimd dma_gather (int16 indices,
    source-bucketed in 32768-row windows), builds the one-hot incidence matrix
    M[edge, dst_slot] on the vector engine (iota compare), broadcasts the
    a_dst scores to edges via transposed-one-hot matmuls, and reduces both the
    softmax denominators and the weighted feature sums with PSUM-accumulated
    matmuls (contracting over edges). Softmax normalization is applied after
    the reduction - mathematically identical to the reference's segment
    softmax (max-subtraction is a no-op at these magnitudes).
  * Host->device traffic is minimized (the axon tunnel is ~65 MB/s with ~75ms
    per-array overhead): ALL inputs are packed into ONE f32 blob per core
    (~2.2MB: f32 weights; int16 gather indices / dst slots / batch ids and
    the bf16 own-shard xT are stored via bitcast views). The drone-feature
    term is an on-device indirect gather of the 64x64 projected table; gread
    offsets are iota-generated on device; the output is returned as bf16.
  * Dispatch uses a cached jitted shard_map executable (compiled once per
    process) plus the JAX persistent compilation cache, so steady-state
    dispatch cost is input upload + execute + output download.
"""

import os
import sys

sys.path.insert(0, "/opt/trn_rl_repo")

import numpy as np

# ---- problem constants (hardcoded; kernel.py must be self-contained) ----
N = 100000
E = 1600000
G = 64
H = 4
CDIM = 64
NODE_F = 32
DRONE_F = 16
OUT_F = 32
LN_EPS = 1e-5
NEG_SLOPE = 0.2
NCORES = 8
P = 128
HC = H * CDIM          # 256
REC = HC + H           # 260: [V(256) | as/ex(4)]
BUCKET = 32768         # int16 index range per dma_gather bucket
TB = 6                 # phase-1 tile batch

REC_DT_NAME = os.environ.get("GAT_REC_DT", "bfloat16")


def _enable_jax_cc():
    import jax
    try:
        jax.config.update("jax_compilation_cache_dir",
                          os.environ.get("JAX_CC_DIR", "/tmp/jax_cc_cache"))
        jax.config.update("jax_persistent_cache_min_entry_size_bytes", 0)
        jax.config.update("jax_persistent_cache_min_compile_time_secs", 0.0)
    except Exception:
        pass


class _Cfg:
    def __init__(self, n, ncores, cbs, rec_dt=REC_DT_NAME, debug=False):
        assert n % ncores == 0
        self.n = n
        self.ncores = ncores
        self.npc = n // ncores
        self.ngroup = -(-self.npc // P)
        self.cbs = cbs                       # [ngroup][nbuckets] chunk counts
        self.nbuckets = len(cbs[0])
        self.chg = [sum(row) for row in cbs]  # chunks per group
        self.chmax = max(self.chg)
        self.cols = sum(self.chg)            # total chunk columns
        self.rec_dt = rec_dt
        self.recp = 320 if rec_dt == "float32" else 384  # padded record elems
        self.debug = debug
        # own-shard tiling (phase 1 + final projection)
        self.nt_full, self.nt_rem = divmod(self.npc, P)
        self.last_cnt = self.npc - (self.ngroup - 1) * P


def _layout(cfg):
    """Single-blob layout. Returns (f32 sections, i16 sections, total f32
    elems). i16 section offsets are in int16 units from the start of the
    int16 region, which begins at f32 elem F32SZ (i16 elem 2*F32SZ)."""
    f32 = {}
    off = 0
    for nm, sh in [("nodeWa", (NODE_F + 1, CDIM)),
                   ("droneTa", (DRONE_F + 1, G)),
                   ("droneWa", (DRONE_F + 1, CDIM)),
                   ("wcomb0", (CDIM, REC + H)),
                   ("wcomb1", (CDIM, REC + H)),
                   ("outWT", (CDIM, OUT_F)),
                   # convb0|convb1|lng0|lng1|lnb0|lnb1|outb rows
                   ("smalls", (1, 6 * CDIM + OUT_F))]:
        f32[nm] = (off, sh)
        off += sh[0] * sh[1]
    f32sz = off
    i16 = {}
    off = 0
    for nm, sh in [("idx16", (16, cfg.cols * 8)),
                   ("batch", (P, cfg.ngroup)),
                   ("xTbf", (NODE_F + 1, cfg.npc))]:
        sz = sh[0] * sh[1]
        i16[nm] = (off, sh)
        off += sz + (sz & 1)                 # keep 32-bit alignment
    i16sz = off
    i8 = {}
    off = 0
    for nm, sh in [("dstslot", (P, cfg.cols))]:
        sz = sh[0] * sh[1]
        i8[nm] = (off, sh)
        off += sz + (-sz) % 4                # keep 32-bit alignment
    total = f32sz + i16sz // 2 + off // 4
    return f32, i16, i8, f32sz, i16sz, total


# --------------------------------------------------------------------------
# host-side preprocessing
# --------------------------------------------------------------------------

def _lpt(loads, caps):
    """LPT packing into len(caps) bins with given item capacities, balancing
    total load. Returns assignment array."""
    import heapq

    nbins = len(caps)
    order = np.argsort(-loads, kind="stable")
    heap = [(0, b) for b in range(nbins)]
    heapq.heapify(heap)
    cnt = np.zeros(nbins, np.int64)
    tot = np.zeros(nbins, np.int64)
    assign = np.empty(len(loads), np.int32)
    for i in order:
        while True:
            _, b = heapq.heappop(heap)
            if cnt[b] < caps[b]:
                break
        assign[i] = b
        cnt[b] += 1
        tot[b] += loads[i]
        if cnt[b] < caps[b]:
            heapq.heappush(heap, (int(tot[b]), b))
    return assign


def _host_prep(edge_index, n, ncores):
    """Node permutation + per-core gather index streams."""
    npc = n // ncores
    ngroup = -(-npc // P)
    last_cnt = npc - (ngroup - 1) * P
    nbuckets = -(-n // BUCKET)

    loop = np.arange(n, dtype=np.int64)
    src = np.concatenate([edge_index[0].astype(np.int64), loop])
    dst = np.concatenate([edge_index[1].astype(np.int64), loop])
    deg = np.bincount(dst, minlength=n)

    core_of = _lpt(deg, [npc] * ncores)
    group_of = np.empty(n, np.int32)
    slot_of = np.empty(n, np.int32)
    pos_of = np.empty(n, np.int64)
    order = np.empty(n, np.int64)
    caps = [P] * (ngroup - 1) + [last_cnt]
    for k in range(ncores):
        nodes_k = np.where(core_of == k)[0]
        g_assign = _lpt(deg[nodes_k], caps)
        o = np.argsort(g_assign, kind="stable")
        cnts = np.bincount(g_assign, minlength=ngroup)
        starts = np.concatenate([[0], np.cumsum(cnts)])[:-1]
        slot = np.empty(len(nodes_k), np.int64)
        slot[o] = np.arange(len(nodes_k)) - starts[g_assign[o]]
        group_of[nodes_k] = g_assign
        slot_of[nodes_k] = slot
        pos = k * npc + g_assign * P + slot
        pos_of[nodes_k] = pos
        order[pos] = nodes_k

    # per-(group,bucket) edge counts per core -> uniform chunk schedule
    e_core = core_of[dst]
    e_group = group_of[dst]
    e_bucket = pos_of[src] // BUCKET
    cnts = np.zeros((ncores, ngroup, nbuckets), np.int64)
    np.add.at(cnts, (e_core, e_group, e_bucket), 1)
    cbs_np = -(-cnts.max(axis=0) // P)       # [ngroup, nbuckets] chunks
    cbs = [[int(c) for c in row] for row in cbs_np]
    chg = np.array([sum(row) for row in cbs])
    cols = int(chg.sum())
    goff = np.concatenate([[0], np.cumsum(chg)])[:-1]
    boff = np.zeros((ngroup, nbuckets), np.int64)
    for g in range(ngroup):
        o = goff[g]
        for b in range(nbuckets):
            boff[g, b] = o
            o += cbs[g][b]

    per_core = []
    for k in range(ncores):
        mask = e_core == k
        es = pos_of[src[mask]]
        eg = e_group[mask]
        eb = e_bucket[mask]
        esl = slot_of[dst[mask]]
        o = np.lexsort((eb, eg))
        es, eg, eb, esl = es[o], eg[o], eb[o], esl[o]
        cnt_k = np.zeros((ngroup, nbuckets), np.int64)
        np.add.at(cnt_k, (eg, eb), 1)
        flat = cnt_k.reshape(-1)
        starts = np.concatenate([[0], np.cumsum(flat)])[:-1].reshape(
            ngroup, nbuckets)
        j = np.arange(len(es)) - starts[eg, eb]      # pos within (g,b)
        slotj = boff[eg, eb] * P + j                 # global slot in stream

        dstslot = np.full((P, cols), -1, np.int16)
        dstslot[slotj % P, slotj // P] = esl
        idx16 = np.zeros((16, cols * 8), np.int16)   # 8 int16 cols per chunk
        idx16[slotj % 16, slotj // 16] = es - eb * BUCKET
        per_core.append(dict(dstslot=dstslot, idx16=idx16))
    return dict(order=order, pos_of=pos_of, cbs=cbs, per_core=per_core)


def _host_weights(inputs, order, n):
    """Permuted/augmented weight + input tensors (all float32)."""
    f = np.float32
    x = np.asarray(inputs["x"], f)[order]            # perm rows
    batch = np.asarray(inputs["batch"])[order]
    xTa = np.concatenate([x.T, np.ones((1, n), f)], 0)           # [33, n]
    droneTa = np.concatenate(
        [np.asarray(inputs["drone_feat"], f).T, np.ones((1, G), f)], 0)
    droneWa = np.concatenate(
        [np.asarray(inputs["drone_W"], f).T, np.asarray(inputs["drone_b"], f)[None]], 0)
    nodeWa = np.concatenate(
        [np.asarray(inputs["node_W"], f).T, np.asarray(inputs["node_b"], f)[None]], 0)
    out = dict(xTa=xTa, batch=batch, droneTa=droneTa, droneWa=droneWa,
               nodeWa=nodeWa,
               outWT=np.ascontiguousarray(np.asarray(inputs["out_W"], f).T),
               outb=np.tile(np.asarray(inputs["out_b"], f), (P, 1)))
    for l in range(2):
        W = np.asarray(inputs[f"convW{l}"], f)       # [HC, CDIM]
        a_s = np.asarray(inputs[f"att_src{l}"], f)   # [H, CDIM]
        a_d = np.asarray(inputs[f"att_dst{l}"], f)
        Wh = W.reshape(H, CDIM, CDIM)
        Ws = np.einsum("hcf,hc->fh", Wh, a_s)        # [CDIM, H]
        Wd = np.einsum("hcf,hc->fh", Wh, a_d)
        out[f"wcomb{l}"] = np.concatenate([W.T, Ws, Wd], 1)   # [CDIM, 264]
        out[f"convb{l}"] = np.tile(np.asarray(inputs[f"convb{l}"], f), (P, 1))
        out[f"lng{l}"] = np.tile(np.asarray(inputs[f"ln_g{l}"], f), (P, 1))
        out[f"lnb{l}"] = np.tile(np.asarray(inputs[f"ln_b{l}"], f), (P, 1))
    return out


# --------------------------------------------------------------------------
# bass kernel
# --------------------------------------------------------------------------

def _build(cfg):
    import concourse.bass as bass
    import concourse.bacc as bacc
    import concourse.tile as tile
    from concourse import mybir
    from concourse.masks import make_identity

    f32 = mybir.dt.float32
    i32 = mybir.dt.int32
    i16 = mybir.dt.int16
    i8 = mybir.dt.int8
    bf16 = mybir.dt.bfloat16
    rdt = getattr(mybir.dt, cfg.rec_dt)
    is_bf = cfg.rec_dt != "float32"
    Alu = mybir.AluOpType
    Act = mybir.ActivationFunctionType

    n, npc, ngroup = cfg.n, cfg.npc, cfg.ngroup
    RECP, CHMAX = cfg.recp, cfg.chmax
    LAYF, LAYI, LAYB, F32SZ, I16SZ, TOTAL = _layout(cfg)

    nc = bacc.Bacc("TRN2", target_bir_lowering=False, debug=cfg.debug,
                   num_devices=cfg.ncores)

    blob_d = nc.dram_tensor("blob", [TOTAL], f32, kind="ExternalInput")

    def fview(nm):
        o, sh = LAYF[nm]
        return blob_d[o:o + sh[0] * sh[1]].rearrange("(a b) -> a b", a=sh[0])

    def iview(nm, dt):
        o, sh = LAYI[nm]
        sz = sh[0] * sh[1]
        o32 = F32SZ + o // 2                 # o is even by construction
        return blob_d[o32:o32 + (sz + 1) // 2].bitcast(dt)[
            0:sz].rearrange("(a b) -> a b", a=sh[0])

    def bview(nm, dt):
        o, sh = LAYB[nm]
        sz = sh[0] * sh[1]
        o32 = F32SZ + I16SZ // 2 + o // 4    # o is 4-aligned by construction
        return blob_d[o32:o32 + (sz + 3) // 4].bitcast(dt)[
            0:sz].rearrange("(a b) -> a b", a=sh[0])

    out_d = nc.dram_tensor("out", [npc, OUT_F], bf16, kind="ExternalOutput")

    rec_loc_d = nc.dram_tensor("rec_loc", [npc, RECP], rdt)
    rec_d = nc.dram_tensor("rec", [n, RECP], rdt,
                           addr_space="Shared" if cfg.ncores > 1 else "Local")
    had_d = [nc.dram_tensor(f"had{l}", [npc, CDIM + H], f32) for l in range(2)]
    stag_d = [nc.dram_tensor(f"stag{l}", [ngroup * P, CDIM], f32)
              for l in range(2)]
    idxrep_d = nc.dram_tensor("idxrep", [P, cfg.cols * 8], i16)
    dr_d = nc.dram_tensor("dr", [G, CDIM], f32)

    from contextlib import ExitStack
    with tile.TileContext(nc) as tc, ExitStack() as ctx:
        cpool = ctx.enter_context(tc.tile_pool(name="const", bufs=1))
        p1 = ctx.enter_context(tc.tile_pool(name="p1", bufs=2))
        p2 = ctx.enter_context(tc.tile_pool(name="p2", bufs=2))

        def cload(nm):
            o, sh = LAYF[nm]
            t = cpool.tile(list(sh), f32, tag=f"c_{nm}")
            nc.sync.dma_start(out=t[:], in_=fview(nm))
            return t

        droneTa_sb = cload("droneTa")
        droneWa_sb = cload("droneWa")
        nodeWa_sb = cload("nodeWa")
        wcomb_sb = [cload("wcomb0"), cload("wcomb1")]
        outWT_sb = cload("outWT")
        nodeWb_sb = cpool.tile([NODE_F + 1, CDIM], bf16, tag="nodeWb")
        nc.vector.tensor_copy(nodeWb_sb[:], nodeWa_sb[:])

        # broadcast the bias/LN rows [1, 416] to all 128 partitions via a
        # ones-column matmul, then slice views
        SMW = 6 * CDIM + OUT_F
        smrow_sb = cload("smalls")           # [1, SMW]
        ones_sb = cpool.tile([1, P], f32, tag="ones1")
        nc.vector.memset(ones_sb[:], 1.0)
        smallsb = cpool.tile([P, SMW], f32, tag="smallsb")
        with tc.tile_pool(name="pssm", bufs=1, space="PSUM") as ppsm:
            psm = ppsm.tile([P, SMW], f32)
            nc.tensor.matmul(psm[:], lhsT=ones_sb[:], rhs=smrow_sb[:],
                             start=True, stop=True)
            nc.scalar.copy(smallsb[:], psm[:])
        convb_sb = [smallsb[:, 0:CDIM], smallsb[:, CDIM:2 * CDIM]]
        lng_sb = [smallsb[:, 2 * CDIM:3 * CDIM], smallsb[:, 3 * CDIM:4 * CDIM]]
        lnb_sb = [smallsb[:, 4 * CDIM:5 * CDIM], smallsb[:, 5 * CDIM:6 * CDIM]]
        outb_sb = smallsb[:, 6 * CDIM:6 * CDIM + OUT_F]

        # int8/int16 streams: dst slots + batch ids (+ iota / gread)
        dst8_sb = cpool.tile([P, cfg.cols], i8, tag="dst8")
        nc.sync.dma_start(out=dst8_sb[:], in_=bview("dstslot", i8))
        dst16_sb = cpool.tile([P, cfg.cols], i16, tag="dst16")
        nc.vector.tensor_copy(dst16_sb[:], dst8_sb[:])
        bat16_sb = cpool.tile([P, ngroup], i16, tag="bat16")
        nc.sync.dma_start(out=bat16_sb[:], in_=iview("batch", i16))
        bat32_sb = cpool.tile([P, ngroup], i32, tag="bat32")
        nc.vector.tensor_copy(bat32_sb[:], bat16_sb[:])

        # replicate the 16-partition gather-index stream to 128 partitions
        # (dma_gather wants idxs wrapped in 16 partitions x 8 gpsimd cores)
        for k8 in range(8):
            nc.sync.dma_start(out=idxrep_d[k8 * 16:(k8 + 1) * 16, :],
                              in_=iview("idx16", i16))

        iota_sb = cpool.tile([P, P], i32)
        nc.gpsimd.iota(iota_sb[:], pattern=[[1, P]], base=0,
                       channel_multiplier=0)
        iota16_sb = cpool.tile([P, P], i16)
        nc.vector.tensor_copy(iota16_sb[:], iota_sb[:])
        ident_sb = cpool.tile([P, P], f32)
        make_identity(nc, ident_sb[:])
        identr_sb = ident_sb
        if is_bf:
            identr_sb = cpool.tile([P, P], rdt)
            nc.vector.tensor_copy(identr_sb[:], ident_sb[:])

        # gread[p, g] = min(g*128 + p, npc-1): offsets into local had_d
        gread_sb = cpool.tile([P, ngroup], i32, tag="gread")
        nc.gpsimd.iota(gread_sb[:], pattern=[[P, ngroup]], base=0,
                       channel_multiplier=1)
        nc.vector.tensor_scalar(gread_sb[:], gread_sb[:], npc - 1, None,
                                Alu.min)

        # dr = droneTa.T @ droneWa  -> dram (indirect-gather source)
        dr_sb = cpool.tile([G, CDIM], f32)
        with tc.tile_pool(name="psdr", bufs=1, space="PSUM") as ppdr:
            pdr_t = ppdr.tile([P, CDIM], f32)
            pdr = pdr_t[:G]
            nc.tensor.matmul(pdr, lhsT=droneTa_sb[:], rhs=droneWa_sb[:],
                             start=True, stop=True)
            nc.scalar.copy(dr_sb[:], pdr)
        nc.sync.dma_start(out=dr_d[:, :], in_=dr_sb[:])

        # ------------------------------------------------------------------
        def phase1(l):
            """Build rec_loc[npc, RECP] and had[npc, 68] tile by tile
            (own shard only; AllGather replicates rec afterwards)."""
            xT_v = iview("xTbf", bf16)
            with tc.tile_pool(name=f"ps1_{l}", bufs=2, space="PSUM") as pp:

                def do_batch(r0, tb, rows):
                    if l == 0:
                        xb = p1.tile([NODE_F + 1, TB * P], bf16, tag="xb")
                        nc.sync.dma_start(out=xb[:, :rows],
                                          in_=xT_v[:, r0:r0 + rows])
                    hadb = p1.tile([P, TB, CDIM + H], f32, tag="hadb")
                    if l == 1:
                        if rows == tb * P:
                            nc.sync.dma_start(
                                out=hadb[:, :tb, :CDIM],
                                in_=stag_d[0][r0:r0 + rows, :].rearrange(
                                    "(c p) f -> p c f", p=P))
                        else:
                            nc.sync.dma_start(out=hadb[:rows, 0, :CDIM],
                                              in_=stag_d[0][r0:r0 + rows, :])
                    recb = p1.tile([P, TB, RECP], rdt, tag="recb")
                    nc.vector.memset(recb[:, :, REC:], 0.0)
                    for t in range(tb):
                        pr_ = min(P, rows - t * P)
                        g_abs = r0 // P + t
                        if l == 0:
                            drt = p1.tile([P, CDIM], f32, tag="drt")
                            nc.gpsimd.indirect_dma_start(
                                out=drt[:], out_offset=None, in_=dr_d[:],
                                in_offset=bass.IndirectOffsetOnAxis(
                                    ap=bat32_sb[:, g_abs:g_abs + 1], axis=0))
                            ph = pp.tile([P, CDIM], f32, tag="ph")
                            nc.tensor.matmul(ph[:pr_],
                                             lhsT=xb[:, t * P:t * P + pr_],
                                             rhs=nodeWb_sb[:], start=True,
                                             stop=True)
                            nc.vector.tensor_tensor(hadb[:pr_, t, :CDIM],
                                                    ph[:pr_], drt[:pr_],
                                                    Alu.add)
                        pt = pp.tile([CDIM, P], f32, tag="pt")
                        nc.tensor.transpose(pt[:, :pr_], hadb[:pr_, t, :CDIM],
                                            ident_sb[:pr_, :pr_])
                        hT = p1.tile([CDIM, P], f32, tag="hT")
                        nc.scalar.copy(hT[:, :pr_], pt[:, :pr_])
                        prc = pp.tile([P, REC + H], f32, tag="pr")
                        nc.tensor.matmul(prc[:pr_], lhsT=hT[:, :pr_],
                                         rhs=wcomb_sb[l][:], start=True,
                                         stop=True)
                        nc.scalar.copy(recb[:pr_, t, 0:REC], prc[:pr_, 0:REC])
                        nc.vector.tensor_copy(hadb[:pr_, t, CDIM:],
                                              prc[:pr_, REC:REC + H])
                    if rows == tb * P:
                        nc.sync.dma_start(
                            out=rec_loc_d[r0:r0 + rows, :].rearrange(
                                "(c p) f -> p c f", p=P),
                            in_=recb[:, :tb, :])
                        nc.sync.dma_start(
                            out=had_d[l][r0:r0 + rows, :].rearrange(
                                "(c p) f -> p c f", p=P),
                            in_=hadb[:, :tb, :])
                    else:
                        nc.sync.dma_start(out=rec_loc_d[r0:r0 + rows, :],
                                          in_=recb[:rows, 0, :])
                        nc.sync.dma_start(out=had_d[l][r0:r0 + rows, :],
                                          in_=hadb[:rows, 0, :])

                for b0 in range(0, cfg.nt_full, TB):
                    tb = min(TB, cfg.nt_full - b0)
                    do_batch(b0 * P, tb, tb * P)
                if cfg.nt_rem:
                    do_batch(cfg.nt_full * P, 1, cfg.nt_rem)

        def gather_rec():
            nc.gpsimd.collective_compute(
                "AllGather", mybir.AluOpType.bypass,
                replica_groups=[list(range(cfg.ncores))],
                ins=[rec_loc_d[0:npc, :].opt()],
                outs=[rec_d[:, :].opt()])

        # ------------------------------------------------------------------
        def phase2(l):
            with tc.tile_pool(name=f"ps2_{l}", bufs=2, space="PSUM") as pp:
                col0 = 0
                for g in range(ngroup):
                    CH = cfg.chg[g]
                    rows_g = P if g < ngroup - 1 else cfg.last_cnt
                    idxt = p2.tile([P, CHMAX * 8], i16, tag="idxt")
                    nc.sync.dma_start(out=idxt[:, :CH * 8],
                                      in_=idxrep_d[:, col0 * 8:(col0 + CH) * 8])
                    rect = p2.tile([P, CHMAX, RECP], rdt, tag="rect")
                    c0 = 0
                    for b in range(cfg.nbuckets):
                        cb = cfg.cbs[g][b]
                        if cb == 0:
                            continue
                        nrows = min(BUCKET, n - b * BUCKET)
                        done = 0
                        while done < cb:   # HW envelope: <=256 idxs per call
                            st = min(2, cb - done)
                            nc.gpsimd.dma_gather(
                                rect[:, c0 + done:c0 + done + st, :],
                                rec_d[b * BUCKET:b * BUCKET + nrows, :],
                                idxt[:, (c0 + done) * 8:(c0 + done + st) * 8],
                                st * P, st * P, RECP)
                            done += st
                        c0 += cb
                    # h_old + a_dst rows for this group's nodes
                    hadt = p2.tile([P, CDIM + H], f32, tag="hadt")
                    nc.gpsimd.indirect_dma_start(
                        out=hadt[:], out_offset=None, in_=had_d[l][:],
                        in_offset=bass.IndirectOffsetOnAxis(
                            ap=gread_sb[:, g:g + 1], axis=0))
                    ad_rhs = hadt[:, CDIM:]
                    if is_bf:
                        adr = p2.tile([P, H], rdt, tag="adr")
                        nc.vector.tensor_copy(adr[:], hadt[:, CDIM:])
                        ad_rhs = adr[:]
                    # one-hot M[edge, dst_slot]
                    Mt = p2.tile([P, CHMAX, P], rdt, tag="Mt")
                    nc.vector.tensor_tensor(
                        Mt[:, :CH, :],
                        dst16_sb[:, col0:col0 + CH][:, :, None].to_broadcast(
                            [P, CH, P]),
                        iota16_sb[:, None, :].to_broadcast([P, CH, P]),
                        Alu.is_equal)
                    # e_d: broadcast a_dst scores to edges via M^T matmuls
                    ped = pp.tile([P, CHMAX * H], f32, tag="ped")
                    for c in range(CH):
                        pmt = pp.tile([P, P], rdt, tag="pmt")
                        nc.tensor.transpose(pmt[:], Mt[:, c, :], identr_sb[:])
                        mt_sb = p2.tile([P, P], rdt, tag="mt_sb")
                        nc.scalar.copy(mt_sb[:], pmt[:])
                        nc.tensor.matmul(ped[:, c * H:(c + 1) * H],
                                         lhsT=mt_sb[:], rhs=ad_rhs,
                                         start=True, stop=True)
                    # e = lrelu(as + ad); ex = exp(e) -> rec[..., 256:260]
                    et = p2.tile([P, CHMAX, H], f32, tag="et")
                    nc.vector.tensor_tensor(
                        et[:, :CH, :], rect[:, :CH, HC:REC],
                        ped[:, 0:CH * H].rearrange("p (c h) -> p c h", h=H),
                        Alu.add)
                    lt = p2.tile([P, CHMAX, H], f32, tag="lt")
                    nc.vector.tensor_scalar_mul(lt[:, :CH, :], et[:, :CH, :],
                                                NEG_SLOPE)
                    nc.vector.tensor_tensor(et[:, :CH, :], lt[:, :CH, :],
                                            et[:, :CH, :], Alu.max)
                    nc.scalar.activation(rect[:, :CH, HC:REC], et[:, :CH, :],
                                         Act.Exp)
                    # V = ex * xh (per head, in place)
                    for h_ in range(H):
                        nc.vector.tensor_tensor(
                            rect[:, :CH, h_ * CDIM:(h_ + 1) * CDIM],
                            rect[:, :CH, h_ * CDIM:(h_ + 1) * CDIM],
                            rect[:, :CH, HC + h_:HC + h_ + 1].to_broadcast(
                                [P, CH, CDIM]),
                            Alu.mult)
                    # contract over edges: psum[:, 0:256]=sum alpha*xh, [256:260]=s
                    pg = pp.tile([P, REC], f32, tag="pg")
                    for c in range(CH):
                        nc.tensor.matmul(pg[:], lhsT=Mt[:, c, :],
                                         rhs=rect[:, c, 0:REC],
                                         start=(c == 0), stop=(c == CH - 1))
                    # r = 1 / (s + eps) / H
                    s4 = p2.tile([P, H], f32, tag="s4")
                    nc.vector.tensor_scalar(s4[:], pg[:, HC:REC], 1e-16, None,
                                            Alu.add)
                    r4 = p2.tile([P, H], f32, tag="r4")
                    nc.vector.reciprocal(r4[:], s4[:])
                    nc.vector.tensor_scalar_mul(r4[:], r4[:], 1.0 / H)
                    # head mean
                    yt = p2.tile([P, CDIM], f32, tag="yt")
                    tmp = p2.tile([P, CDIM], f32, tag="tmp")
                    nc.vector.tensor_scalar(yt[:], pg[:, 0:CDIM], r4[:, 0:1],
                                            None, Alu.mult)
                    for h_ in range(1, H):
                        nc.vector.tensor_scalar(tmp[:],
                                                pg[:, h_ * CDIM:(h_ + 1) * CDIM],
                                                r4[:, h_:h_ + 1], None, Alu.mult)
                        nc.vector.tensor_add(yt[:], yt[:], tmp[:])
                    nc.vector.tensor_add(yt[:], yt[:], convb_sb[l])
                    # layernorm
                    mu = p2.tile([P, 1], f32, tag="mu")
                    nc.vector.tensor_reduce(mu[:], yt[:], mybir.AxisListType.X,
                                            Alu.add)
                    nc.vector.tensor_scalar_mul(mu[:], mu[:], 1.0 / CDIM)
                    nc.vector.tensor_scalar(yt[:], yt[:], mu[:, 0:1], None,
                                            Alu.subtract)
                    sq = p2.tile([P, CDIM], f32, tag="sq")
                    var = p2.tile([P, 1], f32, tag="var")
                    nc.scalar.activation(sq[:], yt[:], Act.Square,
                                         accum_out=var[:])
                    nc.vector.tensor_scalar(var[:], var[:], 1.0 / CDIM, LN_EPS,
                                            Alu.mult, Alu.add)
                    sd = p2.tile([P, 1], f32, tag="sd")
                    nc.scalar.sqrt(sd[:], var[:])
                    inv = p2.tile([P, 1], f32, tag="inv")
                    nc.vector.reciprocal(inv[:], sd[:])
                    nc.vector.tensor_scalar(yt[:], yt[:], inv[:, 0:1], None,
                                            Alu.mult)
                    nc.vector.tensor_mul(yt[:], yt[:], lng_sb[l])
                    nc.vector.tensor_add(yt[:], yt[:], lnb_sb[l])
                    nc.vector.tensor_scalar_max(yt[:], yt[:], 0.0)
                    # residual + contiguous staging write
                    nc.vector.tensor_add(yt[:], yt[:], hadt[:, 0:CDIM])
                    nc.sync.dma_start(out=stag_d[l][g * P:g * P + rows_g, :],
                                      in_=yt[:rows_g, :])
                    col0 += CH

        # ------------------------------------------------------------------
        phase1(0)
        gather_rec()
        phase2(0)
        phase1(1)
        gather_rec()
        phase2(1)

        # final projection over own rows
        with tc.tile_pool(name="psf", bufs=2, space="PSUM") as pp:
            for t0 in range(0, npc, P):
                wr = min(P, npc - t0)
                ht2 = p2.tile([P, CDIM], f32, tag="ht2")
                nc.sync.dma_start(out=ht2[:wr], in_=stag_d[1][t0:t0 + wr, :])
                pt2 = pp.tile([CDIM, P], f32, tag="pt2")
                nc.tensor.transpose(pt2[:, :wr], ht2[:wr], ident_sb[:wr, :wr])
                hT2 = p2.tile([CDIM, P], f32, tag="hT2")
                nc.scalar.copy(hT2[:, :wr], pt2[:, :wr])
                po = pp.tile([P, OUT_F], f32, tag="po")
                nc.tensor.matmul(po[:wr], lhsT=hT2[:, :wr], rhs=outWT_sb[:],
                                 start=True, stop=True)
                ot = p2.tile([P, OUT_F], bf16, tag="ot")
                nc.vector.tensor_add(ot[:wr], po[:wr], outb_sb[:wr])
                nc.sync.dma_start(out=out_d[t0:t0 + wr, :], in_=ot[:wr, :])

    nc.compile()
    return nc


# --------------------------------------------------------------------------
# dispatch (cached jitted shard_map; mirrors bass2jax.run_bass_via_pjrt)
# --------------------------------------------------------------------------

_DISPATCH_CACHE = {}


def _make_dispatch(nc, ncores):
    key = id(nc)
    if key in _DISPATCH_CACHE:
        return _DISPATCH_CACHE[key]

    _enable_jax_cc()
    import jax
    from jax.sharding import Mesh, PartitionSpec
    from jax.experimental.shard_map import shard_map
    from concourse import bass2jax, mybir

    bass2jax.install_neuronx_cc_hook()
    partition_name = (nc.partition_id_tensor.name
                      if nc.partition_id_tensor else None)
    in_names, out_names, out_avals, out_shapes = [], [], [], []
    for alloc in nc.m.functions[0].allocations:
        if not isinstance(alloc, mybir.MemoryLocationSet):
            continue
        name = alloc.memorylocations[0].name
        if alloc.kind == "ExternalInput":
            if name != partition_name:
                in_names.append(name)
        elif alloc.kind == "ExternalOutput":
            out_names.append(name)
            shape = tuple(alloc.tensor_shape)
            dtype = mybir.dt.np(alloc.dtype)
            out_avals.append(jax.core.ShapedArray(shape, dtype))
            out_shapes.append((shape, dtype))
    n_params = len(in_names)
    n_outs = len(out_avals)
    all_names = list(in_names)
    if partition_name is not None:
        all_names.append(partition_name)

    def _body(*args):
        operands = list(args)
        if partition_name is not None:
            operands.append(bass2jax.partition_id_tensor())
        outs = bass2jax._bass_exec_p.bind(
            *operands, out_avals=tuple(out_avals),
            in_names=tuple(all_names), out_names=tuple(out_names),
            lowering_input_output_aliases=(), sim_require_finite=True,
            sim_require_nnan=True, nc=nc)
        return tuple(outs)

    devices = jax.devices()[:ncores]
    mesh = Mesh(np.asarray(devices), ("core",))
    sharded = jax.jit(
        shard_map(_body, mesh=mesh,
                  in_specs=(PartitionSpec("core"),) * n_params,
                  out_specs=(PartitionSpec("core"),) * n_outs,
                  check_rep=False),
        keep_unused=True)

    def run(maps):
        concat_in = [np.concatenate([np.asarray(m[nm]) for m in maps], axis=0)
                     for nm in in_names]
        out_arrs = sharded(*concat_in)
        return [
            {name: np.asarray(out_arrs[i]).reshape(
                ncores, *out_shapes[i][0])[c]
             for i, name in enumerate(out_names)}
            for c in range(ncores)
        ]

    _DISPATCH_CACHE[key] = run
    return run


# --------------------------------------------------------------------------
# entry point
# --------------------------------------------------------------------------

def _in_maps(cfg, prep, wts):
    """Pack per-core inputs into one f32 blob (with i16/bf16 sections)."""
    import ml_dtypes
    LAYF, LAYI, LAYB, F32SZ, I16SZ, TOTAL = _layout(cfg)
    npc = cfg.npc
    o16_base = 2 * F32SZ
    o8_base = 4 * F32SZ + 2 * I16SZ

    blob_shared = np.zeros(TOTAL, np.float32)
    for nm in ("nodeWa", "droneTa", "droneWa", "wcomb0", "wcomb1", "outWT"):
        o, sh = LAYF[nm]
        blob_shared[o:o + sh[0] * sh[1]] = np.asarray(
            wts[nm], np.float32).ravel()
    o, sh = LAYF["smalls"]
    smalls = np.concatenate([np.asarray(wts[nm], np.float32)[0]
                             for nm in ("convb0", "convb1", "lng0", "lng1",
                                        "lnb0", "lnb1", "outb")])
    blob_shared[o:o + sh[0] * sh[1]] = smalls

    batch = np.asarray(wts["batch"]).astype(np.int16)
    maps = []
    for k in range(cfg.ncores):
        blob = blob_shared.copy()
        b16 = blob.view(np.int16)
        b8 = blob.view(np.int8)
        pc = prep["per_core"][k]

        def put16(nm, data16):
            o, sh = LAYI[nm]
            sz = sh[0] * sh[1]
            b16[o16_base + o:o16_base + o + sz] = data16.ravel()

        put16("idx16", pc["idx16"])
        bp = np.zeros(cfg.ngroup * P, np.int16)
        bp[:npc] = batch[k * npc:(k + 1) * npc]
        put16("batch", np.ascontiguousarray(bp.reshape(cfg.ngroup, P).T))
        xbf = np.ascontiguousarray(
            wts["xTa"][:, k * npc:(k + 1) * npc]).astype(
                ml_dtypes.bfloat16).view(np.int16)
        put16("xTbf", xbf)
        o, sh = LAYB["dstslot"]
        b8[o8_base + o:o8_base + o + sh[0] * sh[1]] = \
            pc["dstslot"].astype(np.int8).ravel()
        maps.append(dict(blob=blob))
    return maps


def kernel(**inputs):
    edge_index = np.asarray(inputs["edge_index"])
    prep = _host_prep(edge_index, N, NCORES)
    cfg = _Cfg(N, NCORES, prep["cbs"])
    wts = _host_weights(inputs, prep["order"], N)
    nc = _build(cfg)
    maps = _in_maps(cfg, prep, wts)

    run = _make_dispatch(nc, NCORES)
    res = run(maps)
    out = np.empty((N, OUT_F), np.float32)
    for k in range(NCORES):
        out[prep["order"][k * cfg.npc:(k + 1) * cfg.npc]] = \
            res[k]["out"].astype(np.float32)
    return out


# revision 12
# speedup vs baseline: 30.7177x; 1.0329x over previous
"""GAT (2-layer, 4-head, segment-softmax) message-passing kernel for 8 Trainium2
NeuronCores.

Strategy (dst-sharded, edge aggregation as one-hot matmuls):
  * Nodes are assigned to cores/groups with degree-balanced packing (LPT). The
    node permutation is (core, group, slot) order, so each core owns a
    contiguous block of rows and each group's 128 nodes are contiguous.
  * Phase 1 is SHARDED: each core computes the record table
    rec[n] = [xh(256) | a_src-score(4) | pad] only for its own npc rows, plus
    had[n] = [h(64) | ad(4)]; an 8-core AllGather replicates rec on-device
    (NeuronLink) so phase 2 can gather any source node's record locally.
  * For each destination group (128 nodes), the core gathers the records of
    the group's in-edges' source nodes with gpsimd dma_gather (int16 indices,
    source-bucketed in 32768-row windows), builds the one-hot incidence matrix
    M[edge, dst_slot] on the vector engine (iota compare), broadcasts the
    a_dst scores to edges via transposed-one-hot matmuls, and reduces both the
    softmax denominators and the weighted feature sums with PSUM-accumulated
    matmuls (contracting over edges). Softmax normalization is applied after
    the reduction - mathematically identical to the reference's segment
    softmax (max-subtraction is a no-op at these magnitudes).
  * Host->device traffic is minimized (the axon tunnel is ~65 MB/s with ~75ms
    per-array overhead): ALL inputs are packed into ONE f32 blob per core
    (~2.2MB: f32 weights; int16 gather indices / dst slots / batch ids and
    the bf16 own-shard xT are stored via bitcast views). The drone-feature
    term is an on-device indirect gather of the 64x64 projected table; gread
    offsets are iota-generated on device; the output is returned as bf16.
  * Dispatch uses a cached jitted shard_map executable (compiled once per
    process) plus the JAX persistent compilation cache, so steady-state
    dispatch cost is input upload + execute + output download.
"""

import os
import sys

sys.path.insert(0, "/opt/trn_rl_repo")

import numpy as np

# ---- problem constants (hardcoded; kernel.py must be self-contained) ----
N = 100000
E = 1600000
G = 64
H = 4
CDIM = 64
NODE_F = 32
DRONE_F = 16
OUT_F = 32
LN_EPS = 1e-5
NEG_SLOPE = 0.2
NCORES = 8
P = 128
HC = H * CDIM          # 256
REC = HC + H           # 260: [V(256) | as/ex(4)]
BUCKET = 32768         # int16 index range per dma_gather bucket
TB = 6                 # phase-1 tile batch

REC_DT_NAME = os.environ.get("GAT_REC_DT", "bfloat16")


def _enable_jax_cc():
    import jax
    try:
        jax.config.update("jax_compilation_cache_dir",
                          os.environ.get("JAX_CC_DIR", "/tmp/jax_cc_cache"))
        jax.config.update("jax_persistent_cache_min_entry_size_bytes", 0)
        jax.config.update("jax_persistent_cache_min_compile_time_secs", 0.0)
    except Exception:
        pass


class _Cfg:
    def __init__(self, n, ncores, cbs, rec_dt=REC_DT_NAME, debug=False):
        assert n % ncores == 0
        self.n = n
        self.ncores = ncores
        self.npc = n // ncores
        self.ngroup = -(-self.npc // P)
        self.cbs = cbs                       # [ngroup][nbuckets] chunk counts
        self.nbuckets = len(cbs[0])
        self.chg = [sum(row) for row in cbs]  # chunks per group
        self.chmax = max(self.chg)
        self.cols = sum(self.chg)            # total chunk columns
        self.rec_dt = rec_dt
        self.recp = 320 if rec_dt == "float32" else 384  # padded record elems
        self.debug = debug
        # own-shard tiling (phase 1 + final projection)
        self.nt_full, self.nt_rem = divmod(self.npc, P)
        self.last_cnt = self.npc - (self.ngroup - 1) * P


def _layout(cfg):
    """Single-blob layout. Returns (f32 sections, i16 sections, total f32
    elems). i16 section offsets are in int16 units from the start of the
    int16 region, which begins at f32 elem F32SZ (i16 elem 2*F32SZ)."""
    f32 = {}
    off = 0
    for nm, sh in [("nodeWa", (NODE_F + 1, CDIM)),
                   ("droneTa", (DRONE_F + 1, G)),
                   ("droneWa", (DRONE_F + 1, CDIM)),
                   ("wcomb0", (CDIM, REC + H)),
                   ("wcomb1", (CDIM, REC + H)),
                   ("outWT", (CDIM, OUT_F)),
                   # convb0|convb1|lng0|lng1|lnb0|lnb1|outb rows
                   ("smalls", (1, 6 * CDIM + OUT_F))]:
        f32[nm] = (off, sh)
        off += sh[0] * sh[1]
    f32sz = off
    i16 = {}
    off = 0
    for nm, sh in [("idx16", (16, cfg.cols * 8)),
                   ("batch", (P, cfg.ngroup)),
                   ("xTbf", (NODE_F + 1, cfg.npc))]:
        sz = sh[0] * sh[1]
        i16[nm] = (off, sh)
        off += sz + (sz & 1)                 # keep 32-bit alignment
    i16sz = off
    i8 = {}
    off = 0
    for nm, sh in [("dstslot", (P, cfg.cols))]:
        sz = sh[0] * sh[1]
        i8[nm] = (off, sh)
        off += sz + (-sz) % 4                # keep 32-bit alignment
    total = f32sz + i16sz // 2 + off // 4
    return f32, i16, i8, f32sz, i16sz, total


# --------------------------------------------------------------------------
# host-side preprocessing
# --------------------------------------------------------------------------

def _lpt(loads, caps):
    """LPT packing into len(caps) bins with given item capacities, balancing
    total load. Returns assignment array."""
    import heapq

    nbins = len(caps)
    order = np.argsort(-loads, kind="stable")
    heap = [(0, b) for b in range(nbins)]
    heapq.heapify(heap)
    cnt = np.zeros(nbins, np.int64)
    tot = np.zeros(nbins, np.int64)
    assign = np.empty(len(loads), np.int32)
    for i in order:
        while True:
            _, b = heapq.heappop(heap)
            if cnt[b] < caps[b]:
                break
        assign[i] = b
        cnt[b] += 1
        tot[b] += loads[i]
        if cnt[b] < caps[b]:
            heapq.heappush(heap, (int(tot[b]), b))
    return assign


def _host_prep(edge_index, n, ncores):
    """Node permutation + per-core gather index streams."""
    npc = n // ncores
    ngroup = -(-npc // P)
    last_cnt = npc - (ngroup - 1) * P
    nbuckets = -(-n // BUCKET)

    loop = np.arange(n, dtype=np.int64)
    src = np.concatenate([edge_index[0].astype(np.int64), loop])
    dst = np.concatenate([edge_index[1].astype(np.int64), loop])
    deg = np.bincount(dst, minlength=n)

    core_of = _lpt(deg, [npc] * ncores)
    group_of = np.empty(n, np.int32)
    slot_of = np.empty(n, np.int32)
    pos_of = np.empty(n, np.int64)
    order = np.empty(n, np.int64)
    caps = [P] * (ngroup - 1) + [last_cnt]
    for k in range(ncores):
        nodes_k = np.where(core_of == k)[0]
        g_assign = _lpt(deg[nodes_k], caps)
        o = np.argsort(g_assign, kind="stable")
        cnts = np.bincount(g_assign, minlength=ngroup)
        starts = np.concatenate([[0], np.cumsum(cnts)])[:-1]
        slot = np.empty(len(nodes_k), np.int64)
        slot[o] = np.arange(len(nodes_k)) - starts[g_assign[o]]
        group_of[nodes_k] = g_assign
        slot_of[nodes_k] = slot
        pos = k * npc + g_assign * P + slot
        pos_of[nodes_k] = pos
        order[pos] = nodes_k

    # per-(group,bucket) edge counts per core -> uniform chunk schedule
    e_core = core_of[dst]
    e_group = group_of[dst]
    e_bucket = pos_of[src] // BUCKET
    cnts = np.zeros((ncores, ngroup, nbuckets), np.int64)
    np.add.at(cnts, (e_core, e_group, e_bucket), 1)
    cbs_np = -(-cnts.max(axis=0) // P)       # [ngroup, nbuckets] chunks
    cbs = [[int(c) for c in row] for row in cbs_np]
    chg = np.array([sum(row) for row in cbs])
    cols = int(chg.sum())
    goff = np.concatenate([[0], np.cumsum(chg)])[:-1]
    boff = np.zeros((ngroup, nbuckets), np.int64)
    for g in range(ngroup):
        o = goff[g]
        for b in range(nbuckets):
            boff[g, b] = o
            o += cbs[g][b]

    per_core = []
    for k in range(ncores):
        mask = e_core == k
        es = pos_of[src[mask]]
        eg = e_group[mask]
        eb = e_bucket[mask]
        esl = slot_of[dst[mask]]
        o = np.lexsort((eb, eg))
        es, eg, eb, esl = es[o], eg[o], eb[o], esl[o]
        cnt_k = np.zeros((ngroup, nbuckets), np.int64)
        np.add.at(cnt_k, (eg, eb), 1)
        flat = cnt_k.reshape(-1)
        starts = np.concatenate([[0], np.cumsum(flat)])[:-1].reshape(
            ngroup, nbuckets)
        j = np.arange(len(es)) - starts[eg, eb]      # pos within (g,b)
        slotj = boff[eg, eb] * P + j                 # global slot in stream

        dstslot = np.full((P, cols), -1, np.int16)
        dstslot[slotj % P, slotj // P] = esl
        idx16 = np.zeros((16, cols * 8), np.int16)   # 8 int16 cols per chunk
        idx16[slotj % 16, slotj // 16] = es - eb * BUCKET
        per_core.append(dict(dstslot=dstslot, idx16=idx16))
    return dict(order=order, pos_of=pos_of, cbs=cbs, per_core=per_core)


def _host_weights(inputs, order, n):
    """Permuted/augmented weight + input tensors (all float32)."""
    f = np.float32
    x = np.asarray(inputs["x"], f)[order]            # perm rows
    batch = np.asarray(inputs["batch"])[order]
    xTa = np.concatenate([x.T, np.ones((1, n), f)], 0)           # [33, n]
    droneTa = np.concatenate(
        [np.asarray(inputs["drone_feat"], f).T, np.ones((1, G), f)], 0)
    droneWa = np.concatenate(
        [np.asarray(inputs["drone_W"], f).T, np.asarray(inputs["drone_b"], f)[None]], 0)
    nodeWa = np.concatenate(
        [np.asarray(inputs["node_W"], f).T, np.asarray(inputs["node_b"], f)[None]], 0)
    out = dict(xTa=xTa, batch=batch, droneTa=droneTa, droneWa=droneWa,
               nodeWa=nodeWa,
               outWT=np.ascontiguousarray(np.asarray(inputs["out_W"], f).T),
               outb=np.tile(np.asarray(inputs["out_b"], f), (P, 1)))
    for l in range(2):
        W = np.asarray(inputs[f"convW{l}"], f)       # [HC, CDIM]
        a_s = np.asarray(inputs[f"att_src{l}"], f)   # [H, CDIM]
        a_d = np.asarray(inputs[f"att_dst{l}"], f)
        Wh = W.reshape(H, CDIM, CDIM)
        Ws = np.einsum("hcf,hc->fh", Wh, a_s)        # [CDIM, H]
        Wd = np.einsum("hcf,hc->fh", Wh, a_d)
        out[f"wcomb{l}"] = np.concatenate([W.T, Ws, Wd], 1)   # [CDIM, 264]
        out[f"convb{l}"] = np.tile(np.asarray(inputs[f"convb{l}"], f), (P, 1))
        out[f"lng{l}"] = np.tile(np.asarray(inputs[f"ln_g{l}"], f), (P, 1))
        out[f"lnb{l}"] = np.tile(np.asarray(inputs[f"ln_b{l}"], f), (P, 1))
    return out


# --------------------------------------------------------------------------
# bass kernel
# --------------------------------------------------------------------------

def _build(cfg):
    import concourse.bass as bass
    import concourse.bacc as bacc
    import concourse.tile as tile
    from concourse import mybir
    from concourse.masks import make_identity

    f32 = mybir.dt.float32
    i32 = mybir.dt.int32
    i16 = mybir.dt.int16
    i8 = mybir.dt.int8
    bf16 = mybir.dt.bfloat16
    rdt = getattr(mybir.dt, cfg.rec_dt)
    is_bf = cfg.rec_dt != "float32"
    Alu = mybir.AluOpType
    Act = mybir.ActivationFunctionType

    n, npc, ngroup = cfg.n, cfg.npc, cfg.ngroup
    RECP, CHMAX = cfg.recp, cfg.chmax
    LAYF, LAYI, LAYB, F32SZ, I16SZ, TOTAL = _layout(cfg)

    nc = bacc.Bacc("TRN2", target_bir_lowering=False, debug=cfg.debug,
                   num_devices=cfg.ncores)

    blob_d = nc.dram_tensor("blob", [TOTAL], f32, kind="ExternalInput")

    def fview(nm):
        o, sh = LAYF[nm]
        return blob_d[o:o + sh[0] * sh[1]].rearrange("(a b) -> a b", a=sh[0])

    def iview(nm, dt):
        o, sh = LAYI[nm]
        sz = sh[0] * sh[1]
        o32 = F32SZ + o // 2                 # o is even by construction
        return blob_d[o32:o32 + (sz + 1) // 2].bitcast(dt)[
            0:sz].rearrange("(a b) -> a b", a=sh[0])

    def bview(nm, dt):
        o, sh = LAYB[nm]
        sz = sh[0] * sh[1]
        o32 = F32SZ + I16SZ // 2 + o // 4    # o is 4-aligned by construction
        return blob_d[o32:o32 + (sz + 3) // 4].bitcast(dt)[
            0:sz].rearrange("(a b) -> a b", a=sh[0])

    out_d = nc.dram_tensor("out", [npc, OUT_F], bf16, kind="ExternalOutput")

    rec_loc_d = nc.dram_tensor("rec_loc", [npc, RECP], rdt)
    rec_d = nc.dram_tensor("rec", [n, RECP], rdt,
                           addr_space="Shared" if cfg.ncores > 1 else "Local")
    had_d = [nc.dram_tensor(f"had{l}", [npc, CDIM + H], f32) for l in range(2)]
    stag_d = [nc.dram_tensor(f"stag{l}", [ngroup * P, CDIM], f32)
              for l in range(2)]
    idxrep_d = nc.dram_tensor("idxrep", [P, cfg.cols * 8], i16)
    dr_d = nc.dram_tensor("dr", [G, CDIM], f32)

    from contextlib import ExitStack
    with tile.TileContext(nc) as tc, ExitStack() as ctx:
        cpool = ctx.enter_context(tc.tile_pool(name="const", bufs=1))
        p1 = ctx.enter_context(tc.tile_pool(name="p1", bufs=2))
        p2 = ctx.enter_context(tc.tile_pool(name="p2", bufs=2))

        def cload(nm):
            o, sh = LAYF[nm]
            t = cpool.tile(list(sh), f32, tag=f"c_{nm}")
            nc.sync.dma_start(out=t[:], in_=fview(nm))
            return t

        droneTa_sb = cload("droneTa")
        droneWa_sb = cload("droneWa")
        nodeWa_sb = cload("nodeWa")
        wcomb_sb = [cload("wcomb0"), cload("wcomb1")]
        outWT_sb = cload("outWT")
        nodeWb_sb = cpool.tile([NODE_F + 1, CDIM], bf16, tag="nodeWb")
        nc.vector.tensor_copy(nodeWb_sb[:], nodeWa_sb[:])

        # broadcast the bias/LN rows [1, 416] to all 128 partitions via a
        # ones-column matmul, then slice views
        SMW = 6 * CDIM + OUT_F
        smrow_sb = cload("smalls")           # [1, SMW]
        ones_sb = cpool.tile([1, P], f32, tag="ones1")
        nc.vector.memset(ones_sb[:], 1.0)
        smallsb = cpool.tile([P, SMW], f32, tag="smallsb")
        with tc.tile_pool(name="pssm", bufs=1, space="PSUM") as ppsm:
            psm = ppsm.tile([P, SMW], f32)
            nc.tensor.matmul(psm[:], lhsT=ones_sb[:], rhs=smrow_sb[:],
                             start=True, stop=True)
            nc.scalar.copy(smallsb[:], psm[:])
        convb_sb = [smallsb[:, 0:CDIM], smallsb[:, CDIM:2 * CDIM]]
        lng_sb = [smallsb[:, 2 * CDIM:3 * CDIM], smallsb[:, 3 * CDIM:4 * CDIM]]
        lnb_sb = [smallsb[:, 4 * CDIM:5 * CDIM], smallsb[:, 5 * CDIM:6 * CDIM]]
        outb_sb = smallsb[:, 6 * CDIM:6 * CDIM + OUT_F]

        # int8/int16 streams: dst slots + batch ids (+ iota / gread)
        dst8_sb = cpool.tile([P, cfg.cols], i8, tag="dst8")
        nc.sync.dma_start(out=dst8_sb[:], in_=bview("dstslot", i8))
        dst16_sb = cpool.tile([P, cfg.cols], i16, tag="dst16")
        nc.vector.tensor_copy(dst16_sb[:], dst8_sb[:])
        bat16_sb = cpool.tile([P, ngroup], i16, tag="bat16")
        nc.sync.dma_start(out=bat16_sb[:], in_=iview("batch", i16))
        bat32_sb = cpool.tile([P, ngroup], i32, tag="bat32")
        nc.vector.tensor_copy(bat32_sb[:], bat16_sb[:])

        # replicate the 16-partition gather-index stream to 128 partitions
        # (dma_gather wants idxs wrapped in 16 partitions x 8 gpsimd cores)
        for k8 in range(8):
            nc.sync.dma_start(out=idxrep_d[k8 * 16:(k8 + 1) * 16, :],
                              in_=iview("idx16", i16))

        iota_sb = cpool.tile([P, P], i32)
        nc.gpsimd.iota(iota_sb[:], pattern=[[1, P]], base=0,
                       channel_multiplier=0)
        iota16_sb = cpool.tile([P, P], i16)
        nc.vector.tensor_copy(iota16_sb[:], iota_sb[:])
        ident_sb = cpool.tile([P, P], f32)
        make_identity(nc, ident_sb[:])
        identr_sb = ident_sb
        if is_bf:
            identr_sb = cpool.tile([P, P], rdt)
            nc.vector.tensor_copy(identr_sb[:], ident_sb[:])

        # gread[p, g] = min(g*128 + p, npc-1): offsets into local had_d
        gread_sb = cpool.tile([P, ngroup], i32, tag="gread")
        nc.gpsimd.iota(gread_sb[:], pattern=[[P, ngroup]], base=0,
                       channel_multiplier=1)
        nc.vector.tensor_scalar(gread_sb[:], gread_sb[:], npc - 1, None,
                                Alu.min)

        # dr = droneTa.T @ droneWa  -> dram (indirect-gather source)
        dr_sb = cpool.tile([G, CDIM], f32)
        with tc.tile_pool(name="psdr", bufs=1, space="PSUM") as ppdr:
            pdr_t = ppdr.tile([P, CDIM], f32)
            pdr = pdr_t[:G]
            nc.tensor.matmul(pdr, lhsT=droneTa_sb[:], rhs=droneWa_sb[:],
                             start=True, stop=True)
            nc.scalar.copy(dr_sb[:], pdr)
        nc.sync.dma_start(out=dr_d[:, :], in_=dr_sb[:])

        # ------------------------------------------------------------------
        def phase1(l):
            """Build rec_loc[npc, RECP] and had[npc, 68] tile by tile
            (own shard only; AllGather replicates rec afterwards)."""
            xT_v = iview("xTbf", bf16)
            with tc.tile_pool(name=f"ps1_{l}", bufs=2, space="PSUM") as pp:

                def do_batch(r0, tb, rows):
                    if l == 0:
                        xb = p1.tile([NODE_F + 1, TB * P], bf16, tag="xb")
                        nc.sync.dma_start(out=xb[:, :rows],
                                          in_=xT_v[:, r0:r0 + rows])
                    hadb = p1.tile([P, TB, CDIM + H], f32, tag="hadb")
                    if l == 1:
                        if rows == tb * P:
                            nc.sync.dma_start(
                                out=hadb[:, :tb, :CDIM],
                                in_=stag_d[0][r0:r0 + rows, :].rearrange(
                                    "(c p) f -> p c f", p=P))
                        else:
                            nc.sync.dma_start(out=hadb[:rows, 0, :CDIM],
                                              in_=stag_d[0][r0:r0 + rows, :])
                    recb = p1.tile([P, TB, RECP], rdt, tag="recb")
                    nc.vector.memset(recb[:, :, REC:], 0.0)
                    for t in range(tb):
                        pr_ = min(P, rows - t * P)
                        g_abs = r0 // P + t
                        if l == 0:
                            drt = p1.tile([P, CDIM], f32, tag="drt")
                            nc.gpsimd.indirect_dma_start(
                                out=drt[:], out_offset=None, in_=dr_d[:],
                                in_offset=bass.IndirectOffsetOnAxis(
                                    ap=bat32_sb[:, g_abs:g_abs + 1], axis=0))
                            ph = pp.tile([P, CDIM], f32, tag="ph")
                            nc.tensor.matmul(ph[:pr_],
                                             lhsT=xb[:, t * P:t * P + pr_],
                                             rhs=nodeWb_sb[:], start=True,
                                             stop=True)
                            nc.vector.tensor_tensor(hadb[:pr_, t, :CDIM],
                                                    ph[:pr_], drt[:pr_],
                                                    Alu.add)
                        pt = pp.tile([CDIM, P], f32, tag="pt")
                        nc.tensor.transpose(pt[:, :pr_], hadb[:pr_, t, :CDIM],
                                            ident_sb[:pr_, :pr_])
                        hT = p1.tile([CDIM, P], f32, tag="hT")
                        nc.scalar.copy(hT[:, :pr_], pt[:, :pr_])
                        prc = pp.tile([P, REC + H], f32, tag="pr")
                        nc.tensor.matmul(prc[:pr_], lhsT=hT[:, :pr_],
                                         rhs=wcomb_sb[l][:], start=True,
                                         stop=True)
                        nc.scalar.copy(recb[:pr_, t, 0:REC], prc[:pr_, 0:REC])
                        nc.vector.tensor_copy(hadb[:pr_, t, CDIM:],
                                              prc[:pr_, REC:REC + H])
                    if rows == tb * P:
                        nc.sync.dma_start(
                            out=rec_loc_d[r0:r0 + rows, :].rearrange(
                                "(c p) f -> p c f", p=P),
                            in_=recb[:, :tb, :])
                        nc.sync.dma_start(
                            out=had_d[l][r0:r0 + rows, :].rearrange(
                                "(c p) f -> p c f", p=P),
                            in_=hadb[:, :tb, :])
                    else:
                        nc.sync.dma_start(out=rec_loc_d[r0:r0 + rows, :],
                                          in_=recb[:rows, 0, :])
                        nc.sync.dma_start(out=had_d[l][r0:r0 + rows, :],
                                          in_=hadb[:rows, 0, :])

                for b0 in range(0, cfg.nt_full, TB):
                    tb = min(TB, cfg.nt_full - b0)
                    do_batch(b0 * P, tb, tb * P)
                if cfg.nt_rem:
                    do_batch(cfg.nt_full * P, 1, cfg.nt_rem)

        def gather_rec():
            nc.gpsimd.collective_compute(
                "AllGather", mybir.AluOpType.bypass,
                replica_groups=[list(range(cfg.ncores))],
                ins=[rec_loc_d[0:npc, :].opt()],
                outs=[rec_d[:, :].opt()])

        # ------------------------------------------------------------------
        def phase2(l):
            with tc.tile_pool(name=f"ps2_{l}", bufs=2, space="PSUM") as pp:
                col0 = 0
                for g in range(ngroup):
                    CH = cfg.chg[g]
                    rows_g = P if g < ngroup - 1 else cfg.last_cnt
                    idxt = p2.tile([P, CHMAX * 8], i16, tag="idxt")
                    nc.sync.dma_start(out=idxt[:, :CH * 8],
                                      in_=idxrep_d[:, col0 * 8:(col0 + CH) * 8])
                    rect = p2.tile([P, CHMAX, RECP], rdt, tag="rect")
                    c0 = 0
                    for b in range(cfg.nbuckets):
                        cb = cfg.cbs[g][b]
                        if cb == 0:
                            continue
                        nrows = min(BUCKET, n - b * BUCKET)
                        done = 0
                        while done < cb:   # HW envelope: <=256 idxs per call
                            st = min(2, cb - done)
                            nc.gpsimd.dma_gather(
                                rect[:, c0 + done:c0 + done + st, :],
                                rec_d[b * BUCKET:b * BUCKET + nrows, :],
                                idxt[:, (c0 + done) * 8:(c0 + done + st) * 8],
                                st * P, st * P, RECP)
                            done += st
                        c0 += cb
                    # h_old + a_dst rows for this group's nodes
                    hadt = p2.tile([P, CDIM + H], f32, tag="hadt")
                    nc.gpsimd.indirect_dma_start(
                        out=hadt[:], out_offset=None, in_=had_d[l][:],
                        in_offset=bass.IndirectOffsetOnAxis(
                            ap=gread_sb[:, g:g + 1], axis=0))
                    ad_rhs = hadt[:, CDIM:]
                    if is_bf:
                        adr = p2.tile([P, H], rdt, tag="adr")
                        nc.vector.tensor_copy(adr[:], hadt[:, CDIM:])
                        ad_rhs = adr[:]
                    # one-hot M[edge, dst_slot]
                    Mt = p2.tile([P, CHMAX, P], rdt, tag="Mt")
                    nc.vector.tensor_tensor(
                        Mt[:, :CH, :],
                        dst16_sb[:, col0:col0 + CH][:, :, None].to_broadcast(
                            [P, CH, P]),
                        iota16_sb[:, None, :].to_broadcast([P, CH, P]),
                        Alu.is_equal)
                    # e_d: broadcast a_dst scores to edges via M^T matmuls
                    ped = pp.tile([P, CHMAX * H], f32, tag="ped")
                    for c in range(CH):
                        pmt = pp.tile([P, P], rdt, tag="pmt")
                        nc.tensor.transpose(pmt[:], Mt[:, c, :], identr_sb[:])
                        mt_sb = p2.tile([P, P], rdt, tag="mt_sb")
                        nc.scalar.copy(mt_sb[:], pmt[:])
                        nc.tensor.matmul(ped[:, c * H:(c + 1) * H],
                                         lhsT=mt_sb[:], rhs=ad_rhs,
                                         start=True, stop=True)
                    # e = lrelu(as + ad); ex = exp(e) -> rec[..., 256:260]
                    et = p2.tile([P, CHMAX, H], f32, tag="et")
                    nc.vector.tensor_tensor(
                        et[:, :CH, :], rect[:, :CH, HC:REC],
                        ped[:, 0:CH * H].rearrange("p (c h) -> p c h", h=H),
                        Alu.add)
                    lt = p2.tile([P, CHMAX, H], f32, tag="lt")
                    nc.vector.tensor_scalar_mul(lt[:, :CH, :], et[:, :CH, :],
                                                NEG_SLOPE)
                    nc.vector.tensor_tensor(et[:, :CH, :], lt[:, :CH, :],
                                            et[:, :CH, :], Alu.max)
                    nc.scalar.activation(rect[:, :CH, HC:REC], et[:, :CH, :],
                                         Act.Exp)
                    # V = ex * xh (per head, in place)
                    for h_ in range(H):
                        nc.vector.tensor_tensor(
                            rect[:, :CH, h_ * CDIM:(h_ + 1) * CDIM],
                            rect[:, :CH, h_ * CDIM:(h_ + 1) * CDIM],
                            rect[:, :CH, HC + h_:HC + h_ + 1].to_broadcast(
                                [P, CH, CDIM]),
                            Alu.mult)
                    # contract over edges: psum[:, 0:256]=sum alpha*xh, [256:260]=s
                    pg = pp.tile([P, REC], f32, tag="pg")
                    for c in range(CH):
                        nc.tensor.matmul(pg[:], lhsT=Mt[:, c, :],
                                         rhs=rect[:, c, 0:REC],
                                         start=(c == 0), stop=(c == CH - 1))
                    # r = 1 / (s + eps) / H
                    s4 = p2.tile([P, H], f32, tag="s4")
                    nc.vector.tensor_scalar(s4[:], pg[:, HC:REC], 1e-16, None,
                                            Alu.add)
                    r4 = p2.tile([P, H], f32, tag="r4")
                    nc.vector.reciprocal(r4[:], s4[:])
                    nc.vector.tensor_scalar_mul(r4[:], r4[:], 1.0 / H)
                    # head mean
                    yt = p2.tile([P, CDIM], f32, tag="yt")
                    tmp = p2.tile([P, CDIM], f32, tag="tmp")
                    nc.vector.tensor_scalar(yt[:], pg[:, 0:CDIM], r4[:, 0:1],
                                            None, Alu.mult)
                    for h_ in range(1, H):
                        nc.vector.tensor_scalar(tmp[:],
                                                pg[:, h_ * CDIM:(h_ + 1) * CDIM],
                                                r4[:, h_:h_ + 1], None, Alu.mult)
                        nc.vector.tensor_add(yt[:], yt[:], tmp[:])
                    nc.vector.tensor_add(yt[:], yt[:], convb_sb[l])
                    # layernorm
                    mu = p2.tile([P, 1], f32, tag="mu")
                    nc.vector.tensor_reduce(mu[:], yt[:], mybir.AxisListType.X,
                                            Alu.add)
                    nc.vector.tensor_scalar_mul(mu[:], mu[:], 1.0 / CDIM)
                    nc.vector.tensor_scalar(yt[:], yt[:], mu[:, 0:1], None,
                                            Alu.subtract)
                    sq = p2.tile([P, CDIM], f32, tag="sq")
                    var = p2.tile([P, 1], f32, tag="var")
                    nc.scalar.activation(sq[:], yt[:], Act.Square,
                                         accum_out=var[:])
                    nc.vector.tensor_scalar(var[:], var[:], 1.0 / CDIM, LN_EPS,
                                            Alu.mult, Alu.add)
                    sd = p2.tile([P, 1], f32, tag="sd")
                    nc.scalar.sqrt(sd[:], var[:])
                    inv = p2.tile([P, 1], f32, tag="inv")
                    nc.vector.reciprocal(inv[:], sd[:])
                    nc.vector.tensor_scalar(yt[:], yt[:], inv[:, 0:1], None,
                                            Alu.mult)
                    nc.vector.tensor_mul(yt[:], yt[:], lng_sb[l])
                    nc.vector.tensor_add(yt[:], yt[:], lnb_sb[l])
                    nc.vector.tensor_scalar_max(yt[:], yt[:], 0.0)
                    # residual + contiguous staging write
                    nc.vector.tensor_add(yt[:], yt[:], hadt[:, 0:CDIM])
                    nc.sync.dma_start(out=stag_d[l][g * P:g * P + rows_g, :],
                                      in_=yt[:rows_g, :])
                    col0 += CH

        # ------------------------------------------------------------------
        phase1(0)
        gather_rec()
        phase2(0)
        phase1(1)
        gather_rec()
        phase2(1)

        # final projection over own rows
        with tc.tile_pool(name="psf", bufs=2, space="PSUM") as pp:
            for t0 in range(0, npc, P):
                wr = min(P, npc - t0)
                ht2 = p2.tile([P, CDIM], f32, tag="ht2")
                nc.sync.dma_start(out=ht2[:wr], in_=stag_d[1][t0:t0 + wr, :])
                pt2 = pp.tile([CDIM, P], f32, tag="pt2")
                nc.tensor.transpose(pt2[:, :wr], ht2[:wr], ident_sb[:wr, :wr])
                hT2 = p2.tile([CDIM, P], f32, tag="hT2")
                nc.scalar.copy(hT2[:, :wr], pt2[:, :wr])
                po = pp.tile([P, OUT_F], f32, tag="po")
                nc.tensor.matmul(po[:wr], lhsT=hT2[:, :wr], rhs=outWT_sb[:],
                                 start=True, stop=True)
                ot = p2.tile([P, OUT_F], bf16, tag="ot")
                nc.vector.tensor_add(ot[:wr], po[:wr], outb_sb[:wr])
                nc.sync.dma_start(out=out_d[t0:t0 + wr, :], in_=ot[:wr, :])

    nc.compile()
    return nc


# --------------------------------------------------------------------------
# dispatch (cached jitted shard_map; mirrors bass2jax.run_bass_via_pjrt)
# --------------------------------------------------------------------------

_DISPATCH_CACHE = {}


def _make_dispatch(nc, ncores):
    key = id(nc)
    if key in _DISPATCH_CACHE:
        return _DISPATCH_CACHE[key]

    _enable_jax_cc()
    import jax
    from jax.sharding import Mesh, PartitionSpec
    from jax.experimental.shard_map import shard_map
    from concourse import bass2jax, mybir

    bass2jax.install_neuronx_cc_hook()
    partition_name = (nc.partition_id_tensor.name
                      if nc.partition_id_tensor else None)
    in_names, out_names, out_avals, out_shapes = [], [], [], []
    for alloc in nc.m.functions[0].allocations:
        if not isinstance(alloc, mybir.MemoryLocationSet):
            continue
        name = alloc.memorylocations[0].name
        if alloc.kind == "ExternalInput":
            if name != partition_name:
                in_names.append(name)
        elif alloc.kind == "ExternalOutput":
            out_names.append(name)
            shape = tuple(alloc.tensor_shape)
            dtype = mybir.dt.np(alloc.dtype)
            out_avals.append(jax.core.ShapedArray(shape, dtype))
            out_shapes.append((shape, dtype))
    n_params = len(in_names)
    n_outs = len(out_avals)
    all_names = list(in_names)
    if partition_name is not None:
        all_names.append(partition_name)

    def _body(*args):
        operands = list(args)
        if partition_name is not None:
            operands.append(bass2jax.partition_id_tensor())
        outs = bass2jax._bass_exec_p.bind(
            *operands, out_avals=tuple(out_avals),
            in_names=tuple(all_names), out_names=tuple(out_names),
            lowering_input_output_aliases=(), sim_require_finite=True,
            sim_require_nnan=True, nc=nc)
        return tuple(outs)

    devices = jax.devices()[:ncores]
    mesh = Mesh(np.asarray(devices), ("core",))
    sharded = jax.jit(
        shard_map(_body, mesh=mesh,
                  in_specs=(PartitionSpec("core"),) * n_params,
                  out_specs=(PartitionSpec("core"),) * n_outs,
                  check_rep=False),
        keep_unused=True)

    import concurrent.futures as _cf
    _pool = _cf.ThreadPoolExecutor(ncores)

    def run(maps):
        concat_in = [np.concatenate([np.asarray(m[nm]) for m in maps], axis=0)
                     for nm in in_names]
        out_arrs = sharded(*concat_in)
        res = [dict() for _ in range(ncores)]
        for i, name in enumerate(out_names):
            rows = out_shapes[i][0][0]
            shards = out_arrs[i].addressable_shards
            datas = list(_pool.map(lambda s: np.asarray(s.data), shards))
            for s, d in zip(shards, datas):
                res[s.index[0].start // rows][name] = d
        return res

    _DISPATCH_CACHE[key] = run
    return run


# --------------------------------------------------------------------------
# entry point
# --------------------------------------------------------------------------

def _in_maps(cfg, prep, wts):
    """Pack per-core inputs into one f32 blob (with i16/bf16 sections)."""
    import ml_dtypes
    LAYF, LAYI, LAYB, F32SZ, I16SZ, TOTAL = _layout(cfg)
    npc = cfg.npc
    o16_base = 2 * F32SZ
    o8_base = 4 * F32SZ + 2 * I16SZ

    blob_shared = np.zeros(TOTAL, np.float32)
    for nm in ("nodeWa", "droneTa", "droneWa", "wcomb0", "wcomb1", "outWT"):
        o, sh = LAYF[nm]
        blob_shared[o:o + sh[0] * sh[1]] = np.asarray(
            wts[nm], np.float32).ravel()
    o, sh = LAYF["smalls"]
    smalls = np.concatenate([np.asarray(wts[nm], np.float32)[0]
                             for nm in ("convb0", "convb1", "lng0", "lng1",
                                        "lnb0", "lnb1", "outb")])
    blob_shared[o:o + sh[0] * sh[1]] = smalls

    batch = np.asarray(wts["batch"]).astype(np.int16)
    maps = []
    for k in range(cfg.ncores):
        blob = blob_shared.copy()
        b16 = blob.view(np.int16)
        b8 = blob.view(np.int8)
        pc = prep["per_core"][k]

        def put16(nm, data16):
            o, sh = LAYI[nm]
            sz = sh[0] * sh[1]
            b16[o16_base + o:o16_base + o + sz] = data16.ravel()

        put16("idx16", pc["idx16"])
        bp = np.zeros(cfg.ngroup * P, np.int16)
        bp[:npc] = batch[k * npc:(k + 1) * npc]
        put16("batch", np.ascontiguousarray(bp.reshape(cfg.ngroup, P).T))
        xbf = np.ascontiguousarray(
            wts["xTa"][:, k * npc:(k + 1) * npc]).astype(
                ml_dtypes.bfloat16).view(np.int16)
        put16("xTbf", xbf)
        o, sh = LAYB["dstslot"]
        b8[o8_base + o:o8_base + o + sh[0] * sh[1]] = \
            pc["dstslot"].astype(np.int8).ravel()
        maps.append(dict(blob=blob))
    return maps


def kernel(**inputs):
    edge_index = np.asarray(inputs["edge_index"])
    prep = _host_prep(edge_index, N, NCORES)
    cfg = _Cfg(N, NCORES, prep["cbs"])
    wts = _host_weights(inputs, prep["order"], N)
    nc = _build(cfg)
    maps = _in_maps(cfg, prep, wts)

    run = _make_dispatch(nc, NCORES)
    res = run(maps)
    out = np.empty((N, OUT_F), np.float32)
    for k in range(NCORES):
        out[prep["order"][k * cfg.npc:(k + 1) * cfg.npc]] = \
            res[k]["out"].astype(np.float32)
    return out


# revision 19
# speedup vs baseline: 34.1117x; 1.1105x over previous
"""GAT (2-layer, 4-head, segment-softmax) message-passing kernel for 8 Trainium2
NeuronCores.

Strategy (dst-sharded, edge aggregation as one-hot matmuls):
  * Nodes are assigned to cores/groups with degree-balanced packing (LPT). The
    node permutation is (core, group, slot) order, so each core owns a
    contiguous block of rows and each group's 128 nodes are contiguous.
  * Phase 1 is SHARDED: each core computes the record table
    rec[n] = [xh(256) | a_src-score(4) | pad] only for its own npc rows, plus
    had[n] = [h(64) | ad(4)]; an 8-core AllGather replicates rec on-device
    (NeuronLink) so phase 2 can gather any source node's record locally.
  * For each destination group (128 nodes), the core gathers the records of
    the group's in-edges' source nodes with gpsimd dma_gather (int16 indices,
    source-bucketed in 32768-row windows), builds the one-hot incidence matrix
    M[edge, dst_slot] on the vector engine (iota compare), broadcasts the
    a_dst scores to edges via transposed-one-hot matmuls, and reduces both the
    softmax denominators and the weighted feature sums with PSUM-accumulated
    matmuls (contracting over edges). Softmax normalization is applied after
    the reduction - mathematically identical to the reference's segment
    softmax (max-subtraction is a no-op at these magnitudes).
  * Host->device traffic is minimized (the axon tunnel is ~65 MB/s with ~75ms
    per-array overhead): ALL inputs are packed into ONE f32 blob per core
    (~2.2MB: f32 weights; int16 gather indices / dst slots / batch ids and
    the bf16 own-shard xT are stored via bitcast views). The drone-feature
    term is an on-device indirect gather of the 64x64 projected table; gread
    offsets are iota-generated on device; the output is returned as bf16.
  * Dispatch uses a cached jitted shard_map executable (compiled once per
    process) plus the JAX persistent compilation cache, so steady-state
    dispatch cost is input upload + execute + output download.
"""

import os
import sys

sys.path.insert(0, "/opt/trn_rl_repo")

import numpy as np

# ---- problem constants (hardcoded; kernel.py must be self-contained) ----
N = 100000
E = 1600000
G = 64
H = 4
CDIM = 64
NODE_F = 32
DRONE_F = 16
OUT_F = 32
LN_EPS = 1e-5
NEG_SLOPE = 0.2
NCORES = 8
P = 128
HC = H * CDIM          # 256
REC = HC + H           # 260: [V(256) | as/ex(4)]
BUCKET = 32768         # int16 index range per dma_gather bucket
TB = 6                 # phase-1 tile batch

REC_DT_NAME = os.environ.get("GAT_REC_DT", "bfloat16")


def _enable_jax_cc():
    import jax
    try:
        jax.config.update("jax_compilation_cache_dir",
                          os.environ.get("JAX_CC_DIR", "/tmp/jax_cc_cache"))
        jax.config.update("jax_persistent_cache_min_entry_size_bytes", 0)
        jax.config.update("jax_persistent_cache_min_compile_time_secs", 0.0)
    except Exception:
        pass


class _Cfg:
    def __init__(self, n, ncores, cbs, rec_dt=REC_DT_NAME, debug=False):
        assert n % ncores == 0
        self.n = n
        self.ncores = ncores
        self.npc = n // ncores
        self.ngroup = -(-self.npc // P)
        self.cbs = cbs                       # [ngroup][nbuckets] chunk counts
        self.nbuckets = len(cbs[0])
        self.chg = [sum(row) for row in cbs]  # chunks per group
        self.chmax = max(self.chg)
        self.cols = sum(self.chg)            # total chunk columns
        self.rec_dt = rec_dt
        self.recp = 320 if rec_dt == "float32" else 384  # padded record elems
        self.debug = debug
        # own-shard tiling (phase 1 + final projection)
        self.nt_full, self.nt_rem = divmod(self.npc, P)
        self.last_cnt = self.npc - (self.ngroup - 1) * P


def _layout(cfg):
    """Single-blob layout. Returns (f32 sections, i16 sections, total f32
    elems). i16 section offsets are in int16 units from the start of the
    int16 region, which begins at f32 elem F32SZ (i16 elem 2*F32SZ)."""
    f32 = {}
    off = 0
    for nm, sh in [("nodeWa", (NODE_F, CDIM)),
                   ("droneTa", (DRONE_F + 1, G)),
                   ("droneWa", (DRONE_F + 1, CDIM)),
                   ("outWT", (CDIM, OUT_F)),
                   # convb0|convb1|lng0|lng1|lnb0|lnb1|outb rows
                   ("smalls", (1, 6 * CDIM + OUT_F))]:
        f32[nm] = (off, sh)
        off += sh[0] * sh[1]
    f32sz = off
    i16 = {}
    off = 0
    for nm, sh in [("idx16", (16, cfg.cols * 8)),
                   ("wcomb0", (CDIM, REC + H)),   # bf16 bits
                   ("wcomb1", (CDIM, REC + H)),   # bf16 bits
                   ("xTbf", (NODE_F, cfg.npc))]:
        sz = sh[0] * sh[1]
        i16[nm] = (off, sh)
        off += sz + (sz & 1)                 # keep 32-bit alignment
    i16sz = off
    i8 = {}
    off = 0
    for nm, sh in [("dstslot", (P, cfg.cols)),
                   ("batch", (P, cfg.ngroup))]:
        sz = sh[0] * sh[1]
        i8[nm] = (off, sh)
        off += sz + (-sz) % 4                # keep 32-bit alignment
    total = f32sz + i16sz // 2 + off // 4
    return f32, i16, i8, f32sz, i16sz, total


# --------------------------------------------------------------------------
# host-side preprocessing
# --------------------------------------------------------------------------

def _lpt(loads, caps):
    """LPT packing into len(caps) bins with given item capacities, balancing
    total load. Returns assignment array."""
    import heapq

    nbins = len(caps)
    order = np.argsort(-loads, kind="stable")
    heap = [(0, b) for b in range(nbins)]
    heapq.heapify(heap)
    cnt = np.zeros(nbins, np.int64)
    tot = np.zeros(nbins, np.int64)
    assign = np.empty(len(loads), np.int32)
    for i in order:
        while True:
            _, b = heapq.heappop(heap)
            if cnt[b] < caps[b]:
                break
        assign[i] = b
        cnt[b] += 1
        tot[b] += loads[i]
        if cnt[b] < caps[b]:
            heapq.heappush(heap, (int(tot[b]), b))
    return assign


def _host_prep(edge_index, n, ncores):
    """Node permutation + per-core gather index streams."""
    npc = n // ncores
    ngroup = -(-npc // P)
    last_cnt = npc - (ngroup - 1) * P
    nbuckets = -(-n // BUCKET)

    loop = np.arange(n, dtype=np.int64)
    src = np.concatenate([edge_index[0].astype(np.int64), loop])
    dst = np.concatenate([edge_index[1].astype(np.int64), loop])
    deg = np.bincount(dst, minlength=n)

    core_of = _lpt(deg, [npc] * ncores)
    group_of = np.empty(n, np.int32)
    slot_of = np.empty(n, np.int32)
    pos_of = np.empty(n, np.int64)
    order = np.empty(n, np.int64)
    caps = [P] * (ngroup - 1) + [last_cnt]
    for k in range(ncores):
        nodes_k = np.where(core_of == k)[0]
        g_assign = _lpt(deg[nodes_k], caps)
        o = np.argsort(g_assign, kind="stable")
        cnts = np.bincount(g_assign, minlength=ngroup)
        starts = np.concatenate([[0], np.cumsum(cnts)])[:-1]
        slot = np.empty(len(nodes_k), np.int64)
        slot[o] = np.arange(len(nodes_k)) - starts[g_assign[o]]
        group_of[nodes_k] = g_assign
        slot_of[nodes_k] = slot
        pos = k * npc + g_assign * P + slot
        pos_of[nodes_k] = pos
        order[pos] = nodes_k

    # per-(group,bucket) edge counts per core -> uniform chunk schedule
    e_core = core_of[dst]
    e_group = group_of[dst]
    e_bucket = pos_of[src] // BUCKET
    cnts = np.zeros((ncores, ngroup, nbuckets), np.int64)
    np.add.at(cnts, (e_core, e_group, e_bucket), 1)
    cbs_np = -(-cnts.max(axis=0) // P)       # [ngroup, nbuckets] chunks
    cbs = [[int(c) for c in row] for row in cbs_np]
    chg = np.array([sum(row) for row in cbs])
    cols = int(chg.sum())
    goff = np.concatenate([[0], np.cumsum(chg)])[:-1]
    boff = np.zeros((ngroup, nbuckets), np.int64)
    for g in range(ngroup):
        o = goff[g]
        for b in range(nbuckets):
            boff[g, b] = o
            o += cbs[g][b]

    per_core = []
    for k in range(ncores):
        mask = e_core == k
        es = pos_of[src[mask]]
        eg = e_group[mask]
        eb = e_bucket[mask]
        esl = slot_of[dst[mask]]
        o = np.lexsort((eb, eg))
        es, eg, eb, esl = es[o], eg[o], eb[o], esl[o]
        cnt_k = np.zeros((ngroup, nbuckets), np.int64)
        np.add.at(cnt_k, (eg, eb), 1)
        flat = cnt_k.reshape(-1)
        starts = np.concatenate([[0], np.cumsum(flat)])[:-1].reshape(
            ngroup, nbuckets)
        j = np.arange(len(es)) - starts[eg, eb]      # pos within (g,b)
        slotj = boff[eg, eb] * P + j                 # global slot in stream

        dstslot = np.full((P, cols), -1, np.int16)
        dstslot[slotj % P, slotj // P] = esl
        idx16 = np.zeros((16, cols * 8), np.int16)   # 8 int16 cols per chunk
        idx16[slotj % 16, slotj // 16] = es - eb * BUCKET
        per_core.append(dict(dstslot=dstslot, idx16=idx16))
    return dict(order=order, pos_of=pos_of, cbs=cbs, per_core=per_core)


def _host_weights(inputs, order, n):
    """Permuted/augmented weight + input tensors (all float32)."""
    f = np.float32
    x = np.asarray(inputs["x"], f)[order]            # perm rows
    batch = np.asarray(inputs["batch"])[order]
    xTa = np.ascontiguousarray(x.T)                  # [32, n]
    droneTa = np.concatenate(
        [np.asarray(inputs["drone_feat"], f).T, np.ones((1, G), f)], 0)
    # node bias folded into the drone-table bias row (every node gets both)
    droneWa = np.concatenate(
        [np.asarray(inputs["drone_W"], f).T,
         (np.asarray(inputs["drone_b"], f)
          + np.asarray(inputs["node_b"], f))[None]], 0)
    nodeWa = np.ascontiguousarray(np.asarray(inputs["node_W"], f).T)
    out = dict(xTa=xTa, batch=batch, droneTa=droneTa, droneWa=droneWa,
               nodeWa=nodeWa,
               outWT=np.ascontiguousarray(np.asarray(inputs["out_W"], f).T),
               outb=np.tile(np.asarray(inputs["out_b"], f), (P, 1)))
    for l in range(2):
        W = np.asarray(inputs[f"convW{l}"], f)       # [HC, CDIM]
        a_s = np.asarray(inputs[f"att_src{l}"], f)   # [H, CDIM]
        a_d = np.asarray(inputs[f"att_dst{l}"], f)
        Wh = W.reshape(H, CDIM, CDIM)
        Ws = np.einsum("hcf,hc->fh", Wh, a_s)        # [CDIM, H]
        Wd = np.einsum("hcf,hc->fh", Wh, a_d)
        out[f"wcomb{l}"] = np.concatenate([W.T, Ws, Wd], 1)   # [CDIM, 264]
        out[f"convb{l}"] = np.tile(np.asarray(inputs[f"convb{l}"], f), (P, 1))
        out[f"lng{l}"] = np.tile(np.asarray(inputs[f"ln_g{l}"], f), (P, 1))
        out[f"lnb{l}"] = np.tile(np.asarray(inputs[f"ln_b{l}"], f), (P, 1))
    return out


# --------------------------------------------------------------------------
# bass kernel
# --------------------------------------------------------------------------

def _build(cfg):
    import concourse.bass as bass
    import concourse.bacc as bacc
    import concourse.tile as tile
    from concourse import mybir
    from concourse.masks import make_identity

    f32 = mybir.dt.float32
    i32 = mybir.dt.int32
    i16 = mybir.dt.int16
    i8 = mybir.dt.int8
    bf16 = mybir.dt.bfloat16
    rdt = getattr(mybir.dt, cfg.rec_dt)
    is_bf = cfg.rec_dt != "float32"
    Alu = mybir.AluOpType
    Act = mybir.ActivationFunctionType

    n, npc, ngroup = cfg.n, cfg.npc, cfg.ngroup
    RECP, CHMAX = cfg.recp, cfg.chmax
    LAYF, LAYI, LAYB, F32SZ, I16SZ, TOTAL = _layout(cfg)

    nc = bacc.Bacc("TRN2", target_bir_lowering=False, debug=cfg.debug,
                   num_devices=cfg.ncores)

    blob_d = nc.dram_tensor("blob", [TOTAL], f32, kind="ExternalInput")

    def fview(nm):
        o, sh = LAYF[nm]
        return blob_d[o:o + sh[0] * sh[1]].rearrange("(a b) -> a b", a=sh[0])

    def iview(nm, dt):
        o, sh = LAYI[nm]
        sz = sh[0] * sh[1]
        o32 = F32SZ + o // 2                 # o is even by construction
        return blob_d[o32:o32 + (sz + 1) // 2].bitcast(dt)[
            0:sz].rearrange("(a b) -> a b", a=sh[0])

    def bview(nm, dt):
        o, sh = LAYB[nm]
        sz = sh[0] * sh[1]
        o32 = F32SZ + I16SZ // 2 + o // 4    # o is 4-aligned by construction
        return blob_d[o32:o32 + (sz + 3) // 4].bitcast(dt)[
            0:sz].rearrange("(a b) -> a b", a=sh[0])

    out_d = nc.dram_tensor("out", [npc, OUT_F], bf16, kind="ExternalOutput")

    rec_loc_d = nc.dram_tensor("rec_loc", [npc, RECP], rdt)
    rec_d = nc.dram_tensor("rec", [n, RECP], rdt,
                           addr_space="Shared" if cfg.ncores > 1 else "Local")
    had_d = [nc.dram_tensor(f"had{l}", [npc, CDIM + H], f32) for l in range(2)]
    stag_d = [nc.dram_tensor(f"stag{l}", [ngroup * P, CDIM], f32)
              for l in range(2)]
    idxrep_d = nc.dram_tensor("idxrep", [P, cfg.cols * 8], i16)
    dr_d = nc.dram_tensor("dr", [G, CDIM], f32)

    from contextlib import ExitStack
    with tile.TileContext(nc) as tc, ExitStack() as ctx:
        cpool = ctx.enter_context(tc.tile_pool(name="const", bufs=1))
        p1 = ctx.enter_context(tc.tile_pool(name="p1", bufs=2))
        p2 = ctx.enter_context(tc.tile_pool(name="p2", bufs=2))

        def cload(nm):
            o, sh = LAYF[nm]
            t = cpool.tile(list(sh), f32, tag=f"c_{nm}")
            nc.sync.dma_start(out=t[:], in_=fview(nm))
            return t

        droneTa_sb = cload("droneTa")
        droneWa_sb = cload("droneWa")
        nodeWa_sb = cload("nodeWa")
        outWT_sb = cload("outWT")
        nodeWb_sb = cpool.tile([NODE_F, CDIM], bf16, tag="nodeWb")
        nc.vector.tensor_copy(nodeWb_sb[:], nodeWa_sb[:])
        wcomb_sb = []
        for l in range(2):
            t = cpool.tile([CDIM, REC + H], bf16, tag=f"c_wcomb{l}")
            nc.sync.dma_start(out=t[:], in_=iview(f"wcomb{l}", bf16))
            wcomb_sb.append(t)

        # broadcast the bias/LN rows [1, 416] to all 128 partitions via a
        # ones-column matmul, then slice views
        SMW = 6 * CDIM + OUT_F
        smrow_sb = cload("smalls")           # [1, SMW]
        ones_sb = cpool.tile([1, P], f32, tag="ones1")
        nc.vector.memset(ones_sb[:], 1.0)
        smallsb = cpool.tile([P, SMW], f32, tag="smallsb")
        with tc.tile_pool(name="pssm", bufs=1, space="PSUM") as ppsm:
            psm = ppsm.tile([P, SMW], f32)
            nc.tensor.matmul(psm[:], lhsT=ones_sb[:], rhs=smrow_sb[:],
                             start=True, stop=True)
            nc.scalar.copy(smallsb[:], psm[:])
        convb_sb = [smallsb[:, 0:CDIM], smallsb[:, CDIM:2 * CDIM]]
        lng_sb = [smallsb[:, 2 * CDIM:3 * CDIM], smallsb[:, 3 * CDIM:4 * CDIM]]
        lnb_sb = [smallsb[:, 4 * CDIM:5 * CDIM], smallsb[:, 5 * CDIM:6 * CDIM]]
        outb_sb = smallsb[:, 6 * CDIM:6 * CDIM + OUT_F]

        # int8 streams: dst slots + batch ids (+ iota / gread)
        dst8_sb = cpool.tile([P, cfg.cols], i8, tag="dst8")
        nc.sync.dma_start(out=dst8_sb[:], in_=bview("dstslot", i8))
        dst16_sb = cpool.tile([P, cfg.cols], i16, tag="dst16")
        nc.vector.tensor_copy(dst16_sb[:], dst8_sb[:])
        bat8_sb = cpool.tile([P, ngroup], i8, tag="bat8")
        nc.sync.dma_start(out=bat8_sb[:], in_=bview("batch", i8))
        bat32_sb = cpool.tile([P, ngroup], i32, tag="bat32")
        nc.vector.tensor_copy(bat32_sb[:], bat8_sb[:])

        # replicate the 16-partition gather-index stream to 128 partitions
        # (dma_gather wants idxs wrapped in 16 partitions x 8 gpsimd cores)
        for k8 in range(8):
            nc.sync.dma_start(out=idxrep_d[k8 * 16:(k8 + 1) * 16, :],
                              in_=iview("idx16", i16))

        iota_sb = cpool.tile([P, P], i32)
        nc.gpsimd.iota(iota_sb[:], pattern=[[1, P]], base=0,
                       channel_multiplier=0)
        iota16_sb = cpool.tile([P, P], i16)
        nc.vector.tensor_copy(iota16_sb[:], iota_sb[:])
        ident_sb = cpool.tile([P, P], f32)
        make_identity(nc, ident_sb[:])
        identr_sb = ident_sb
        if is_bf:
            identr_sb = cpool.tile([P, P], rdt)
            nc.vector.tensor_copy(identr_sb[:], ident_sb[:])

        # gread[p, g] = min(g*128 + p, npc-1): offsets into local had_d
        gread_sb = cpool.tile([P, ngroup], i32, tag="gread")
        nc.gpsimd.iota(gread_sb[:], pattern=[[P, ngroup]], base=0,
                       channel_multiplier=1)
        nc.vector.tensor_scalar(gread_sb[:], gread_sb[:], npc - 1, None,
                                Alu.min)

        # dr = droneTa.T @ droneWa  -> dram (indirect-gather source)
        dr_sb = cpool.tile([G, CDIM], f32)
        with tc.tile_pool(name="psdr", bufs=1, space="PSUM") as ppdr:
            pdr_t = ppdr.tile([P, CDIM], f32)
            pdr = pdr_t[:G]
            nc.tensor.matmul(pdr, lhsT=droneTa_sb[:], rhs=droneWa_sb[:],
                             start=True, stop=True)
            nc.scalar.copy(dr_sb[:], pdr)
        nc.sync.dma_start(out=dr_d[:, :], in_=dr_sb[:])

        # ------------------------------------------------------------------
        def phase1(l):
            """Build rec_loc[npc, RECP] and had[npc, 68] tile by tile
            (own shard only; AllGather replicates rec afterwards)."""
            xT_v = iview("xTbf", bf16)
            with tc.tile_pool(name=f"ps1_{l}", bufs=2, space="PSUM") as pp:

                def do_batch(r0, tb, rows):
                    if l == 0:
                        xb = p1.tile([NODE_F, TB * P], bf16, tag="xb")
                        nc.sync.dma_start(out=xb[:, :rows],
                                          in_=xT_v[:, r0:r0 + rows])
                    hadb = p1.tile([P, TB, CDIM + H], f32, tag="hadb")
                    if l == 1:
                        if rows == tb * P:
                            nc.sync.dma_start(
                                out=hadb[:, :tb, :CDIM],
                                in_=stag_d[0][r0:r0 + rows, :].rearrange(
                                    "(c p) f -> p c f", p=P))
                        else:
                            nc.sync.dma_start(out=hadb[:rows, 0, :CDIM],
                                              in_=stag_d[0][r0:r0 + rows, :])
                    recb = p1.tile([P, TB, RECP], rdt, tag="recb")
                    nc.vector.memset(recb[:, :, REC:], 0.0)
                    for t in range(tb):
                        pr_ = min(P, rows - t * P)
                        g_abs = r0 // P + t
                        if l == 0:
                            drt = p1.tile([P, CDIM], f32, tag="drt")
                            nc.gpsimd.indirect_dma_start(
                                out=drt[:], out_offset=None, in_=dr_d[:],
                                in_offset=bass.IndirectOffsetOnAxis(
                                    ap=bat32_sb[:, g_abs:g_abs + 1], axis=0))
                            ph = pp.tile([P, CDIM], f32, tag="ph")
                            nc.tensor.matmul(ph[:pr_],
                                             lhsT=xb[:, t * P:t * P + pr_],
                                             rhs=nodeWb_sb[:], start=True,
                                             stop=True)
                            nc.vector.tensor_tensor(hadb[:pr_, t, :CDIM],
                                                    ph[:pr_], drt[:pr_],
                                                    Alu.add)
                        pt = pp.tile([CDIM, P], f32, tag="pt")
                        nc.tensor.transpose(pt[:, :pr_], hadb[:pr_, t, :CDIM],
                                            ident_sb[:pr_, :pr_])
                        hT = p1.tile([CDIM, P], bf16, tag="hT")
                        nc.scalar.copy(hT[:, :pr_], pt[:, :pr_])
                        prc = pp.tile([P, REC + H], f32, tag="pr")
                        nc.tensor.matmul(prc[:pr_], lhsT=hT[:, :pr_],
                                         rhs=wcomb_sb[l][:], start=True,
                                         stop=True)
                        nc.scalar.copy(recb[:pr_, t, 0:REC], prc[:pr_, 0:REC])
                        nc.vector.tensor_copy(hadb[:pr_, t, CDIM:],
                                              prc[:pr_, REC:REC + H])
                    if rows == tb * P:
                        nc.sync.dma_start(
                            out=rec_loc_d[r0:r0 + rows, :].rearrange(
                                "(c p) f -> p c f", p=P),
                            in_=recb[:, :tb, :])
                        nc.sync.dma_start(
                            out=had_d[l][r0:r0 + rows, :].rearrange(
                                "(c p) f -> p c f", p=P),
                            in_=hadb[:, :tb, :])
                    else:
                        nc.sync.dma_start(out=rec_loc_d[r0:r0 + rows, :],
                                          in_=recb[:rows, 0, :])
                        nc.sync.dma_start(out=had_d[l][r0:r0 + rows, :],
                                          in_=hadb[:rows, 0, :])

                for b0 in range(0, cfg.nt_full, TB):
                    tb = min(TB, cfg.nt_full - b0)
                    do_batch(b0 * P, tb, tb * P)
                if cfg.nt_rem:
                    do_batch(cfg.nt_full * P, 1, cfg.nt_rem)

        def gather_rec():
            nc.gpsimd.collective_compute(
                "AllGather", mybir.AluOpType.bypass,
                replica_groups=[list(range(cfg.ncores))],
                ins=[rec_loc_d[0:npc, :].opt()],
                outs=[rec_d[:, :].opt()])

        # ------------------------------------------------------------------
        def phase2(l):
            with tc.tile_pool(name=f"ps2_{l}", bufs=2, space="PSUM") as pp:
                col0 = 0
                for g in range(ngroup):
                    CH = cfg.chg[g]
                    rows_g = P if g < ngroup - 1 else cfg.last_cnt
                    idxt = p2.tile([P, CHMAX * 8], i16, tag="idxt")
                    nc.sync.dma_start(out=idxt[:, :CH * 8],
                                      in_=idxrep_d[:, col0 * 8:(col0 + CH) * 8])
                    rect = p2.tile([P, CHMAX, RECP], rdt, tag="rect")
                    c0 = 0
                    for b in range(cfg.nbuckets):
                        cb = cfg.cbs[g][b]
                        if cb == 0:
                            continue
                        nrows = min(BUCKET, n - b * BUCKET)
                        done = 0
                        while done < cb:   # HW envelope: <=256 idxs per call
                            st = min(2, cb - done)
                            nc.gpsimd.dma_gather(
                                rect[:, c0 + done:c0 + done + st, :],
                                rec_d[b * BUCKET:b * BUCKET + nrows, :],
                                idxt[:, (c0 + done) * 8:(c0 + done + st) * 8],
                                st * P, st * P, RECP)
                            done += st
                        c0 += cb
                    # h_old + a_dst rows for this group's nodes
                    hadt = p2.tile([P, CDIM + H], f32, tag="hadt")
                    nc.gpsimd.indirect_dma_start(
                        out=hadt[:], out_offset=None, in_=had_d[l][:],
                        in_offset=bass.IndirectOffsetOnAxis(
                            ap=gread_sb[:, g:g + 1], axis=0))
                    ad_rhs = hadt[:, CDIM:]
                    if is_bf:
                        adr = p2.tile([P, H], rdt, tag="adr")
                        nc.vector.tensor_copy(adr[:], hadt[:, CDIM:])
                        ad_rhs = adr[:]
                    # one-hot M[edge, dst_slot]
                    Mt = p2.tile([P, CHMAX, P], rdt, tag="Mt")
                    nc.vector.tensor_tensor(
                        Mt[:, :CH, :],
                        dst16_sb[:, col0:col0 + CH][:, :, None].to_broadcast(
                            [P, CH, P]),
                        iota16_sb[:, None, :].to_broadcast([P, CH, P]),
                        Alu.is_equal)
                    # e_d: broadcast a_dst scores to edges via M^T matmuls
                    ped = pp.tile([P, CHMAX * H], f32, tag="ped")
                    for c in range(CH):
                        pmt = pp.tile([P, P], rdt, tag="pmt")
                        nc.tensor.transpose(pmt[:], Mt[:, c, :], identr_sb[:])
                        mt_sb = p2.tile([P, P], rdt, tag="mt_sb")
                        nc.scalar.copy(mt_sb[:], pmt[:])
                        nc.tensor.matmul(ped[:, c * H:(c + 1) * H],
                                         lhsT=mt_sb[:], rhs=ad_rhs,
                                         start=True, stop=True)
                    # e = lrelu(as + ad); ex = exp(e) -> rec[..., 256:260]
                    et = p2.tile([P, CHMAX, H], f32, tag="et")
                    nc.vector.tensor_tensor(
                        et[:, :CH, :], rect[:, :CH, HC:REC],
                        ped[:, 0:CH * H].rearrange("p (c h) -> p c h", h=H),
                        Alu.add)
                    lt = p2.tile([P, CHMAX, H], f32, tag="lt")
                    nc.vector.tensor_scalar_mul(lt[:, :CH, :], et[:, :CH, :],
                                                NEG_SLOPE)
                    nc.vector.tensor_tensor(et[:, :CH, :], lt[:, :CH, :],
                                            et[:, :CH, :], Alu.max)
                    nc.scalar.activation(rect[:, :CH, HC:REC], et[:, :CH, :],
                                         Act.Exp)
                    # V = ex * xh (per head, in place)
                    for h_ in range(H):
                        nc.vector.tensor_tensor(
                            rect[:, :CH, h_ * CDIM:(h_ + 1) * CDIM],
                            rect[:, :CH, h_ * CDIM:(h_ + 1) * CDIM],
                            rect[:, :CH, HC + h_:HC + h_ + 1].to_broadcast(
                                [P, CH, CDIM]),
                            Alu.mult)
                    # contract over edges: psum[:, 0:256]=sum alpha*xh, [256:260]=s
                    pg = pp.tile([P, REC], f32, tag="pg")
                    for c in range(CH):
                        nc.tensor.matmul(pg[:], lhsT=Mt[:, c, :],
                                         rhs=rect[:, c, 0:REC],
                                         start=(c == 0), stop=(c == CH - 1))
                    # r = 1 / (s + eps) / H
                    s4 = p2.tile([P, H], f32, tag="s4")
                    nc.vector.tensor_scalar(s4[:], pg[:, HC:REC], 1e-16, None,
                                            Alu.add)
                    r4 = p2.tile([P, H], f32, tag="r4")
                    nc.vector.reciprocal(r4[:], s4[:])
                    nc.vector.tensor_scalar_mul(r4[:], r4[:], 1.0 / H)
                    # head mean
                    yt = p2.tile([P, CDIM], f32, tag="yt")
                    tmp = p2.tile([P, CDIM], f32, tag="tmp")
                    nc.vector.tensor_scalar(yt[:], pg[:, 0:CDIM], r4[:, 0:1],
                                            None, Alu.mult)
                    for h_ in range(1, H):
                        nc.vector.tensor_scalar(tmp[:],
                                                pg[:, h_ * CDIM:(h_ + 1) * CDIM],
                                                r4[:, h_:h_ + 1], None, Alu.mult)
                        nc.vector.tensor_add(yt[:], yt[:], tmp[:])
                    nc.vector.tensor_add(yt[:], yt[:], convb_sb[l])
                    # layernorm
                    mu = p2.tile([P, 1], f32, tag="mu")
                    nc.vector.tensor_reduce(mu[:], yt[:], mybir.AxisListType.X,
                                            Alu.add)
                    nc.vector.tensor_scalar_mul(mu[:], mu[:], 1.0 / CDIM)
                    nc.vector.tensor_scalar(yt[:], yt[:], mu[:, 0:1], None,
                                            Alu.subtract)
                    sq = p2.tile([P, CDIM], f32, tag="sq")
                    var = p2.tile([P, 1], f32, tag="var")
                    nc.scalar.activation(sq[:], yt[:], Act.Square,
                                         accum_out=var[:])
                    nc.vector.tensor_scalar(var[:], var[:], 1.0 / CDIM, LN_EPS,
                                            Alu.mult, Alu.add)
                    sd = p2.tile([P, 1], f32, tag="sd")
                    nc.scalar.sqrt(sd[:], var[:])
                    inv = p2.tile([P, 1], f32, tag="inv")
                    nc.vector.reciprocal(inv[:], sd[:])
                    nc.vector.tensor_scalar(yt[:], yt[:], inv[:, 0:1], None,
                                            Alu.mult)
                    nc.vector.tensor_mul(yt[:], yt[:], lng_sb[l])
                    nc.vector.tensor_add(yt[:], yt[:], lnb_sb[l])
                    nc.vector.tensor_scalar_max(yt[:], yt[:], 0.0)
                    # residual + contiguous staging write
                    nc.vector.tensor_add(yt[:], yt[:], hadt[:, 0:CDIM])
                    nc.sync.dma_start(out=stag_d[l][g * P:g * P + rows_g, :],
                                      in_=yt[:rows_g, :])
                    col0 += CH

        # ------------------------------------------------------------------
        phase1(0)
        gather_rec()
        phase2(0)
        phase1(1)
        gather_rec()
        phase2(1)

        # final projection over own rows
        with tc.tile_pool(name="psf", bufs=2, space="PSUM") as pp:
            for t0 in range(0, npc, P):
                wr = min(P, npc - t0)
                ht2 = p2.tile([P, CDIM], f32, tag="ht2")
                nc.sync.dma_start(out=ht2[:wr], in_=stag_d[1][t0:t0 + wr, :])
                pt2 = pp.tile([CDIM, P], f32, tag="pt2")
                nc.tensor.transpose(pt2[:, :wr], ht2[:wr], ident_sb[:wr, :wr])
                hT2 = p2.tile([CDIM, P], f32, tag="hT2")
                nc.scalar.copy(hT2[:, :wr], pt2[:, :wr])
                po = pp.tile([P, OUT_F], f32, tag="po")
                nc.tensor.matmul(po[:wr], lhsT=hT2[:, :wr], rhs=outWT_sb[:],
                                 start=True, stop=True)
                ot = p2.tile([P, OUT_F], bf16, tag="ot")
                nc.vector.tensor_add(ot[:wr], po[:wr], outb_sb[:wr])
                nc.sync.dma_start(out=out_d[t0:t0 + wr, :], in_=ot[:wr, :])

    nc.compile()
    return nc


# --------------------------------------------------------------------------
# dispatch (cached jitted shard_map; mirrors bass2jax.run_bass_via_pjrt)
# --------------------------------------------------------------------------

_DISPATCH_CACHE = {}


def _make_dispatch(nc, ncores):
    key = id(nc)
    if key in _DISPATCH_CACHE:
        return _DISPATCH_CACHE[key]

    _enable_jax_cc()
    import jax
    from jax.sharding import Mesh, PartitionSpec
    from jax.experimental.shard_map import shard_map
    from concourse import bass2jax, mybir

    bass2jax.install_neuronx_cc_hook()
    partition_name = (nc.partition_id_tensor.name
                      if nc.partition_id_tensor else None)
    in_names, out_names, out_avals, out_shapes = [], [], [], []
    for alloc in nc.m.functions[0].allocations:
        if not isinstance(alloc, mybir.MemoryLocationSet):
            continue
        name = alloc.memorylocations[0].name
        if alloc.kind == "ExternalInput":
            if name != partition_name:
                in_names.append(name)
        elif alloc.kind == "ExternalOutput":
            out_names.append(name)
            shape = tuple(alloc.tensor_shape)
            dtype = mybir.dt.np(alloc.dtype)
            out_avals.append(jax.core.ShapedArray(shape, dtype))
            out_shapes.append((shape, dtype))
    n_params = len(in_names)
    n_outs = len(out_avals)
    all_names = list(in_names)
    if partition_name is not None:
        all_names.append(partition_name)

    def _body(*args):
        operands = list(args)
        if partition_name is not None:
            operands.append(bass2jax.partition_id_tensor())
        outs = bass2jax._bass_exec_p.bind(
            *operands, out_avals=tuple(out_avals),
            in_names=tuple(all_names), out_names=tuple(out_names),
            lowering_input_output_aliases=(), sim_require_finite=True,
            sim_require_nnan=True, nc=nc)
        return tuple(outs)

    devices = jax.devices()[:ncores]
    mesh = Mesh(np.asarray(devices), ("core",))
    sharded = jax.jit(
        shard_map(_body, mesh=mesh,
                  in_specs=(PartitionSpec("core"),) * n_params,
                  out_specs=(PartitionSpec("core"),) * n_outs,
                  check_rep=False),
        keep_unused=True)

    import concurrent.futures as _cf
    _pool = _cf.ThreadPoolExecutor(ncores)

    def run(maps):
        concat_in = [np.concatenate([np.asarray(m[nm]) for m in maps], axis=0)
                     for nm in in_names]
        out_arrs = sharded(*concat_in)
        res = [dict() for _ in range(ncores)]
        for i, name in enumerate(out_names):
            rows = out_shapes[i][0][0]
            shards = out_arrs[i].addressable_shards
            datas = list(_pool.map(lambda s: np.asarray(s.data), shards))
            for s, d in zip(shards, datas):
                res[s.index[0].start // rows][name] = d
        return res

    _DISPATCH_CACHE[key] = run
    return run


# --------------------------------------------------------------------------
# entry point
# --------------------------------------------------------------------------

def _in_maps(cfg, prep, wts):
    """Pack per-core inputs into one f32 blob (with i16/bf16 sections)."""
    import ml_dtypes
    LAYF, LAYI, LAYB, F32SZ, I16SZ, TOTAL = _layout(cfg)
    npc = cfg.npc
    o16_base = 2 * F32SZ
    o8_base = 4 * F32SZ + 2 * I16SZ

    blob_shared = np.zeros(TOTAL, np.float32)
    for nm in ("nodeWa", "droneTa", "droneWa", "outWT"):
        o, sh = LAYF[nm]
        blob_shared[o:o + sh[0] * sh[1]] = np.asarray(
            wts[nm], np.float32).ravel()
    o, sh = LAYF["smalls"]
    smalls = np.concatenate([np.asarray(wts[nm], np.float32)[0]
                             for nm in ("convb0", "convb1", "lng0", "lng1",
                                        "lnb0", "lnb1", "outb")])
    blob_shared[o:o + sh[0] * sh[1]] = smalls

    b16s = blob_shared.view(np.int16)
    for l in range(2):
        o, sh = LAYI[f"wcomb{l}"]
        b16s[o16_base + o:o16_base + o + sh[0] * sh[1]] = np.asarray(
            wts[f"wcomb{l}"], np.float32).astype(
                ml_dtypes.bfloat16).view(np.int16).ravel()

    batch = np.asarray(wts["batch"]).astype(np.int8)
    maps = []
    for k in range(cfg.ncores):
        blob = blob_shared.copy()
        b16 = blob.view(np.int16)
        b8 = blob.view(np.int8)
        pc = prep["per_core"][k]

        def put16(nm, data16):
            o, sh = LAYI[nm]
            sz = sh[0] * sh[1]
            b16[o16_base + o:o16_base + o + sz] = data16.ravel()

        def put8(nm, data8):
            o, sh = LAYB[nm]
            sz = sh[0] * sh[1]
            b8[o8_base + o:o8_base + o + sz] = data8.ravel()

        put16("idx16", pc["idx16"])
        xbf = np.ascontiguousarray(
            wts["xTa"][:, k * npc:(k + 1) * npc]).astype(
                ml_dtypes.bfloat16).view(np.int16)
        put16("xTbf", xbf)
        put8("dstslot", pc["dstslot"].astype(np.int8))
        bp = np.zeros(cfg.ngroup * P, np.int8)
        bp[:npc] = batch[k * npc:(k + 1) * npc]
        put8("batch", np.ascontiguousarray(bp.reshape(cfg.ngroup, P).T))
        maps.append(dict(blob=blob))
    return maps


def kernel(**inputs):
    edge_index = np.asarray(inputs["edge_index"])
    prep = _host_prep(edge_index, N, NCORES)
    cfg = _Cfg(N, NCORES, prep["cbs"])
    wts = _host_weights(inputs, prep["order"], N)
    nc = _build(cfg)
    maps = _in_maps(cfg, prep, wts)

    run = _make_dispatch(nc, NCORES)
    res = run(maps)
    out = np.empty((N, OUT_F), np.float32)
    for k in range(NCORES):
        out[prep["order"][k * cfg.npc:(k + 1) * cfg.npc]] = \
            res[k]["out"].astype(np.float32)
    return out


# revision 23
# speedup vs baseline: 35.1291x; 1.0298x over previous
"""GAT (2-layer, 4-head, segment-softmax) message-passing kernel for 8 Trainium2
NeuronCores.

Strategy (dst-sharded, edge aggregation as one-hot matmuls):
  * Nodes are assigned to cores/groups with degree-balanced packing (LPT). The
    node permutation is (core, group, slot) order, so each core owns a
    contiguous block of rows and each group's 128 nodes are contiguous.
  * Phase 1 is SHARDED: each core computes the record table
    rec[n] = [xh(256) | a_src-score(4) | pad] only for its own npc rows, plus
    had[n] = [h(64) | ad(4)]; an 8-core AllGather replicates rec on-device
    (NeuronLink) so phase 2 can gather any source node's record locally.
  * For each destination group (128 nodes), the core gathers the records of
    the group's in-edges' source nodes with gpsimd dma_gather (int16 indices,
    source-bucketed in 32768-row windows), builds the one-hot incidence matrix
    M[edge, dst_slot] on the vector engine (iota compare), broadcasts the
    a_dst scores to edges via transposed-one-hot matmuls, and reduces both the
    softmax denominators and the weighted feature sums with PSUM-accumulated
    matmuls (contracting over edges). Softmax normalization is applied after
    the reduction - mathematically identical to the reference's segment
    softmax (max-subtraction is a no-op at these magnitudes).
  * Host->device traffic is minimized (the axon tunnel is ~65 MB/s with ~75ms
    per-array overhead): ALL inputs are packed into ONE f32 blob per core
    (~2.2MB: f32 weights; int16 gather indices / dst slots / batch ids and
    the bf16 own-shard xT are stored via bitcast views). The drone-feature
    term is an on-device indirect gather of the 64x64 projected table; gread
    offsets are iota-generated on device; the output is returned as bf16.
  * Dispatch uses a cached jitted shard_map executable (compiled once per
    process) plus the JAX persistent compilation cache, so steady-state
    dispatch cost is input upload + execute + output download.
"""

import os
import sys

sys.path.insert(0, "/opt/trn_rl_repo")

import numpy as np

# ---- problem constants (hardcoded; kernel.py must be self-contained) ----
N = 100000
E = 1600000
G = 64
H = 4
CDIM = 64
NODE_F = 32
DRONE_F = 16
OUT_F = 32
LN_EPS = 1e-5
NEG_SLOPE = 0.2
NCORES = 8
P = 128
HC = H * CDIM          # 256
REC = HC + H           # 260: [V(256) | as/ex(4)]
BUCKET = 32768         # int16 index range per dma_gather bucket
TB = 6                 # phase-1 tile batch

REC_DT_NAME = os.environ.get("GAT_REC_DT", "bfloat16")


def _enable_jax_cc():
    import jax
    try:
        jax.config.update("jax_compilation_cache_dir",
                          os.environ.get("JAX_CC_DIR", "/tmp/jax_cc_cache"))
        jax.config.update("jax_persistent_cache_min_entry_size_bytes", 0)
        jax.config.update("jax_persistent_cache_min_compile_time_secs", 0.0)
    except Exception:
        pass


class _Cfg:
    def __init__(self, n, ncores, cbs, rec_dt=REC_DT_NAME, debug=False):
        assert n % ncores == 0
        self.n = n
        self.ncores = ncores
        self.npc = n // ncores
        self.ngroup = -(-self.npc // P)
        self.chg = cbs["chg"]                # chunks per group
        self.pbase = cbs["pbase"]            # per-group per-pair window bases
        self.chmax = max(self.chg)
        self.cols = sum(self.chg)            # total chunk columns
        self.rec_dt = rec_dt
        self.recp = 320 if rec_dt == "float32" else 384  # padded record elems
        self.debug = debug
        # own-shard tiling (phase 1 + final projection)
        self.nt_full, self.nt_rem = divmod(self.npc, P)
        self.last_cnt = self.npc - (self.ngroup - 1) * P


def _layout(cfg):
    """Single-blob layout. Returns (f32 sections, i16 sections, total f32
    elems). i16 section offsets are in int16 units from the start of the
    int16 region, which begins at f32 elem F32SZ (i16 elem 2*F32SZ)."""
    f32 = {}
    off = 0
    for nm, sh in [("nodeWa", (NODE_F, CDIM)),
                   ("droneTa", (DRONE_F + 1, G)),
                   ("droneWa", (DRONE_F + 1, CDIM)),
                   ("outWT", (CDIM, OUT_F)),
                   # convb0|convb1|lng0|lng1|lnb0|lnb1|outb rows
                   ("smalls", (1, 6 * CDIM + OUT_F))]:
        f32[nm] = (off, sh)
        off += sh[0] * sh[1]
    f32sz = off
    i16 = {}
    off = 0
    for nm, sh in [("idx16", (16, cfg.cols * 8)),
                   ("wcomb0", (CDIM, REC + H)),   # bf16 bits
                   ("wcomb1", (CDIM, REC + H)),   # bf16 bits
                   ("xTbf", (NODE_F, cfg.npc))]:
        sz = sh[0] * sh[1]
        i16[nm] = (off, sh)
        off += sz + (sz & 1)                 # keep 32-bit alignment
    i16sz = off
    i8 = {}
    off = 0
    for nm, sh in [("dstslot", (P, cfg.cols)),
                   ("batch", (P, cfg.ngroup))]:
        sz = sh[0] * sh[1]
        i8[nm] = (off, sh)
        off += sz + (-sz) % 4                # keep 32-bit alignment
    total = f32sz + i16sz // 2 + off // 4
    return f32, i16, i8, f32sz, i16sz, total


# --------------------------------------------------------------------------
# host-side preprocessing
# --------------------------------------------------------------------------

def _lpt(loads, caps):
    """LPT packing into len(caps) bins with given item capacities, balancing
    total load. Returns assignment array."""
    import heapq

    nbins = len(caps)
    order = np.argsort(-loads, kind="stable")
    heap = [(0, b) for b in range(nbins)]
    heapq.heapify(heap)
    cnt = np.zeros(nbins, np.int64)
    tot = np.zeros(nbins, np.int64)
    assign = np.empty(len(loads), np.int32)
    for i in order:
        while True:
            _, b = heapq.heappop(heap)
            if cnt[b] < caps[b]:
                break
        assign[i] = b
        cnt[b] += 1
        tot[b] += loads[i]
        if cnt[b] < caps[b]:
            heapq.heappush(heap, (int(tot[b]), b))
    return assign


def _host_prep(edge_index, n, ncores):
    """Node permutation + per-core gather index streams."""
    npc = n // ncores
    ngroup = -(-npc // P)
    last_cnt = npc - (ngroup - 1) * P
    loop = np.arange(n, dtype=np.int64)
    src = np.concatenate([edge_index[0].astype(np.int64), loop])
    dst = np.concatenate([edge_index[1].astype(np.int64), loop])
    deg = np.bincount(dst, minlength=n)

    core_of = _lpt(deg, [npc] * ncores)
    group_of = np.empty(n, np.int32)
    slot_of = np.empty(n, np.int32)
    pos_of = np.empty(n, np.int64)
    order = np.empty(n, np.int64)
    caps = [P] * (ngroup - 1) + [last_cnt]
    for k in range(ncores):
        nodes_k = np.where(core_of == k)[0]
        g_assign = _lpt(deg[nodes_k], caps)
        o = np.argsort(g_assign, kind="stable")
        cnts = np.bincount(g_assign, minlength=ngroup)
        starts = np.concatenate([[0], np.cumsum(cnts)])[:-1]
        slot = np.empty(len(nodes_k), np.int64)
        slot[o] = np.arange(len(nodes_k)) - starts[g_assign[o]]
        group_of[nodes_k] = g_assign
        slot_of[nodes_k] = slot
        pos = k * npc + g_assign * P + slot
        pos_of[nodes_k] = pos
        order[pos] = nodes_k

    # shared chunk schedule: per-core edges sorted by (group, src pos);
    # chunk = 128 consecutive sorted edges, chunk PAIRS share a 32768-row
    # gather window whose base is the min src pos over all cores (LPT makes
    # per-core group quantiles nearly identical, so the shared window holds
    # every core's pair span with huge margin - asserted below).
    e_core = core_of[dst]
    e_group = group_of[dst]
    cnts = np.zeros((ncores, ngroup), np.int64)
    np.add.at(cnts, (e_core, e_group), 1)
    chg = [int(c) for c in -(-cnts.max(axis=0) // P)]   # chunks per group
    cols = int(sum(chg))
    goff = np.concatenate([[0], np.cumsum(chg)])[:-1]
    npair = [-(-c // 2) for c in chg]
    poff = np.concatenate([[0], np.cumsum(npair)])[:-1]
    tpairs = int(sum(npair))

    pmin = np.full(tpairs, np.iinfo(np.int64).max)
    pmax = np.full(tpairs, -1, np.int64)
    streams = []
    for k in range(ncores):
        mask = e_core == k
        es = pos_of[src[mask]]
        eg = e_group[mask]
        esl = slot_of[dst[mask]]
        o = np.lexsort((es, eg))
        es, eg, esl = es[o], eg[o], esl[o]
        cnt_k = np.bincount(eg, minlength=ngroup)
        starts = np.concatenate([[0], np.cumsum(cnt_k)])[:-1]
        r = np.arange(len(es)) - starts[eg]          # rank within group
        pr = poff[eg] + (r // P) // 2                # global pair id
        np.minimum.at(pmin, pr, es)
        np.maximum.at(pmax, pr, es)
        streams.append((es, eg, esl, r, pr))

    base = np.where(pmin <= pmax, pmin, 0)
    span = pmax - base
    assert span.max() < BUCKET, f"gather window overflow: {span.max()}"
    pbase = [[int(base[poff[g] + j]) for j in range(npair[g])]
             for g in range(ngroup)]

    per_core = []
    for k in range(ncores):
        es, eg, esl, r, pr = streams[k]
        slotj = goff[eg] * P + r                     # global slot in stream
        dstslot = np.full((P, cols), -1, np.int16)
        dstslot[slotj % P, slotj // P] = esl
        idx16 = np.zeros((16, cols * 8), np.int16)   # 8 int16 cols per chunk
        idx16[slotj % 16, slotj // 16] = es - base[pr]
        per_core.append(dict(dstslot=dstslot, idx16=idx16))
    return dict(order=order, pos_of=pos_of,
                cbs=dict(chg=chg, pbase=pbase), per_core=per_core)


def _host_weights(inputs, order, n):
    """Permuted/augmented weight + input tensors (all float32)."""
    f = np.float32
    x = np.asarray(inputs["x"], f)[order]            # perm rows
    batch = np.asarray(inputs["batch"])[order]
    xTa = np.ascontiguousarray(x.T)                  # [32, n]
    droneTa = np.concatenate(
        [np.asarray(inputs["drone_feat"], f).T, np.ones((1, G), f)], 0)
    # node bias folded into the drone-table bias row (every node gets both)
    droneWa = np.concatenate(
        [np.asarray(inputs["drone_W"], f).T,
         (np.asarray(inputs["drone_b"], f)
          + np.asarray(inputs["node_b"], f))[None]], 0)
    nodeWa = np.ascontiguousarray(np.asarray(inputs["node_W"], f).T)
    out = dict(xTa=xTa, batch=batch, droneTa=droneTa, droneWa=droneWa,
               nodeWa=nodeWa,
               outWT=np.ascontiguousarray(np.asarray(inputs["out_W"], f).T),
               outb=np.tile(np.asarray(inputs["out_b"], f), (P, 1)))
    for l in range(2):
        W = np.asarray(inputs[f"convW{l}"], f)       # [HC, CDIM]
        a_s = np.asarray(inputs[f"att_src{l}"], f)   # [H, CDIM]
        a_d = np.asarray(inputs[f"att_dst{l}"], f)
        Wh = W.reshape(H, CDIM, CDIM)
        Ws = np.einsum("hcf,hc->fh", Wh, a_s)        # [CDIM, H]
        Wd = np.einsum("hcf,hc->fh", Wh, a_d)
        out[f"wcomb{l}"] = np.concatenate([W.T, Ws, Wd], 1)   # [CDIM, 264]
        out[f"convb{l}"] = np.tile(np.asarray(inputs[f"convb{l}"], f), (P, 1))
        out[f"lng{l}"] = np.tile(np.asarray(inputs[f"ln_g{l}"], f), (P, 1))
        out[f"lnb{l}"] = np.tile(np.asarray(inputs[f"ln_b{l}"], f), (P, 1))
    return out


# --------------------------------------------------------------------------
# bass kernel
# --------------------------------------------------------------------------

def _build(cfg):
    import concourse.bass as bass
    import concourse.bacc as bacc
    import concourse.tile as tile
    from concourse import mybir
    from concourse.masks import make_identity

    f32 = mybir.dt.float32
    i32 = mybir.dt.int32
    i16 = mybir.dt.int16
    i8 = mybir.dt.int8
    bf16 = mybir.dt.bfloat16
    rdt = getattr(mybir.dt, cfg.rec_dt)
    is_bf = cfg.rec_dt != "float32"
    Alu = mybir.AluOpType
    Act = mybir.ActivationFunctionType

    n, npc, ngroup = cfg.n, cfg.npc, cfg.ngroup
    RECP, CHMAX = cfg.recp, cfg.chmax
    LAYF, LAYI, LAYB, F32SZ, I16SZ, TOTAL = _layout(cfg)

    nc = bacc.Bacc("TRN2", target_bir_lowering=False, debug=cfg.debug,
                   num_devices=cfg.ncores)

    blob_d = nc.dram_tensor("blob", [TOTAL], f32, kind="ExternalInput")

    def fview(nm):
        o, sh = LAYF[nm]
        return blob_d[o:o + sh[0] * sh[1]].rearrange("(a b) -> a b", a=sh[0])

    def iview(nm, dt):
        o, sh = LAYI[nm]
        sz = sh[0] * sh[1]
        o32 = F32SZ + o // 2                 # o is even by construction
        return blob_d[o32:o32 + (sz + 1) // 2].bitcast(dt)[
            0:sz].rearrange("(a b) -> a b", a=sh[0])

    def bview(nm, dt):
        o, sh = LAYB[nm]
        sz = sh[0] * sh[1]
        o32 = F32SZ + I16SZ // 2 + o // 4    # o is 4-aligned by construction
        return blob_d[o32:o32 + (sz + 3) // 4].bitcast(dt)[
            0:sz].rearrange("(a b) -> a b", a=sh[0])

    out_d = nc.dram_tensor("out", [npc, OUT_F], bf16, kind="ExternalOutput")

    rec_loc_d = nc.dram_tensor("rec_loc", [npc, RECP], rdt)
    rec_d = nc.dram_tensor("rec", [n, RECP], rdt,
                           addr_space="Shared" if cfg.ncores > 1 else "Local")
    had_d = [nc.dram_tensor(f"had{l}", [npc, CDIM + H], f32) for l in range(2)]
    stag_d = [nc.dram_tensor(f"stag{l}", [ngroup * P, CDIM], f32)
              for l in range(2)]
    idxrep_d = nc.dram_tensor("idxrep", [P, cfg.cols * 8], i16)
    dr_d = nc.dram_tensor("dr", [G, CDIM], f32)

    from contextlib import ExitStack
    with tile.TileContext(nc) as tc, ExitStack() as ctx:
        cpool = ctx.enter_context(tc.tile_pool(name="const", bufs=1))
        p1 = ctx.enter_context(tc.tile_pool(name="p1", bufs=2))
        p2 = ctx.enter_context(tc.tile_pool(name="p2", bufs=2))

        def cload(nm):
            o, sh = LAYF[nm]
            t = cpool.tile(list(sh), f32, tag=f"c_{nm}")
            nc.sync.dma_start(out=t[:], in_=fview(nm))
            return t

        droneTa_sb = cload("droneTa")
        droneWa_sb = cload("droneWa")
        nodeWa_sb = cload("nodeWa")
        outWT_sb = cload("outWT")
        nodeWb_sb = cpool.tile([NODE_F, CDIM], bf16, tag="nodeWb")
        nc.vector.tensor_copy(nodeWb_sb[:], nodeWa_sb[:])
        wcomb_sb = []
        for l in range(2):
            t = cpool.tile([CDIM, REC + H], bf16, tag=f"c_wcomb{l}")
            nc.sync.dma_start(out=t[:], in_=iview(f"wcomb{l}", bf16))
            wcomb_sb.append(t)

        # broadcast the bias/LN rows [1, 416] to all 128 partitions via a
        # ones-column matmul, then slice views
        SMW = 6 * CDIM + OUT_F
        smrow_sb = cload("smalls")           # [1, SMW]
        ones_sb = cpool.tile([1, P], f32, tag="ones1")
        nc.vector.memset(ones_sb[:], 1.0)
        smallsb = cpool.tile([P, SMW], f32, tag="smallsb")
        with tc.tile_pool(name="pssm", bufs=1, space="PSUM") as ppsm:
            psm = ppsm.tile([P, SMW], f32)
            nc.tensor.matmul(psm[:], lhsT=ones_sb[:], rhs=smrow_sb[:],
                             start=True, stop=True)
            nc.scalar.copy(smallsb[:], psm[:])
        convb_sb = [smallsb[:, 0:CDIM], smallsb[:, CDIM:2 * CDIM]]
        lng_sb = [smallsb[:, 2 * CDIM:3 * CDIM], smallsb[:, 3 * CDIM:4 * CDIM]]
        lnb_sb = [smallsb[:, 4 * CDIM:5 * CDIM], smallsb[:, 5 * CDIM:6 * CDIM]]
        outb_sb = smallsb[:, 6 * CDIM:6 * CDIM + OUT_F]

        # int8 streams: dst slots + batch ids (+ iota / gread)
        dst8_sb = cpool.tile([P, cfg.cols], i8, tag="dst8")
        nc.sync.dma_start(out=dst8_sb[:], in_=bview("dstslot", i8))
        dst16_sb = cpool.tile([P, cfg.cols], i16, tag="dst16")
        nc.vector.tensor_copy(dst16_sb[:], dst8_sb[:])
        bat8_sb = cpool.tile([P, ngroup], i8, tag="bat8")
        nc.sync.dma_start(out=bat8_sb[:], in_=bview("batch", i8))
        bat32_sb = cpool.tile([P, ngroup], i32, tag="bat32")
        nc.vector.tensor_copy(bat32_sb[:], bat8_sb[:])

        # replicate the 16-partition gather-index stream to 128 partitions
        # (dma_gather wants idxs wrapped in 16 partitions x 8 gpsimd cores)
        for k8 in range(8):
            nc.sync.dma_start(out=idxrep_d[k8 * 16:(k8 + 1) * 16, :],
                              in_=iview("idx16", i16))

        iota_sb = cpool.tile([P, P], i32)
        nc.gpsimd.iota(iota_sb[:], pattern=[[1, P]], base=0,
                       channel_multiplier=0)
        iota16_sb = cpool.tile([P, P], i16)
        nc.vector.tensor_copy(iota16_sb[:], iota_sb[:])
        ident_sb = cpool.tile([P, P], f32)
        make_identity(nc, ident_sb[:])
        identr_sb = ident_sb
        if is_bf:
            identr_sb = cpool.tile([P, P], rdt)
            nc.vector.tensor_copy(identr_sb[:], ident_sb[:])

        # gread[p, g] = min(g*128 + p, npc-1): offsets into local had_d
        gread_sb = cpool.tile([P, ngroup], i32, tag="gread")
        nc.gpsimd.iota(gread_sb[:], pattern=[[P, ngroup]], base=0,
                       channel_multiplier=1)
        nc.vector.tensor_scalar(gread_sb[:], gread_sb[:], npc - 1, None,
                                Alu.min)

        # dr = droneTa.T @ droneWa  -> dram (indirect-gather source)
        dr_sb = cpool.tile([G, CDIM], f32)
        with tc.tile_pool(name="psdr", bufs=1, space="PSUM") as ppdr:
            pdr_t = ppdr.tile([P, CDIM], f32)
            pdr = pdr_t[:G]
            nc.tensor.matmul(pdr, lhsT=droneTa_sb[:], rhs=droneWa_sb[:],
                             start=True, stop=True)
            nc.scalar.copy(dr_sb[:], pdr)
        nc.sync.dma_start(out=dr_d[:, :], in_=dr_sb[:])

        # ------------------------------------------------------------------
        def phase1(l):
            """Build rec_loc[npc, RECP] and had[npc, 68] tile by tile
            (own shard only; AllGather replicates rec afterwards)."""
            xT_v = iview("xTbf", bf16)
            with tc.tile_pool(name=f"ps1_{l}", bufs=2, space="PSUM") as pp:

                def do_batch(r0, tb, rows):
                    if l == 0:
                        xb = p1.tile([NODE_F, TB * P], bf16, tag="xb")
                        nc.sync.dma_start(out=xb[:, :rows],
                                          in_=xT_v[:, r0:r0 + rows])
                    hadb = p1.tile([P, TB, CDIM + H], f32, tag="hadb")
                    if l == 1:
                        if rows == tb * P:
                            nc.sync.dma_start(
                                out=hadb[:, :tb, :CDIM],
                                in_=stag_d[0][r0:r0 + rows, :].rearrange(
                                    "(c p) f -> p c f", p=P))
                        else:
                            nc.sync.dma_start(out=hadb[:rows, 0, :CDIM],
                                              in_=stag_d[0][r0:r0 + rows, :])
                    recb = p1.tile([P, TB, RECP], rdt, tag="recb")
                    nc.vector.memset(recb[:, :, REC:], 0.0)
                    for t in range(tb):
                        pr_ = min(P, rows - t * P)
                        g_abs = r0 // P + t
                        if l == 0:
                            drt = p1.tile([P, CDIM], f32, tag="drt")
                            nc.gpsimd.indirect_dma_start(
                                out=drt[:], out_offset=None, in_=dr_d[:],
                                in_offset=bass.IndirectOffsetOnAxis(
                                    ap=bat32_sb[:, g_abs:g_abs + 1], axis=0))
                            ph = pp.tile([P, CDIM], f32, tag="ph")
                            nc.tensor.matmul(ph[:pr_],
                                             lhsT=xb[:, t * P:t * P + pr_],
                                             rhs=nodeWb_sb[:], start=True,
                                             stop=True)
                            nc.vector.tensor_tensor(hadb[:pr_, t, :CDIM],
                                                    ph[:pr_], drt[:pr_],
                                                    Alu.add)
                        pt = pp.tile([CDIM, P], f32, tag="pt")
                        nc.tensor.transpose(pt[:, :pr_], hadb[:pr_, t, :CDIM],
                                            ident_sb[:pr_, :pr_])
                        hT = p1.tile([CDIM, P], bf16, tag="hT")
                        nc.scalar.copy(hT[:, :pr_], pt[:, :pr_])
                        prc = pp.tile([P, REC + H], f32, tag="pr")
                        nc.tensor.matmul(prc[:pr_], lhsT=hT[:, :pr_],
                                         rhs=wcomb_sb[l][:], start=True,
                                         stop=True)
                        nc.scalar.copy(recb[:pr_, t, 0:REC], prc[:pr_, 0:REC])
                        nc.vector.tensor_copy(hadb[:pr_, t, CDIM:],
                                              prc[:pr_, REC:REC + H])
                    if rows == tb * P:
                        nc.sync.dma_start(
                            out=rec_loc_d[r0:r0 + rows, :].rearrange(
                                "(c p) f -> p c f", p=P),
                            in_=recb[:, :tb, :])
                        nc.sync.dma_start(
                            out=had_d[l][r0:r0 + rows, :].rearrange(
                                "(c p) f -> p c f", p=P),
                            in_=hadb[:, :tb, :])
                    else:
                        nc.sync.dma_start(out=rec_loc_d[r0:r0 + rows, :],
                                          in_=recb[:rows, 0, :])
                        nc.sync.dma_start(out=had_d[l][r0:r0 + rows, :],
                                          in_=hadb[:rows, 0, :])

                for b0 in range(0, cfg.nt_full, TB):
                    tb = min(TB, cfg.nt_full - b0)
                    do_batch(b0 * P, tb, tb * P)
                if cfg.nt_rem:
                    do_batch(cfg.nt_full * P, 1, cfg.nt_rem)

        def gather_rec():
            nc.gpsimd.collective_compute(
                "AllGather", mybir.AluOpType.bypass,
                replica_groups=[list(range(cfg.ncores))],
                ins=[rec_loc_d[0:npc, :].opt()],
                outs=[rec_d[:, :].opt()])

        # ------------------------------------------------------------------
        def phase2(l):
            with tc.tile_pool(name=f"ps2_{l}", bufs=2, space="PSUM") as pp:
                col0 = 0
                for g in range(ngroup):
                    CH = cfg.chg[g]
                    rows_g = P if g < ngroup - 1 else cfg.last_cnt
                    idxt = p2.tile([P, CHMAX * 8], i16, tag="idxt")
                    nc.sync.dma_start(out=idxt[:, :CH * 8],
                                      in_=idxrep_d[:, col0 * 8:(col0 + CH) * 8])
                    rect = p2.tile([P, CHMAX, RECP], rdt, tag="rect")
                    done = 0
                    for pi in range(-(-CH // 2)):
                        st = min(2, CH - done)  # <=256 idxs per call (HW)
                        base = cfg.pbase[g][pi]
                        nrows = min(BUCKET, n - base)
                        nc.gpsimd.dma_gather(
                            rect[:, done:done + st, :],
                            rec_d[base:base + nrows, :],
                            idxt[:, done * 8:(done + st) * 8],
                            st * P, st * P, RECP)
                        done += st
                    # h_old + a_dst rows for this group's nodes
                    hadt = p2.tile([P, CDIM + H], f32, tag="hadt")
                    nc.gpsimd.indirect_dma_start(
                        out=hadt[:], out_offset=None, in_=had_d[l][:],
                        in_offset=bass.IndirectOffsetOnAxis(
                            ap=gread_sb[:, g:g + 1], axis=0))
                    ad_rhs = hadt[:, CDIM:]
                    if is_bf:
                        adr = p2.tile([P, H], rdt, tag="adr")
                        nc.vector.tensor_copy(adr[:], hadt[:, CDIM:])
                        ad_rhs = adr[:]
                    # one-hot M[edge, dst_slot]
                    Mt = p2.tile([P, CHMAX, P], rdt, tag="Mt")
                    nc.vector.tensor_tensor(
                        Mt[:, :CH, :],
                        dst16_sb[:, col0:col0 + CH][:, :, None].to_broadcast(
                            [P, CH, P]),
                        iota16_sb[:, None, :].to_broadcast([P, CH, P]),
                        Alu.is_equal)
                    # e_d: broadcast a_dst scores to edges via M^T matmuls
                    ped = pp.tile([P, CHMAX * H], f32, tag="ped")
                    for c in range(CH):
                        pmt = pp.tile([P, P], rdt, tag="pmt")
                        nc.tensor.transpose(pmt[:], Mt[:, c, :], identr_sb[:])
                        mt_sb = p2.tile([P, P], rdt, tag="mt_sb")
                        nc.scalar.copy(mt_sb[:], pmt[:])
                        nc.tensor.matmul(ped[:, c * H:(c + 1) * H],
                                         lhsT=mt_sb[:], rhs=ad_rhs,
                                         start=True, stop=True)
                    # e = lrelu(as + ad); ex = exp(e) -> rec[..., 256:260]
                    et = p2.tile([P, CHMAX, H], f32, tag="et")
                    nc.vector.tensor_tensor(
                        et[:, :CH, :], rect[:, :CH, HC:REC],
                        ped[:, 0:CH * H].rearrange("p (c h) -> p c h", h=H),
                        Alu.add)
                    lt = p2.tile([P, CHMAX, H], f32, tag="lt")
                    nc.vector.tensor_scalar_mul(lt[:, :CH, :], et[:, :CH, :],
                                                NEG_SLOPE)
                    nc.vector.tensor_tensor(et[:, :CH, :], lt[:, :CH, :],
                                            et[:, :CH, :], Alu.max)
                    nc.scalar.activation(rect[:, :CH, HC:REC], et[:, :CH, :],
                                         Act.Exp)
                    # V = ex * xh (per head, in place)
                    for h_ in range(H):
                        nc.vector.tensor_tensor(
                            rect[:, :CH, h_ * CDIM:(h_ + 1) * CDIM],
                            rect[:, :CH, h_ * CDIM:(h_ + 1) * CDIM],
                            rect[:, :CH, HC + h_:HC + h_ + 1].to_broadcast(
                                [P, CH, CDIM]),
                            Alu.mult)
                    # contract over edges: psum[:, 0:256]=sum alpha*xh, [256:260]=s
                    pg = pp.tile([P, REC], f32, tag="pg")
                    for c in range(CH):
                        nc.tensor.matmul(pg[:], lhsT=Mt[:, c, :],
                                         rhs=rect[:, c, 0:REC],
                                         start=(c == 0), stop=(c == CH - 1))
                    # r = 1 / (s + eps) / H
                    s4 = p2.tile([P, H], f32, tag="s4")
                    nc.vector.tensor_scalar(s4[:], pg[:, HC:REC], 1e-16, None,
                                            Alu.add)
                    r4 = p2.tile([P, H], f32, tag="r4")
                    nc.vector.reciprocal(r4[:], s4[:])
                    nc.vector.tensor_scalar_mul(r4[:], r4[:], 1.0 / H)
                    # head mean
                    yt = p2.tile([P, CDIM], f32, tag="yt")
                    tmp = p2.tile([P, CDIM], f32, tag="tmp")
                    nc.vector.tensor_scalar(yt[:], pg[:, 0:CDIM], r4[:, 0:1],
                                            None, Alu.mult)
                    for h_ in range(1, H):
                        nc.vector.tensor_scalar(tmp[:],
                                                pg[:, h_ * CDIM:(h_ + 1) * CDIM],
                                                r4[:, h_:h_ + 1], None, Alu.mult)
                        nc.vector.tensor_add(yt[:], yt[:], tmp[:])
                    nc.vector.tensor_add(yt[:], yt[:], convb_sb[l])
                    # layernorm
                    mu = p2.tile([P, 1], f32, tag="mu")
                    nc.vector.tensor_reduce(mu[:], yt[:], mybir.AxisListType.X,
                                            Alu.add)
                    nc.vector.tensor_scalar_mul(mu[:], mu[:], 1.0 / CDIM)
                    nc.vector.tensor_scalar(yt[:], yt[:], mu[:, 0:1], None,
                                            Alu.subtract)
                    sq = p2.tile([P, CDIM], f32, tag="sq")
                    var = p2.tile([P, 1], f32, tag="var")
                    nc.scalar.activation(sq[:], yt[:], Act.Square,
                                         accum_out=var[:])
                    nc.vector.tensor_scalar(var[:], var[:], 1.0 / CDIM, LN_EPS,
                                            Alu.mult, Alu.add)
                    sd = p2.tile([P, 1], f32, tag="sd")
                    nc.scalar.sqrt(sd[:], var[:])
                    inv = p2.tile([P, 1], f32, tag="inv")
                    nc.vector.reciprocal(inv[:], sd[:])
                    nc.vector.tensor_scalar(yt[:], yt[:], inv[:, 0:1], None,
                                            Alu.mult)
                    nc.vector.tensor_mul(yt[:], yt[:], lng_sb[l])
                    nc.vector.tensor_add(yt[:], yt[:], lnb_sb[l])
                    nc.vector.tensor_scalar_max(yt[:], yt[:], 0.0)
                    # residual + contiguous staging write
                    nc.vector.tensor_add(yt[:], yt[:], hadt[:, 0:CDIM])
                    nc.sync.dma_start(out=stag_d[l][g * P:g * P + rows_g, :],
                                      in_=yt[:rows_g, :])
                    col0 += CH

        # ------------------------------------------------------------------
        phase1(0)
        gather_rec()
        phase2(0)
        phase1(1)
        gather_rec()
        phase2(1)

        # final projection over own rows
        with tc.tile_pool(name="psf", bufs=2, space="PSUM") as pp:
            for t0 in range(0, npc, P):
                wr = min(P, npc - t0)
                ht2 = p2.tile([P, CDIM], f32, tag="ht2")
                nc.sync.dma_start(out=ht2[:wr], in_=stag_d[1][t0:t0 + wr, :])
                pt2 = pp.tile([CDIM, P], f32, tag="pt2")
                nc.tensor.transpose(pt2[:, :wr], ht2[:wr], ident_sb[:wr, :wr])
                hT2 = p2.tile([CDIM, P], f32, tag="hT2")
                nc.scalar.copy(hT2[:, :wr], pt2[:, :wr])
                po = pp.tile([P, OUT_F], f32, tag="po")
                nc.tensor.matmul(po[:wr], lhsT=hT2[:, :wr], rhs=outWT_sb[:],
                                 start=True, stop=True)
                ot = p2.tile([P, OUT_F], bf16, tag="ot")
                nc.vector.tensor_add(ot[:wr], po[:wr], outb_sb[:wr])
                nc.sync.dma_start(out=out_d[t0:t0 + wr, :], in_=ot[:wr, :])

    nc.compile()
    return nc


# --------------------------------------------------------------------------
# dispatch (cached jitted shard_map; mirrors bass2jax.run_bass_via_pjrt)
# --------------------------------------------------------------------------

_DISPATCH_CACHE = {}


def _make_dispatch(nc, ncores):
    key = id(nc)
    if key in _DISPATCH_CACHE:
        return _DISPATCH_CACHE[key]

    _enable_jax_cc()
    import jax
    from jax.sharding import Mesh, PartitionSpec
    from jax.experimental.shard_map import shard_map
    from concourse import bass2jax, mybir

    bass2jax.install_neuronx_cc_hook()
    partition_name = (nc.partition_id_tensor.name
                      if nc.partition_id_tensor else None)
    in_names, out_names, out_avals, out_shapes = [], [], [], []
    for alloc in nc.m.functions[0].allocations:
        if not isinstance(alloc, mybir.MemoryLocationSet):
            continue
        name = alloc.memorylocations[0].name
        if alloc.kind == "ExternalInput":
            if name != partition_name:
                in_names.append(name)
        elif alloc.kind == "ExternalOutput":
            out_names.append(name)
            shape = tuple(alloc.tensor_shape)
            dtype = mybir.dt.np(alloc.dtype)
            out_avals.append(jax.core.ShapedArray(shape, dtype))
            out_shapes.append((shape, dtype))
    n_params = len(in_names)
    n_outs = len(out_avals)
    all_names = list(in_names)
    if partition_name is not None:
        all_names.append(partition_name)

    def _body(*args):
        operands = list(args)
        if partition_name is not None:
            operands.append(bass2jax.partition_id_tensor())
        outs = bass2jax._bass_exec_p.bind(
            *operands, out_avals=tuple(out_avals),
            in_names=tuple(all_names), out_names=tuple(out_names),
            lowering_input_output_aliases=(), sim_require_finite=True,
            sim_require_nnan=True, nc=nc)
        return tuple(outs)

    devices = jax.devices()[:ncores]
    mesh = Mesh(np.asarray(devices), ("core",))
    sharded = jax.jit(
        shard_map(_body, mesh=mesh,
                  in_specs=(PartitionSpec("core"),) * n_params,
                  out_specs=(PartitionSpec("core"),) * n_outs,
                  check_rep=False),
        keep_unused=True)

    import concurrent.futures as _cf
    _pool = _cf.ThreadPoolExecutor(ncores)

    def run(maps):
        concat_in = [np.concatenate([np.asarray(m[nm]) for m in maps], axis=0)
                     for nm in in_names]
        out_arrs = sharded(*concat_in)
        res = [dict() for _ in range(ncores)]
        for i, name in enumerate(out_names):
            rows = out_shapes[i][0][0]
            shards = out_arrs[i].addressable_shards
            datas = list(_pool.map(lambda s: np.asarray(s.data), shards))
            for s, d in zip(shards, datas):
                res[s.index[0].start // rows][name] = d
        return res

    _DISPATCH_CACHE[key] = run
    return run


# --------------------------------------------------------------------------
# entry point
# --------------------------------------------------------------------------

def _in_maps(cfg, prep, wts):
    """Pack per-core inputs into one f32 blob (with i16/bf16 sections)."""
    import ml_dtypes
    LAYF, LAYI, LAYB, F32SZ, I16SZ, TOTAL = _layout(cfg)
    npc = cfg.npc
    o16_base = 2 * F32SZ
    o8_base = 4 * F32SZ + 2 * I16SZ

    blob_shared = np.zeros(TOTAL, np.float32)
    for nm in ("nodeWa", "droneTa", "droneWa", "outWT"):
        o, sh = LAYF[nm]
        blob_shared[o:o + sh[0] * sh[1]] = np.asarray(
            wts[nm], np.float32).ravel()
    o, sh = LAYF["smalls"]
    smalls = np.concatenate([np.asarray(wts[nm], np.float32)[0]
                             for nm in ("convb0", "convb1", "lng0", "lng1",
                                        "lnb0", "lnb1", "outb")])
    blob_shared[o:o + sh[0] * sh[1]] = smalls

    b16s = blob_shared.view(np.int16)
    for l in range(2):
        o, sh = LAYI[f"wcomb{l}"]
        b16s[o16_base + o:o16_base + o + sh[0] * sh[1]] = np.asarray(
            wts[f"wcomb{l}"], np.float32).astype(
                ml_dtypes.bfloat16).view(np.int16).ravel()

    batch = np.asarray(wts["batch"]).astype(np.int8)
    maps = []
    for k in range(cfg.ncores):
        blob = blob_shared.copy()
        b16 = blob.view(np.int16)
        b8 = blob.view(np.int8)
        pc = prep["per_core"][k]

        def put16(nm, data16):
            o, sh = LAYI[nm]
            sz = sh[0] * sh[1]
            b16[o16_base + o:o16_base + o + sz] = data16.ravel()

        def put8(nm, data8):
            o, sh = LAYB[nm]
            sz = sh[0] * sh[1]
            b8[o8_base + o:o8_base + o + sz] = data8.ravel()

        put16("idx16", pc["idx16"])
        xbf = np.ascontiguousarray(
            wts["xTa"][:, k * npc:(k + 1) * npc]).astype(
                ml_dtypes.bfloat16).view(np.int16)
        put16("xTbf", xbf)
        put8("dstslot", pc["dstslot"].astype(np.int8))
        bp = np.zeros(cfg.ngroup * P, np.int8)
        bp[:npc] = batch[k * npc:(k + 1) * npc]
        put8("batch", np.ascontiguousarray(bp.reshape(cfg.ngroup, P).T))
        maps.append(dict(blob=blob))
    return maps


def kernel(**inputs):
    edge_index = np.asarray(inputs["edge_index"])
    prep = _host_prep(edge_index, N, NCORES)
    cfg = _Cfg(N, NCORES, prep["cbs"])
    wts = _host_weights(inputs, prep["order"], N)
    nc = _build(cfg)
    maps = _in_maps(cfg, prep, wts)

    run = _make_dispatch(nc, NCORES)
    res = run(maps)
    out = np.empty((N, OUT_F), np.float32)
    for k in range(NCORES):
        out[prep["order"][k * cfg.npc:(k + 1) * cfg.npc]] = \
            res[k]["out"].astype(np.float32)
    return out
